# revision 5
# baseline (speedup 1.0000x reference)
"""Trainium2 Bass kernel for DecoderAttnRNN (LSTM + attention decoder).

Sharding: hybrid over 8 cores = 4 batch-groups x 2 vocab-halves.
Each core handles 16 batches and 16000 vocab columns:
  phase 0: gather embeddings (indirect DMA), transpose, precompute x@W_ih.T+bias
  phase 1: 72-step LSTM recurrence in transposed layout (features on
           partitions, batch on free dim)
  phase 2: attention for all (b,t) at once (ctx does not feed the recurrence)
  phase 3: logits = Z @ W_lin^T + b_lin as (1152 x 512) @ (512 x 16000) bf16;
           half of W_lin^T is preloaded to SBUF during phases 0-2, the rest
           streams; outputs written straight to DRAM.
"""

import numpy as np
import ml_dtypes

import concourse.bass as bass
import concourse.mybir as mybir
import concourse.tile as tile
from concourse import bacc
from concourse.bass_utils import run_bass_kernel_spmd
from concourse.masks import make_identity

B, T, S, E, H, V = 64, 72, 72, 128, 256, 32000
NCORES = 8
NBG = 4                   # batch groups
NVH = 2                   # vocab halves
BL = B // NBG             # 16 batches per core
BT = BL * T               # 1152 (t-major: flat index = t*BL + b)
VL = V // NVH             # 16000 vocab cols per core
G4H = 4 * H               # 1024
NCH = G4H // 128          # 8 gate chunks of 128
NC_N = 500                # psum n-chunk for phase 3 (4 per group)
NGC = 4 * NC_N            # 2000 cols per n-group
NGN = VL // NGC           # 8 groups
NMT = BT // 128           # 9 m-tiles, exact

f32 = mybir.dt.float32
bf16 = mybir.dt.bfloat16
i32 = mybir.dt.int32

_CACHE = {}


def _build():
    nc = bacc.Bacc(None, target_bir_lowering=False)

    tok_d = nc.declare_dram_parameter("tok", [BT, 1], i32, isOutput=False)
    emb_d = nc.declare_dram_parameter("emb", [V, E], f32, isOutput=False)
    enc_d = nc.declare_dram_parameter("enc", [S, BL, H], bf16, isOutput=False)
    encT_d = nc.declare_dram_parameter("encT", [2, 128, BL, S], bf16, isOutput=False)
    h0T_d = nc.declare_dram_parameter("h0T", [128, 2, BL], f32, isOutput=False)
    c0T_d = nc.declare_dram_parameter("c0T", [128, 2, BL], f32, isOutput=False)
    lens_d = nc.declare_dram_parameter("lens", [BL], i32, isOutput=False)
    biasT_d = nc.declare_dram_parameter("biasT", [128, NCH], f32, isOutput=False)
    wihT_d = nc.declare_dram_parameter("wihT", [E, G4H], bf16, isOutput=False)
    whhT_d = nc.declare_dram_parameter("whhT", [2, 128, G4H], bf16, isOutput=False)
    wlinT_d = nc.declare_dram_parameter("wlinT", [4, 128, VL], bf16, isOutput=False)
    out_d = nc.declare_dram_parameter("logits", [BT, VL], bf16, isOutput=True)

    with tile.TileContext(nc) as tc:
        with tc.tile_pool(name="persist", bufs=1) as pp:
            # ---- persistent tiles ----
            wih_sb = pp.tile([128, G4H], bf16)
            nc.gpsimd.dma_start(out=wih_sb[:], in_=wihT_d[:])
            whh_sb = pp.tile([128, 2, G4H], bf16)
            for k in range(2):
                nc.gpsimd.dma_start(out=whh_sb[:, k, :], in_=whhT_d[k])
            biasT_sb = pp.tile([128, NCH], f32)
            nc.gpsimd.dma_start(out=biasT_sb[:], in_=biasT_d[:])
            encT_sb = pp.tile([128, 2, BL, S], bf16)
            for k in range(2):
                nc.gpsimd.dma_start(out=encT_sb[:, k], in_=encT_d[k])
            enc_sb = pp.tile([S, BL, H], bf16)
            nc.gpsimd.dma_start(out=enc_sb[:], in_=enc_d[:])
            # preload ALL of W_lin^T to SBUF (128 KiB/partition); the loads
            # overlap phases 0-2 so phase 3 streams nothing in
            wpre = pp.tile([128, 4, VL], bf16)
            for k in range(4):
                eng = nc.sync if k % 2 == 0 else nc.scalar
                eng.dma_start(out=wpre[:, k, :], in_=wlinT_d[k])

            xwT = pp.tile([128, T, NCH, BL], bf16)     # x@W_ih.T + bias, t-major
            z01 = pp.tile([128, 2, BL, T], bf16)       # h features (k-tiles 0,1)
            z23 = pp.tile([128, 2, BL, T], bf16)       # ctx features (k-tiles 2,3)
            x_allT = pp.tile([128, BT], bf16)          # embeddings^T, (t,b) cols

            cT = pp.tile([128, 2, BL], f32)
            nc.gpsimd.dma_start(out=cT[:], in_=c0T_d[:])
            h0f = pp.tile([128, 2, BL], f32)
            nc.gpsimd.dma_start(out=h0f[:], in_=h0T_d[:])
            h_init = pp.tile([128, 2, BL], bf16)
            nc.vector.tensor_copy(out=h_init[:], in_=h0f[:])

            ident = pp.tile([128, 128], f32)
            make_identity(nc, ident[:])
            ones_col = pp.tile([S, 1], bf16)
            nc.vector.memset(ones_col[:], 1.0)
            ones_row_f = pp.tile([1, 128], f32)
            nc.vector.memset(ones_row_f[:], 1.0)

            # ---- attention mask: mask01[s, b] = 1.0 if s < len_b else 0.0 ----
            lens_i = pp.tile([S, BL], i32)
            lens_bcast = bass.AP(tensor=lens_d, offset=0, ap=[[0, S], [1, BL]])
            nc.gpsimd.dma_start(out=lens_i[:], in_=lens_bcast)
            lens_f = pp.tile([S, BL], f32)
            nc.vector.tensor_copy(out=lens_f[:], in_=lens_i[:])
            iota_i = pp.tile([S, 1], i32)
            nc.gpsimd.iota(iota_i[:], [[1, 1]], base=0, channel_multiplier=1)
            iota_f = pp.tile([S, 1], f32)
            nc.vector.tensor_copy(out=iota_f[:], in_=iota_i[:])
            mask01 = pp.tile([S, BL], f32)
            nc.vector.tensor_scalar(
                out=mask01[:], in0=lens_f[:], scalar1=iota_f[:], scalar2=None,
                op0=mybir.AluOpType.is_gt,
            )
            # additive mask for exp-bias: 0 where valid, -30000 where masked
            mask_neg = pp.tile([S, BL], f32)
            nc.vector.tensor_scalar(
                out=mask_neg[:], in0=mask01[:], scalar1=30000.0, scalar2=30000.0,
                op0=mybir.AluOpType.mult, op1=mybir.AluOpType.subtract,
            )

            # ---- phase 0: embedding gather + transpose ----
            with (
                tc.tile_pool(name="p0", bufs=2) as wp,
                tc.tile_pool(name="p0ps", bufs=2, space="PSUM") as psp,
            ):
                for j in range(NMT):
                    tok_t = wp.tile([128, 1], i32, tag="tok")
                    nc.gpsimd.dma_start(
                        out=tok_t[:], in_=tok_d[j * 128 : (j + 1) * 128]
                    )
                    x_t = wp.tile([128, E], f32, tag="x")
                    nc.gpsimd.indirect_dma_start(
                        out=x_t[:],
                        out_offset=None,
                        in_=emb_d[:],
                        in_offset=bass.IndirectOffsetOnAxis(ap=tok_t[:, :1], axis=0),
                    )
                    ps_t = psp.tile([128, 128], f32, tag="pst")
                    nc.tensor.transpose(
                        out=ps_t[:], in_=x_t[:], identity=ident[:]
                    )
                    nc.vector.tensor_copy(
                        out=x_allT[:, j * 128 : (j + 1) * 128], in_=ps_t[:]
                    )

                # xW precompute: xwT[t, c, b] = (x @ W_ih.T)[tb, c*128:...] + bias
                for c in range(NCH):
                    ps_xw = psp.tile([128, BT], f32, tag="psxw")
                    for n0, nn in [(0, 512), (512, 512), (1024, BT - 1024)]:
                        nc.tensor.matmul(
                            ps_xw[:, n0 : n0 + nn],
                            wih_sb[:, c * 128 : (c + 1) * 128],
                            x_allT[:, n0 : n0 + nn],
                            start=True,
                            stop=True,
                        )
                    nc.vector.tensor_scalar(
                        out=xwT[:, :, c, :],
                        in0=ps_xw[:].rearrange("p (t b) -> p t b", b=BL),
                        scalar1=biasT_sb[:, c : c + 1],
                        scalar2=None,
                        op0=mybir.AluOpType.add,
                    )

            # ---- phase 1: LSTM recurrence ----
            # gate order is host-permuted to (i, f, o, g):
            # chunks 0-1=i, 2-3=f, 4-5=o, 6-7=g
            with (
                tc.tile_pool(name="p1", bufs=3) as gp,
                tc.tile_pool(name="p1ps", bufs=2, space="PSUM") as psg,
            ):
                for t in range(T):
                    ps_g = psg.tile([128, NCH, BL], f32, tag="psg")
                    for c in range(NCH):
                        for k in range(2):
                            rhs = (
                                h_init[:, k, :]
                                if t == 0
                                else z01[:, k, :, t - 1]
                            )
                            nc.tensor.matmul(
                                ps_g[:, c, :],
                                whh_sb[:, k, c * 128 : (c + 1) * 128],
                                rhs,
                                start=(k == 0),
                                stop=(k == 1),
                            )
                    gates = gp.tile([128, NCH, BL], f32, tag="gates")
                    nc.vector.tensor_tensor(
                        out=gates[:], in0=ps_g[:], in1=xwT[:, t],
                        op=mybir.AluOpType.add,
                    )
                    nc.scalar.activation(
                        out=gates[:, 6:8], in_=gates[:, 6:8],
                        func=mybir.ActivationFunctionType.Tanh,
                    )
                    nc.scalar.activation(
                        out=gates[:, 0:6], in_=gates[:, 0:6],
                        func=mybir.ActivationFunctionType.Sigmoid,
                    )
                    # c = sig(f)*c + sig(i)*tanh(g)
                    nc.vector.tensor_tensor(
                        out=cT[:], in0=gates[:, 2:4], in1=cT[:],
                        op=mybir.AluOpType.mult,
                    )
                    ig = gp.tile([128, 2, BL], f32, tag="ig")
                    nc.vector.tensor_tensor(
                        out=ig[:], in0=gates[:, 0:2], in1=gates[:, 6:8],
                        op=mybir.AluOpType.mult,
                    )
                    nc.vector.tensor_tensor(
                        out=cT[:], in0=cT[:], in1=ig[:], op=mybir.AluOpType.add
                    )
                    th = gp.tile([128, 2, BL], f32, tag="th")
                    nc.scalar.activation(
                        out=th[:], in_=cT[:], func=mybir.ActivationFunctionType.Tanh
                    )
                    # h = sig(o) * tanh(c)  -> straight into Z (bf16)
                    nc.vector.tensor_tensor(
                        out=z01[:, :, :, t], in0=gates[:, 4:6], in1=th[:],
                        op=mybir.AluOpType.mult,
                    )

            # ---- phase 2: attention over all timesteps ----
            with (
                tc.tile_pool(name="p2", bufs=2) as ap,
                tc.tile_pool(name="p2ps", bufs=2, space="PSUM") as ps2,
            ):
                expsc = pp.tile([S, BL, T], bf16)
                for b in range(BL):
                    ps_s = ps2.tile([S, T], f32, tag="ps_s")
                    for k in range(2):
                        nc.tensor.matmul(
                            ps_s[:],
                            encT_sb[:, k, b, :],
                            z01[:, k, b, :],
                            start=(k == 0),
                            stop=(k == 1),
                        )
                    # exp with additive mask folded into the activation bias
                    nc.scalar.activation(
                        out=expsc[:, b, :], in_=ps_s[:],
                        func=mybir.ActivationFunctionType.Exp,
                        scale=float(1.0 / np.sqrt(H)),
                        bias=mask_neg[:, b : b + 1],
                    )
                    ps_d = ps2.tile([1, T], f32, tag="ps_d")
                    nc.tensor.matmul(
                        ps_d[:], ones_col[:], expsc[:, b, :], start=True, stop=True
                    )
                    recip = ap.tile([1, T], f32, tag="recip")
                    nc.vector.reciprocal(out=recip[:], in_=ps_d[:])
                    ps_bc = ps2.tile([128, T], f32, tag="ps_bc")
                    nc.tensor.matmul(
                        ps_bc[:], ones_row_f[:], recip[:], start=True, stop=True
                    )
                    bc_sb = ap.tile([128, T], f32, tag="bc")
                    nc.vector.tensor_copy(out=bc_sb[:], in_=ps_bc[:])
                    for j in range(2):
                        ps_c = ps2.tile([128, T], f32, tag="ps_c")
                        nc.tensor.matmul(
                            ps_c[:],
                            enc_sb[:, b, j * 128 : (j + 1) * 128],
                            expsc[:, b, :],
                            start=True,
                            stop=True,
                        )
                        nc.vector.tensor_tensor(
                            out=z23[:, j, b, :], in0=ps_c[:], in1=bc_sb[:],
                            op=mybir.AluOpType.mult,
                        )

            # ---- phase 3: logits = Z @ W_lin^T + b_lin ----
            zt = [
                z01[:, 0].rearrange("p b t -> p (b t)"),
                z01[:, 1].rearrange("p b t -> p (b t)"),
                z23[:, 0].rearrange("p b t -> p (b t)"),
                z23[:, 1].rearrange("p b t -> p (b t)"),
            ]
            with (
                tc.tile_pool(name="p3rhs", bufs=3) as rp,
                tc.tile_pool(name="p3out", bufs=3) as op_,
                tc.tile_pool(name="p3bl", bufs=2) as blp,
                tc.tile_pool(name="p3ps", bufs=2, space="PSUM") as ps3,
            ):
                for ng in range(NGN):
                    n0 = ng * NGC
                    rhs_t = rp.tile([128, 2, NGC], bf16, tag="rhs")
                    for k in range(2):
                        nc.sync.dma_start(
                            out=rhs_t[:, k, :],
                            in_=wlinT_d[2 + k][:, n0 : n0 + NGC],
                        )
                    # b_lin broadcast to all partitions via stride-0 DMA
                    bl_sb = blp.tile([128, NGC], bf16, tag="blsb")
                    bl_bcast = bass.AP(
                        tensor=blin_d, offset=n0, ap=[[0, 128], [1, NGC]]
                    )
                    nc.scalar.dma_start(out=bl_sb[:], in_=bl_bcast)
                    for mi in range(NMT):
                        m0 = mi * 128
                        ps_o = ps3.tile([128, 4, 512], f32, tag="po")
                        for k in range(4):
                            for n in range(4):
                                rhs = (
                                    wpre[:, k, n0 + n * NC_N : n0 + (n + 1) * NC_N]
                                    if k < 2
                                    else rhs_t[:, k - 2, n * NC_N : (n + 1) * NC_N]
                                )
                                nc.tensor.matmul(
                                    ps_o[:, n, :NC_N],
                                    zt[k][:, m0 : m0 + 128],
                                    rhs,
                                    start=(k == 0),
                                    stop=(k == 3),
                                )
                        o_sb = op_.tile([128, NGC], f32, tag="osb")
                        nc.vector.tensor_tensor(
                            out=o_sb[:].rearrange("p (g n) -> p g n", g=4),
                            in0=ps_o[:, :, :NC_N],
                            in1=bl_sb[:].rearrange("p (g n) -> p g n", g=4),
                            op=mybir.AluOpType.add,
                        )
                        eng = nc.gpsimd if (mi % 2 == 0) else nc.scalar
                        eng.dma_start(
                            out=out_d[m0 : m0 + 128, n0 : n0 + NGC],
                            in_=o_sb[:],
                        )
    nc.compile()
    return nc


def _prep_inputs(inputs):
    bf = ml_dtypes.bfloat16
    target = np.asarray(inputs["target_tensor"])
    enc = np.asarray(inputs["encoder_outputs"], dtype=np.float32)
    lens = np.asarray(inputs["encoder_seq_lens"])
    h0 = np.asarray(inputs["h0"], dtype=np.float32)
    c0 = np.asarray(inputs["c0"], dtype=np.float32)
    emb = np.ascontiguousarray(np.asarray(inputs["emb"], dtype=np.float32))
    W_ih = np.asarray(inputs["W_ih"], dtype=np.float32)
    W_hh = np.asarray(inputs["W_hh"], dtype=np.float32)
    bias = (
        np.asarray(inputs["b_ih"], dtype=np.float32)
        + np.asarray(inputs["b_hh"], dtype=np.float32)
    )
    # permute gate order (i, f, g, o) -> (i, f, o, g) so the device can run
    # one sigmoid over the first 6 chunks and one tanh over the last 2
    perm = np.concatenate(
        [np.arange(0, 2 * H), np.arange(3 * H, 4 * H), np.arange(2 * H, 3 * H)]
    )
    W_ih = W_ih[perm]
    W_hh = W_hh[perm]
    bias = bias[perm]
    W_lin = np.asarray(inputs["W_lin"], dtype=np.float32)
    b_lin = np.asarray(inputs["b_lin"], dtype=np.float32)

    wihT = np.ascontiguousarray(W_ih.T.astype(bf))                # (E, 4H)
    whhT = np.ascontiguousarray(
        W_hh.T.reshape(2, 128, G4H).astype(bf)
    )                                                             # (2,128,4H)
    biasT = np.ascontiguousarray(bias.reshape(NCH, 128).T)        # (128, NCH)
    wlinT_full = W_lin.T.astype(bf)                               # (512, V)

    in_maps = []
    for i in range(NCORES):
        bg = i % NBG
        vh = i // NBG
        sl = slice(bg * BL, (bg + 1) * BL)
        vsl = slice(vh * VL, (vh + 1) * VL)
        tok = np.ascontiguousarray(
            target[sl].T.reshape(BT, 1).astype(np.int32)
        )  # t-major
        enc_i = enc[sl]                                           # (BL, S, H)
        enc_sbh = np.ascontiguousarray(
            enc_i.transpose(1, 0, 2).astype(bf)
        )                                                         # (S, BL, H)
        encT = np.ascontiguousarray(
            enc_i.transpose(2, 0, 1).reshape(2, 128, BL, S).astype(bf)
        )                                                         # (2,128,BL,S)
        h0T = np.ascontiguousarray(h0[sl].T.reshape(2, 128, BL).transpose(1, 0, 2))
        c0T = np.ascontiguousarray(c0[sl].T.reshape(2, 128, BL).transpose(1, 0, 2))
        wlinT = np.ascontiguousarray(
            wlinT_full[:, vsl].reshape(4, 128, VL)
        )                                                         # (4,128,VL)
        blin = np.ascontiguousarray(b_lin[vsl].reshape(1, VL).astype(bf))
        in_maps.append(
            {
                "tok": tok,
                "emb": emb,
                "enc": enc_sbh,
                "encT": encT,
                "h0T": h0T,
                "c0T": c0T,
                "lens": np.ascontiguousarray(lens[sl].astype(np.int32)),
                "biasT": biasT,
                "wihT": wihT,
                "whhT": whhT,
                "wlinT": wlinT,
                "blin": blin,
            }
        )
    return in_maps


LAST_RESULTS = None


def _install_ntff_shim():
    """Provide antenv.axon_hooks if the image's antenv lacks it, so
    trace=True/BASS_TRACE=1 can capture NTFF profiles under axon."""
    import sys
    import types

    try:
        from antenv.axon_hooks import get_axon_ntff_profile_hook  # noqa: F401

        return
    except ImportError:
        pass
    try:
        from trn_agent_boot.trn_boot import _ntff_profile_via_ctypes

        hook = _ntff_profile_via_ctypes("/opt/axon/libaxon_pjrt.so")
        m = types.ModuleType("antenv.axon_hooks")
        m.get_axon_ntff_profile_hook = lambda: hook
        m.set_axon_ntff_profile_hook = lambda h: None
        sys.modules["antenv.axon_hooks"] = m
    except Exception:
        pass


def kernel(**inputs):
    global LAST_RESULTS
    _install_ntff_shim()
    if "nc" not in _CACHE:
        _CACHE["nc"] = _build()
    nc = _CACHE["nc"]
    in_maps = _prep_inputs(inputs)
    res = run_bass_kernel_spmd(nc, in_maps, core_ids=list(range(NCORES)))
    LAST_RESULTS = res
    out = np.empty((B, T, V), dtype=np.float32)
    for i in range(NCORES):
        bg = i % NBG
        vh = i // NBG
        # logits rows = Z columns = b*T + t (b-major)
        out[bg * BL : (bg + 1) * BL, :, vh * VL : (vh + 1) * VL] = (
            res.results[i]["logits"].reshape(BL, T, VL)
        )
    return out



# revision 21
# speedup vs baseline: 1.1122x; 1.1122x over previous
"""Trainium2 Bass kernel for DecoderAttnRNN (LSTM + attention decoder).

Sharding: hybrid over 8 cores = 4 batch-groups x 2 vocab-halves.
Each core handles 16 batches and 16000 vocab columns:
  phase 0: gather embeddings (indirect DMA), transpose, precompute x@W_ih.T+bias
  phase 1: 72-step LSTM recurrence in transposed layout (features on
           partitions, batch on free dim)
  phase 2: attention for all (b,t) at once (ctx does not feed the recurrence)
  phase 3: logits = Z @ W_lin^T as (1152 x 512) @ (512 x 16000) bf16; all of
           W_lin^T is preloaded to SBUF during phases 0-2; outputs written
           bf16 to DRAM, upcast + b_lin added on the host.
"""

import numpy as np
import ml_dtypes

import concourse.bass as bass
import concourse.mybir as mybir
import concourse.tile as tile
from concourse import bacc
from concourse.bass_utils import run_bass_kernel_spmd
from concourse.masks import make_identity

B, T, S, E, H, V = 64, 72, 72, 128, 256, 32000
NCORES = 8
NBG = 4                   # batch groups
NVH = 2                   # vocab halves
BL = B // NBG             # 16 batches per core
BT = BL * T               # 1152 (t-major: flat index = t*BL + b)
VL = V // NVH             # 16000 vocab cols per core
G4H = 4 * H               # 1024
NCH = G4H // 128          # 8 gate chunks of 128
NC_N = 500                # psum n-chunk for phase 3 (4 per group)
NGC = 4 * NC_N            # 2000 cols per n-group
NGN = VL // NGC           # 8 groups
NMT = BT // 128           # 9 m-tiles, exact

f32 = mybir.dt.float32
bf16 = mybir.dt.bfloat16
i32 = mybir.dt.int32

_CACHE = {}


def _build():
    nc = bacc.Bacc(None, target_bir_lowering=False)

    tok_d = nc.declare_dram_parameter("tok", [128, NMT], i32, isOutput=False)
    emb_d = nc.declare_dram_parameter("emb", [V, E], f32, isOutput=False)
    enc_d = nc.declare_dram_parameter("enc", [S, BL, H], bf16, isOutput=False)
    encT_d = nc.declare_dram_parameter("encT", [2, 128, BL, S], bf16, isOutput=False)
    h0T_d = nc.declare_dram_parameter("h0T", [128, 2, BL], f32, isOutput=False)
    c0T_d = nc.declare_dram_parameter("c0T", [128, 2, BL], f32, isOutput=False)
    lens_d = nc.declare_dram_parameter("lens", [BL], i32, isOutput=False)
    biasT_d = nc.declare_dram_parameter("biasT", [128, NCH], f32, isOutput=False)
    wihT_d = nc.declare_dram_parameter("wihT", [E, G4H], bf16, isOutput=False)
    whhT_d = nc.declare_dram_parameter("whhT", [2, 128, G4H], bf16, isOutput=False)
    wlinT_d = nc.declare_dram_parameter("wlinT", [4, 128, VL], bf16, isOutput=False)
    out_d = nc.declare_dram_parameter("logits", [BT, VL], bf16, isOutput=True)

    with tile.TileContext(nc) as tc:
        with tc.tile_pool(name="persist", bufs=1) as pp:
            # ---- persistent tiles; DMA queues ordered so phase-1-critical
            # loads land first (tok/emb path on sync+gpsimd, weights on
            # scalar), bulk loads (W_lin, enc) stream behind them ----
            tok_sb = pp.tile([128, NMT], i32)
            nc.sync.dma_start(out=tok_sb[:], in_=tok_d[:])
            wih_sb = pp.tile([128, G4H], bf16)
            nc.scalar.dma_start(out=wih_sb[:], in_=wihT_d[:])
            biasT_sb = pp.tile([128, NCH], f32)
            nc.scalar.dma_start(out=biasT_sb[:], in_=biasT_d[:])
            whh_sb = pp.tile([128, 2, G4H], bf16)
            for k in range(2):
                nc.scalar.dma_start(out=whh_sb[:, k, :], in_=whhT_d[k])
            # tgc holds [tanh(g) | c] adjacently so one tensor_tensor computes
            # both products of the cell update; c0 lands directly in the c slot
            tgc = pp.tile([128, 4, BL], f32)
            nc.scalar.dma_start(out=tgc[:, 2:4], in_=c0T_d[:])
            h0f = pp.tile([128, 2, BL], f32)
            nc.scalar.dma_start(out=h0f[:], in_=h0T_d[:])
            h_init = pp.tile([128, 2, BL], bf16)
            nc.vector.tensor_copy(out=h_init[:], in_=h0f[:])

            ident = pp.tile([128, 128], f32)
            make_identity(nc, ident[:])

            xwT = pp.tile([128, T, NCH, BL], bf16)     # x@W_ih.T + bias, t-major
            z01 = pp.tile([128, 2, BL, T], bf16)       # h features (k-tiles 0,1)
            z23 = pp.tile([128, 2, BL, T], bf16)       # ctx features (k-tiles 2,3)
            x_allT = pp.tile([128, BT], bf16)          # embeddings^T, (t,b) cols

            # bulk loads, needed only from phase 2 onward
            encT_sb = pp.tile([128, 2, BL, S], bf16)
            for k in range(2):
                nc.sync.dma_start(out=encT_sb[:, k], in_=encT_d[k])
            enc_sb = pp.tile([S, BL, H], bf16)
            nc.scalar.dma_start(out=enc_sb[:], in_=enc_d[:])
            # preload ALL of W_lin^T to SBUF (128 KiB/partition); the loads
            # overlap phases 0-2 so phase 3 streams nothing in
            wpre = pp.tile([128, 4, VL], bf16)
            for k in range(4):
                eng = nc.sync if k % 2 == 0 else nc.scalar
                eng.dma_start(out=wpre[:, k, :], in_=wlinT_d[k])

            ones_col = pp.tile([S, 16], bf16)
            ones_row_f = pp.tile([1, 128], f32)

            # ---- phase 0: embedding gather + transpose ----
            with (
                tc.tile_pool(name="p0", bufs=2) as wp,
                tc.tile_pool(name="p0ps", bufs=2, space="PSUM") as psp,
            ):
                for j in range(NMT):
                    x_t = wp.tile([128, E], f32, tag="x")
                    nc.gpsimd.indirect_dma_start(
                        out=x_t[:],
                        out_offset=None,
                        in_=emb_d[:],
                        in_offset=bass.IndirectOffsetOnAxis(
                            ap=tok_sb[:, j : j + 1], axis=0
                        ),
                    )
                    ps_t = psp.tile([128, 128], f32, tag="pst")
                    nc.tensor.transpose(
                        out=ps_t[:], in_=x_t[:], identity=ident[:]
                    )
                    nc.vector.tensor_copy(
                        out=x_allT[:, j * 128 : (j + 1) * 128], in_=ps_t[:]
                    )

                # xW precompute: xwT[t, c, b] = (x @ W_ih.T)[tb, c*128:...] + bias
                for c in range(NCH):
                    ps_xw = psp.tile([128, BT], f32, tag="psxw")
                    for n0, nn in [(0, 512), (512, 512), (1024, BT - 1024)]:
                        nc.tensor.matmul(
                            ps_xw[:, n0 : n0 + nn],
                            wih_sb[:, c * 128 : (c + 1) * 128],
                            x_allT[:, n0 : n0 + nn],
                            start=True,
                            stop=True,
                        )
                    nc.vector.tensor_scalar(
                        out=xwT[:, :, c, :],
                        in0=ps_xw[:].rearrange("p (t b) -> p t b", b=BL),
                        scalar1=biasT_sb[:, c : c + 1],
                        scalar2=None,
                        op0=mybir.AluOpType.add,
                    )

            # ---- attention mask prep (needed only in phase 2; emitted after
            # phase 0 so these queue entries don't block the gather path) ----
            nc.vector.memset(ones_col[:], 1.0)
            nc.vector.memset(ones_row_f[:], 1.0)
            lens_i = pp.tile([S, BL], i32)
            lens_bcast = bass.AP(tensor=lens_d, offset=0, ap=[[0, S], [1, BL]])
            nc.gpsimd.dma_start(out=lens_i[:], in_=lens_bcast)
            lens_f = pp.tile([S, BL], f32)
            nc.vector.tensor_copy(out=lens_f[:], in_=lens_i[:])
            iota_i = pp.tile([S, 1], i32)
            nc.gpsimd.iota(iota_i[:], [[1, 1]], base=0, channel_multiplier=1)
            iota_f = pp.tile([S, 1], f32)
            nc.vector.tensor_copy(out=iota_f[:], in_=iota_i[:])
            # additive mask: 0 where s < len_b, -30000 where masked
            mask_neg = pp.tile([S, BL], f32)
            nc.vector.tensor_scalar(
                out=mask_neg[:], in0=lens_f[:], scalar1=iota_f[:],
                scalar2=-30000.0, op0=mybir.AluOpType.is_le,
                op1=mybir.AluOpType.mult,
            )

            # ---- phase 1: LSTM recurrence ----
            # gate order is host-permuted to (i, f, o, g):
            # chunks 0-1=i, 2-3=f, 4-5=o, 6-7=g
            with (
                tc.tile_pool(name="p1", bufs=3) as gp,
                tc.tile_pool(name="p1ps", bufs=2, space="PSUM") as psg,
            ):
                for t in range(T):
                    ps_g = psg.tile([128, NCH, BL], f32, tag="psg")
                    # g chunks first so tanh(g) starts while i/f/o matmuls run
                    for c in (6, 7, 0, 1, 2, 3, 4, 5):
                        for k in range(2):
                            rhs = (
                                h_init[:, k, :]
                                if t == 0
                                else z01[:, k, :, t - 1]
                            )
                            nc.tensor.matmul(
                                ps_g[:, c, :],
                                whh_sb[:, k, c * 128 : (c + 1) * 128],
                                rhs,
                                start=(k == 0),
                                stop=(k == 1),
                            )
                    gates = gp.tile([128, NCH, BL], f32, tag="gates")
                    nc.vector.tensor_tensor(
                        out=gates[:, 6:8], in0=ps_g[:, 6:8], in1=xwT[:, t, 6:8],
                        op=mybir.AluOpType.add,
                    )
                    nc.scalar.activation(
                        out=tgc[:, 0:2], in_=gates[:, 6:8],
                        func=mybir.ActivationFunctionType.Tanh,
                    )
                    nc.vector.tensor_tensor(
                        out=gates[:, 0:6], in0=ps_g[:, 0:6], in1=xwT[:, t, 0:6],
                        op=mybir.AluOpType.add,
                    )
                    nc.scalar.activation(
                        out=gates[:, 0:4], in_=gates[:, 0:4],
                        func=mybir.ActivationFunctionType.Sigmoid,
                    )
                    nc.scalar.activation(
                        out=gates[:, 4:6], in_=gates[:, 4:6],
                        func=mybir.ActivationFunctionType.Sigmoid,
                    )
                    # m12 = [sig(i)*tanh(g) | sig(f)*c], then c = m1 + m2
                    m12 = gp.tile([128, 4, BL], f32, tag="m12")
                    nc.vector.tensor_tensor(
                        out=m12[:], in0=gates[:, 0:4], in1=tgc[:],
                        op=mybir.AluOpType.mult,
                    )
                    nc.vector.tensor_tensor(
                        out=tgc[:, 2:4], in0=m12[:, 0:2], in1=m12[:, 2:4],
                        op=mybir.AluOpType.add,
                    )
                    th = gp.tile([128, 2, BL], f32, tag="th")
                    nc.scalar.activation(
                        out=th[:], in_=tgc[:, 2:4],
                        func=mybir.ActivationFunctionType.Tanh,
                    )
                    # h = sig(o) * tanh(c)  -> straight into Z (bf16)
                    nc.vector.tensor_tensor(
                        out=z01[:, :, :, t], in0=gates[:, 4:6], in1=th[:],
                        op=mybir.AluOpType.mult,
                    )

            # ---- phase 2: attention over all timesteps ----
            with (
                tc.tile_pool(name="p2", bufs=2) as ap,
                tc.tile_pool(name="p2ps", bufs=2, space="PSUM") as ps2,
            ):
                expsc = pp.tile([S, BL, T], bf16)
                for b in range(BL):
                    ps_s = ps2.tile([S, T], f32, tag="ps_s")
                    for k in range(2):
                        nc.tensor.matmul(
                            ps_s[:],
                            encT_sb[:, k, b, :],
                            z01[:, k, b, :],
                            start=(k == 0),
                            stop=(k == 1),
                        )
                    # exp with additive mask folded into the activation bias
                    nc.scalar.activation(
                        out=expsc[:, b, :], in_=ps_s[:],
                        func=mybir.ActivationFunctionType.Exp,
                        scale=float(1.0 / np.sqrt(H)),
                        bias=mask_neg[:, b : b + 1],
                    )
                # batched softmax denominators: 4 column-sum matmuls over
                # groups of 4 batches, one reciprocal pass per group; each
                # group's sums fill one 512-wide PSUM bank so the reciprocal
                # reads never share a bank with the next in-flight matmul
                recip_sb = ap.tile([1, BL, T], f32, tag="recip")
                for g in range(4):
                    ps_dg = ps2.tile([16, 512], f32, tag="ps_den")
                    nc.tensor.matmul(
                        ps_dg[:, : 4 * T],
                        ones_col[:],
                        expsc[:, 4 * g : 4 * (g + 1), :].rearrange(
                            "p b t -> p (b t)"
                        ),
                        start=True,
                        stop=True,
                    )
                    nc.vector.reciprocal(
                        out=recip_sb[:, 4 * g : 4 * (g + 1), :].rearrange(
                            "p b t -> p (b t)"
                        ),
                        in_=ps_dg[0:1, : 4 * T],
                    )
                for b in range(BL):
                    ps_bc = ps2.tile([128, T], f32, tag="ps_bc")
                    nc.tensor.matmul(
                        ps_bc[:], ones_row_f[:], recip_sb[:, b, :],
                        start=True, stop=True,
                    )
                    bc_sb = ap.tile([128, T], f32, tag="bc")
                    nc.scalar.copy(out=bc_sb[:], in_=ps_bc[:])
                    for j in range(2):
                        ps_c = ps2.tile([128, T], f32, tag="ps_c")
                        nc.tensor.matmul(
                            ps_c[:],
                            enc_sb[:, b, j * 128 : (j + 1) * 128],
                            expsc[:, b, :],
                            start=True,
                            stop=True,
                        )
                        nc.vector.tensor_tensor(
                            out=z23[:, j, b, :], in0=ps_c[:], in1=bc_sb[:],
                            op=mybir.AluOpType.mult,
                        )

            # ---- phase 3: logits = Z @ W_lin^T (b_lin added on host) ----
            zt = [
                z01[:, 0].rearrange("p b t -> p (b t)"),
                z01[:, 1].rearrange("p b t -> p (b t)"),
                z23[:, 0].rearrange("p b t -> p (b t)"),
                z23[:, 1].rearrange("p b t -> p (b t)"),
            ]
            with (
                tc.tile_pool(name="p3out", bufs=3) as op_,
                tc.tile_pool(name="p3ps", bufs=2, space="PSUM") as ps3,
            ):
                dmaq = [nc.sync, nc.scalar, nc.gpsimd]
                for ng in range(NGN):
                    n0 = ng * NGC
                    for mi in range(NMT):
                        m0 = mi * 128
                        ps_o = ps3.tile([128, 4, 512], f32, tag="po")
                        for k in range(4):
                            for n in range(4):
                                nc.tensor.matmul(
                                    ps_o[:, n, :NC_N],
                                    zt[k][:, m0 : m0 + 128],
                                    wpre[:, k, n0 + n * NC_N : n0 + (n + 1) * NC_N],
                                    start=(k == 0),
                                    stop=(k == 3),
                                )
                        # bf16 eviction split across DVE (banks 0-1) and
                        # ScalarE (banks 2-3) so neither engine bottlenecks
                        o_sb = op_.tile([128, 4, NC_N], bf16, tag="osb")
                        nc.vector.tensor_copy(
                            out=o_sb[:, 0:2, :], in_=ps_o[:, 0:2, :NC_N]
                        )
                        nc.scalar.copy(
                            out=o_sb[:, 2:4, :], in_=ps_o[:, 2:4, :NC_N]
                        )
                        eng = dmaq[(ng * NMT + mi) % 3]
                        eng.dma_start(
                            out=out_d[m0 : m0 + 128, n0 : n0 + NGC],
                            in_=o_sb[:].rearrange("p g n -> p (g n)"),
                        )
    nc.compile()
    return nc


def _prep_inputs(inputs):
    bf = ml_dtypes.bfloat16
    target = np.asarray(inputs["target_tensor"])
    enc = np.asarray(inputs["encoder_outputs"], dtype=np.float32)
    lens = np.asarray(inputs["encoder_seq_lens"])
    h0 = np.asarray(inputs["h0"], dtype=np.float32)
    c0 = np.asarray(inputs["c0"], dtype=np.float32)
    emb = np.ascontiguousarray(np.asarray(inputs["emb"], dtype=np.float32))
    W_ih = np.asarray(inputs["W_ih"], dtype=np.float32)
    W_hh = np.asarray(inputs["W_hh"], dtype=np.float32)
    bias = (
        np.asarray(inputs["b_ih"], dtype=np.float32)
        + np.asarray(inputs["b_hh"], dtype=np.float32)
    )
    # permute gate order (i, f, g, o) -> (i, f, o, g) so the device can run
    # one sigmoid over the first 6 chunks and one tanh over the last 2
    perm = np.concatenate(
        [np.arange(0, 2 * H), np.arange(3 * H, 4 * H), np.arange(2 * H, 3 * H)]
    )
    W_ih = W_ih[perm]
    W_hh = W_hh[perm]
    bias = bias[perm]
    W_lin = np.asarray(inputs["W_lin"], dtype=np.float32)
    b_lin = np.asarray(inputs["b_lin"], dtype=np.float32)

    wihT = np.ascontiguousarray(W_ih.T.astype(bf))                # (E, 4H)
    whhT = np.ascontiguousarray(
        W_hh.T.reshape(2, 128, G4H).astype(bf)
    )                                                             # (2,128,4H)
    biasT = np.ascontiguousarray(bias.reshape(NCH, 128).T)        # (128, NCH)
    wlinT_full = W_lin.T.astype(bf)                               # (512, V)

    in_maps = []
    for i in range(NCORES):
        bg = i % NBG
        vh = i // NBG
        sl = slice(bg * BL, (bg + 1) * BL)
        vsl = slice(vh * VL, (vh + 1) * VL)
        # t-major flat index = t*BL + b, laid out [128, NMT] so tile column j
        # holds rows j*128..j*128+127 of the gather
        tok = np.ascontiguousarray(
            target[sl].T.reshape(BT).reshape(NMT, 128).T.astype(np.int32)
        )
        enc_i = enc[sl]                                           # (BL, S, H)
        enc_sbh = np.ascontiguousarray(
            enc_i.transpose(1, 0, 2).astype(bf)
        )                                                         # (S, BL, H)
        encT = np.ascontiguousarray(
            enc_i.transpose(2, 0, 1).reshape(2, 128, BL, S).astype(bf)
        )                                                         # (2,128,BL,S)
        h0T = np.ascontiguousarray(h0[sl].T.reshape(2, 128, BL).transpose(1, 0, 2))
        c0T = np.ascontiguousarray(c0[sl].T.reshape(2, 128, BL).transpose(1, 0, 2))
        wlinT = np.ascontiguousarray(
            wlinT_full[:, vsl].reshape(4, 128, VL)
        )                                                         # (4,128,VL)
        in_maps.append(
            {
                "tok": tok,
                "emb": emb,
                "enc": enc_sbh,
                "encT": encT,
                "h0T": h0T,
                "c0T": c0T,
                "lens": np.ascontiguousarray(lens[sl].astype(np.int32)),
                "biasT": biasT,
                "wihT": wihT,
                "whhT": whhT,
                "wlinT": wlinT,
            }
        )
    return in_maps, b_lin


LAST_RESULTS = None


def _install_ntff_shim():
    """Provide antenv.axon_hooks if the image's antenv lacks it, so
    trace=True/BASS_TRACE=1 can capture NTFF profiles under axon."""
    import sys
    import types

    try:
        from antenv.axon_hooks import get_axon_ntff_profile_hook  # noqa: F401

        return
    except ImportError:
        pass
    try:
        from trn_agent_boot.trn_boot import _ntff_profile_via_ctypes

        hook = _ntff_profile_via_ctypes("/opt/axon/libaxon_pjrt.so")
        m = types.ModuleType("antenv.axon_hooks")
        m.get_axon_ntff_profile_hook = lambda: hook
        m.set_axon_ntff_profile_hook = lambda h: None
        sys.modules["antenv.axon_hooks"] = m
    except Exception:
        pass


def kernel(**inputs):
    global LAST_RESULTS
    _install_ntff_shim()
    if "nc" not in _CACHE:
        _CACHE["nc"] = _build()
    nc = _CACHE["nc"]
    in_maps, b_lin = _prep_inputs(inputs)
    res = run_bass_kernel_spmd(nc, in_maps, core_ids=list(range(NCORES)))
    LAST_RESULTS = res
    out = np.empty((B, T, V), dtype=np.float32)
    for i in range(NCORES):
        bg = i % NBG
        vh = i // NBG
        vsl = slice(vh * VL, (vh + 1) * VL)
        # logits rows = Z columns = b*T + t (b-major); bf16 -> f32 + bias here
        out[bg * BL : (bg + 1) * BL, :, vsl] = (
            res.results[i]["logits"].astype(np.float32).reshape(BL, T, VL)
            + b_lin[None, None, vsl]
        )
    return out



# revision 31
# speedup vs baseline: 1.1557x; 1.0391x over previous
"""Trainium2 Bass kernel for DecoderAttnRNN (LSTM + attention decoder).

Sharding: hybrid over 8 cores = 4 batch-groups x 2 vocab-halves.
Each core handles 16 batches and 16000 vocab columns.

v4 design — fused pipeline:
  phase 0: embedding gather (deep-pipelined indirect DMA), transpose,
           x@W_ih.T+bias precompute split so steps 0-7 unblock early
  fused loop over 72 LSTM steps in 3 super-blocks of 24 steps:
    - LSTM recurrence using ONLY tanh (sigmoid folded via
      sig(x) = (tanh(x/2)+1)/2 with all x0.5/x2 rescales folded into the
      host-side weights), so attention's exp shares one ACT table set
    - after each super-block: attention for its 24 timesteps
    - logits quarter-groups (4 matmuls -> 500 bf16 cols -> DMA) for
      completed super-blocks are interleaved into the tensor-engine idle
      gaps of later LSTM steps; this also keeps the PE HAM-warm
  tail: remaining logits quarter-groups back-to-back
Output rows are in (superblock, batch, t_in) order; host reorders, upcasts
bf16 -> f32 and adds b_lin.

LSTM cell with stored state C = 2c, Z = 2h, gate order (o, i, f, g):
  t8 = tanh([psum + xw])        (o,i,f rows pre-scaled x0.5 on host)
  AB = (t8[i,f] + 1) * [t8[g] | C]   -> [A | B] = [2*sig_i*tanh_g | 4*sig_f*c]
  C' = 0.5*B + A                (= 2*c_new)
  th = tanh(0.5*C')             (= tanh(c_new))
  Z  = (t8[o] + 1) * th         (= 2*h_new; W_hh, scores-scale, W_lin
                                   h-columns absorb the factor 2)
"""

import numpy as np
import ml_dtypes

import concourse.bass as bass
import concourse.mybir as mybir
import concourse.tile as tile
from concourse import bacc
from concourse.bass_utils import run_bass_kernel_spmd
from concourse.masks import make_identity

B, T, S, E, H, V = 64, 72, 72, 128, 256, 32000
NCORES = 8
NBG = 4                   # batch groups
NVH = 2                   # vocab halves
BL = B // NBG             # 16 batches per core
BT = BL * T               # 1152
VL = V // NVH             # 16000 vocab cols per core
G4H = 4 * H               # 1024
NCH = G4H // 128          # 8 gate chunks of 128
NC_N = 500                # logits n-chunk (one PSUM bank)
NQG_N = VL // NC_N        # 32 n-chunks per m-tile
NMT = BT // 128           # 9 m-tiles
NSB = 3                   # super-blocks of the time axis
TB = T // NSB             # 24 steps per super-block
SBR = BL * TB             # 384 logits rows per super-block (= 3 m-tiles)
MPS = SBR // 128          # m-tiles per super-block

f32 = mybir.dt.float32
bf16 = mybir.dt.bfloat16
i32 = mybir.dt.int32

_CACHE = {}


def _build():
    nc = bacc.Bacc(None, target_bir_lowering=False)

    tok_d = nc.declare_dram_parameter("tok", [128, NMT], i32, isOutput=False)
    emb_d = nc.declare_dram_parameter("emb", [V, E], f32, isOutput=False)
    enc_d = nc.declare_dram_parameter("enc", [S, BL, H], bf16, isOutput=False)
    encT_d = nc.declare_dram_parameter("encT", [2, 128, BL, S], bf16, isOutput=False)
    h0T_d = nc.declare_dram_parameter("h0T", [128, 2, BL], f32, isOutput=False)
    c0T_d = nc.declare_dram_parameter("c0T", [128, 2, BL], f32, isOutput=False)
    mask_d = nc.declare_dram_parameter("mask24", [S, BL, TB], f32, isOutput=False)
    biasT_d = nc.declare_dram_parameter("biasT", [128, NCH], f32, isOutput=False)
    wihT_d = nc.declare_dram_parameter("wihT", [E, G4H], bf16, isOutput=False)
    whhT_d = nc.declare_dram_parameter("whhT", [2, 128, G4H], bf16, isOutput=False)
    wlinT_d = nc.declare_dram_parameter("wlinT", [4, 128, VL], bf16, isOutput=False)
    out_d = nc.declare_dram_parameter("logits", [BT, VL], bf16, isOutput=True)

    with tile.TileContext(nc) as tc:
        with tc.tile_pool(name="persist", bufs=1) as pp:
            # ---- setup DMAs: phase-0-critical loads first ----
            tok_sb = pp.tile([128, NMT], i32)
            nc.sync.dma_start(out=tok_sb[:], in_=tok_d[:])
            wih_sb = pp.tile([128, G4H], bf16)
            nc.scalar.dma_start(out=wih_sb[:], in_=wihT_d[:])
            biasT_sb = pp.tile([128, NCH], f32)
            nc.scalar.dma_start(out=biasT_sb[:], in_=biasT_d[:])
            whh_sb = pp.tile([128, 2, G4H], bf16)
            for k in range(2):
                nc.scalar.dma_start(out=whh_sb[:, k, :], in_=whhT_d[k])
            # tg8 holds the 8 tanh'd gate chunks (o,i,f,g) plus C=2c in
            # slots 8:10, so one fused op computes both cell products
            tg8 = pp.tile([128, 10, BL], f32)
            nc.scalar.dma_start(out=tg8[:, 8:10], in_=c0T_d[:])
            h0f = pp.tile([128, 2, BL], f32)
            nc.scalar.dma_start(out=h0f[:], in_=h0T_d[:])
            h_init = pp.tile([128, 2, BL], bf16)
            nc.vector.tensor_copy(out=h_init[:], in_=h0f[:])
            mask_sb = pp.tile([S, BL, TB], f32)
            nc.sync.dma_start(out=mask_sb[:], in_=mask_d[:])

            ident = pp.tile([128, 128], f32)
            make_identity(nc, ident[:])

            xwT = pp.tile([128, T, NCH, BL], bf16)       # x@W_ih.T + bias
            z01 = pp.tile([128, 2, NSB, BL, TB], bf16)   # Z=2h (k-tiles 0,1)
            z23 = pp.tile([128, 2, NSB, BL, TB], bf16)   # ctx (k-tiles 2,3)
            x_allT = pp.tile([128, BT], bf16)

            # bulk loads, needed later; queued behind the critical ones
            encT_sb = pp.tile([128, 2, BL, S], bf16)
            for k in range(2):
                nc.sync.dma_start(out=encT_sb[:, k], in_=encT_d[k])
            enc_sb = pp.tile([S, BL, H], bf16)
            nc.scalar.dma_start(out=enc_sb[:], in_=enc_d[:])
            wpre = pp.tile([128, 4, VL], bf16)
            for k in range(4):
                eng = nc.sync if k % 2 == 0 else nc.scalar
                eng.dma_start(out=wpre[:, k, :], in_=wlinT_d[k])

            ones_col = pp.tile([S, 16], bf16)
            ones_row_f = pp.tile([1, 128], f32)
            nc.vector.memset(ones_col[:], 1.0)
            nc.vector.memset(ones_row_f[:], 1.0)

            # ---- phase 0: embedding gather + transpose (deep pipeline) ----
            with (
                tc.tile_pool(name="p0", bufs=4) as wp,
                tc.tile_pool(name="p0ps", bufs=2, space="PSUM") as psp,
            ):
                casts = []
                for j in range(NMT):
                    x_t = wp.tile([128, E], f32, tag="x")
                    nc.gpsimd.indirect_dma_start(
                        out=x_t[:],
                        out_offset=None,
                        in_=emb_d[:],
                        in_offset=bass.IndirectOffsetOnAxis(
                            ap=tok_sb[:, j : j + 1], axis=0
                        ),
                    )
                    ps_t = psp.tile([128, 128], f32, tag="pst")
                    nc.tensor.transpose(out=ps_t[:], in_=x_t[:], identity=ident[:])
                    nc.vector.tensor_copy(
                        out=x_allT[:, j * 128 : (j + 1) * 128], in_=ps_t[:]
                    )

                # early xW for t<8 (x_allT cols 0:128) so the LSTM can start
                ps_xw8 = psp.tile([128, NCH, 128], f32, tag="psxw8")
                for c in range(NCH):
                    nc.tensor.matmul(
                        ps_xw8[:, c, :],
                        wih_sb[:, c * 128 : (c + 1) * 128],
                        x_allT[:, 0:128],
                        start=True,
                        stop=True,
                    )
                for c in range(NCH):
                    nc.vector.tensor_scalar(
                        out=xwT[:, 0:8, c, :],
                        in0=ps_xw8[:, c, :].rearrange("p (t b) -> p t b", b=BL),
                        scalar1=biasT_sb[:, c : c + 1],
                        scalar2=None,
                        op0=mybir.AluOpType.add,
                    )

            def emit_xw_rest(c):
                # two 512-wide passes to keep PSUM small (32 t per pass)
                for half in range(2):
                    t0 = 8 + 32 * half
                    ps_xw = xwp.tile([128, 512], f32, tag="psxw")
                    nc.tensor.matmul(
                        ps_xw[:],
                        wih_sb[:, c * 128 : (c + 1) * 128],
                        x_allT[:, t0 * BL : (t0 + 32) * BL],
                        start=True,
                        stop=True,
                    )
                    nc.vector.tensor_scalar(
                        out=xwT[:, t0 : t0 + 32, c, :],
                        in0=ps_xw[:].rearrange("p (t b) -> p t b", b=BL),
                        scalar1=biasT_sb[:, c : c + 1],
                        scalar2=None,
                        op0=mybir.AluOpType.add,
                    )

            # ---- fused loop: LSTM steps + per-superblock attention +
            #      interleaved logits quarter-groups ----
            zt = [
                z01[:, 0].rearrange("p s b t -> p (s b t)"),
                z01[:, 1].rearrange("p s b t -> p (s b t)"),
                z23[:, 0].rearrange("p s b t -> p (s b t)"),
                z23[:, 1].rearrange("p s b t -> p (s b t)"),
            ]
            qready = []          # (mi, ng) logits quarter-groups ready to run
            qcount = [0]
            _attn_state = {}
            _apools = {}

            lstm_pool = tc.tile_pool(name="lstm", bufs=3)
            lp = lstm_pool.__enter__()
            lstmps_pool = tc.tile_pool(name="lstmps", bufs=2, space="PSUM")
            lps = lstmps_pool.__enter__()
            outq_pool = tc.tile_pool(name="outq", bufs=3)
            oqp = outq_pool.__enter__()
            outqps_pool = tc.tile_pool(name="outqps", bufs=2, space="PSUM")
            oqps = outqps_pool.__enter__()
            # innermost: remainder-xW PSUM, released once the attention pools
            # are needed (pools close in stack order)
            xw_pool = tc.tile_pool(name="pxw", bufs=2, space="PSUM")
            xwp = xw_pool.__enter__()
            dmaq = [nc.sync, nc.scalar, nc.gpsimd]

            def emit_qgroup():
                if not qready:
                    return
                mi, nq = qready.pop(0)
                n0 = nq * NC_N
                ps_o = oqps.tile([128, 512], f32, tag="po")
                for k in range(4):
                    nc.tensor.matmul(
                        ps_o[:, :NC_N],
                        zt[k][:, mi * 128 : (mi + 1) * 128],
                        wpre[:, k, n0 : n0 + NC_N],
                        start=(k == 0),
                        stop=(k == 3),
                    )
                o_sb = oqp.tile([128, NC_N], bf16, tag="osb")
                eng = nc.vector.tensor_copy if qcount[0] % 2 == 0 else nc.scalar.copy
                eng(out=o_sb[:], in_=ps_o[:, :NC_N])
                dmaq[qcount[0] % 3].dma_start(
                    out=out_d[mi * 128 : (mi + 1) * 128, n0 : n0 + NC_N],
                    in_=o_sb[:],
                )
                qcount[0] += 1

            def emit_step(t):
                sb, ti = divmod(t, TB)
                ps_g = lps.tile([128, NCH, BL], f32, tag="psg")
                for c in range(NCH):
                    for k in range(2):
                        rhs = (
                            h_init[:, k, :] if t == 0
                            else z01[:, k, (t - 1) // TB, :, (t - 1) % TB]
                        )
                        nc.tensor.matmul(
                            ps_g[:, c, :],
                            whh_sb[:, k, c * 128 : (c + 1) * 128],
                            rhs,
                            start=(k == 0),
                            stop=(k == 1),
                        )
                gsum = lp.tile([128, NCH, BL], f32, tag="gsum")
                nc.vector.tensor_tensor(
                    out=gsum[:], in0=ps_g[:], in1=xwT[:, t],
                    op=mybir.AluOpType.add,
                )
                # single tanh over all gates (o,i,f pre-scaled x0.5 on host)
                nc.scalar.activation(
                    out=tg8[:, 0:8], in_=gsum[:],
                    func=mybir.ActivationFunctionType.Tanh,
                )
                # AB = (t8[i,f]+1) * [t8[g] | C]
                ab = lp.tile([128, 4, BL], f32, tag="ab")
                nc.vector.scalar_tensor_tensor(
                    out=ab[:], in0=tg8[:, 2:6], scalar=1.0, in1=tg8[:, 6:10],
                    op0=mybir.AluOpType.add, op1=mybir.AluOpType.mult,
                )
                # C' = 0.5*B + A
                nc.vector.scalar_tensor_tensor(
                    out=tg8[:, 8:10], in0=ab[:, 2:4], scalar=0.5, in1=ab[:, 0:2],
                    op0=mybir.AluOpType.mult, op1=mybir.AluOpType.add,
                )
                th = lp.tile([128, 2, BL], f32, tag="th")
                nc.scalar.activation(
                    out=th[:], in_=tg8[:, 8:10],
                    func=mybir.ActivationFunctionType.Tanh, scale=0.5,
                )
                # Z = (t8[o]+1) * th  (bf16, = 2*h)
                nc.vector.scalar_tensor_tensor(
                    out=z01[:, :, sb, :, ti], in0=tg8[:, 0:2], scalar=1.0,
                    in1=th[:], op0=mybir.AluOpType.add, op1=mybir.AluOpType.mult,
                )

            def emit_attention_chunk(sb, step):
                # attention for super-block sb, split into 6 chunks emitted
                # across consecutive later steps to bound PE-queue delay
                atp = _apools["atp"]
                atps = _apools["atps"]
                if step == 0:
                    st = atps.tile([S, BL, 32], f32, tag="ps_s")
                    exb = atp.tile([S, BL, TB], bf16, tag="exb")
                    rcp = atp.tile([1, BL, TB], f32, tag="rcp")
                    att = atp.tile([S, BL, TB], bf16, tag="att")
                    den = atps.tile([16, 4, 128], f32, tag="den")
                    bc = atps.tile([128, 512], f32, tag="bc")
                    ctx = atps.tile([128, 2, 256], f32, tag="ctx")
                    _attn_state[sb] = (st, exb, rcp, att, den, bc, ctx)
                st, exb, rcp, att, den, bc, ctx = _attn_state[sb]
                if step in (0, 1):
                    for b in range(8 * step, 8 * (step + 1)):
                        for k in range(2):
                            nc.tensor.matmul(
                                st[:, b, :TB],
                                encT_sb[:, k, b, :],
                                z01[:, k, sb, b, :],
                                start=(k == 0),
                                stop=(k == 1),
                            )
                elif step == 2:
                    # masked exp over all (b,t) of the block; Z=2h so the
                    # score scale halves
                    nc.vector.tensor_tensor(
                        out=exb[:], in0=st[:, :, :TB], in1=mask_sb[:],
                        op=mybir.AluOpType.add,
                    )
                    nc.scalar.activation(
                        out=exb[:], in_=exb[:],
                        func=mybir.ActivationFunctionType.Exp,
                        scale=float(0.5 / np.sqrt(H)),
                    )
                elif step == 3:
                    for g in range(4):
                        nc.tensor.matmul(
                            den[:, g, : 4 * TB],
                            ones_col[:],
                            exb[:, 4 * g : 4 * (g + 1), :].rearrange(
                                "p b t -> p (b t)"
                            ),
                            start=True,
                            stop=True,
                        )
                        nc.vector.reciprocal(
                            out=rcp[:, 4 * g : 4 * (g + 1), :].rearrange(
                                "p b t -> p (b t)"
                            ),
                            in_=den[0:1, g, : 4 * TB],
                        )
                    nc.tensor.matmul(
                        bc[:, : BL * TB], ones_row_f[:],
                        rcp[:].rearrange("p b t -> p (b t)"),
                        start=True, stop=True,
                    )
                    nc.vector.tensor_tensor(
                        out=att[:].rearrange("p b t -> p (b t)"),
                        in0=exb[:].rearrange("p b t -> p (b t)"),
                        in1=bc[0:S, : BL * TB],
                        op=mybir.AluOpType.mult,
                    )
                elif step in (4, 5):
                    # half the batches per chunk; evict frees the PSUM tile
                    # for the second half (attnps has bufs=1)
                    b0 = 8 * (step - 4)
                    for b in range(b0, b0 + 8):
                        for j in range(2):
                            nc.tensor.matmul(
                                ctx[:, j, (b - b0) * TB : (b - b0 + 1) * TB],
                                enc_sb[:, b, j * 128 : (j + 1) * 128],
                                att[:, b, :],
                                start=True,
                                stop=True,
                            )
                    nc.vector.tensor_copy(
                        out=z23[:, :, sb, b0 : b0 + 8, :].rearrange(
                            "p k b t -> p k (b t)"
                        ),
                        in_=ctx[:, :, : 8 * TB],
                    )
                    if step == 5:
                        for mi in range(sb * MPS, (sb + 1) * MPS):
                            for nq in range(NQG_N):
                                qready.append((mi, nq))

            # ---- emit the fused schedule ----
            for t in range(T):
                if 1 <= t <= NCH:
                    emit_xw_rest(t - 1)
                if t == NCH + 1:
                    # xW PSUM freed; attention pools take its place
                    xw_pool.__exit__(None, None, None)
                    attn_pool = tc.tile_pool(name="attn", bufs=2)
                    attnps_pool = tc.tile_pool(name="attnps", bufs=1, space="PSUM")
                    _apools["atp"] = attn_pool.__enter__()
                    _apools["atps"] = attnps_pool.__enter__()
                emit_step(t)
                sb_prev = t // TB - 1
                ph = t % TB
                if sb_prev >= 0 and ph < 6:
                    emit_attention_chunk(sb_prev, ph)
                elif t >= TB + 6:
                    emit_qgroup()
                    emit_qgroup()
            # last super-block's attention, then drain all remaining groups
            for stp in range(6):
                emit_attention_chunk(NSB - 1, stp)
                emit_qgroup()
            while qready:
                emit_qgroup()

            for pool in (
                attnps_pool, attn_pool, outqps_pool, outq_pool,
                lstmps_pool, lstm_pool,
            ):
                pool.__exit__(None, None, None)
    nc.compile()
    return nc


def _prep_inputs(inputs):
    bf = ml_dtypes.bfloat16
    target = np.asarray(inputs["target_tensor"])
    enc = np.asarray(inputs["encoder_outputs"], dtype=np.float32)
    lens = np.asarray(inputs["encoder_seq_lens"])
    h0 = np.asarray(inputs["h0"], dtype=np.float32)
    c0 = np.asarray(inputs["c0"], dtype=np.float32)
    emb = np.ascontiguousarray(np.asarray(inputs["emb"], dtype=np.float32))
    W_ih = np.asarray(inputs["W_ih"], dtype=np.float32)
    W_hh = np.asarray(inputs["W_hh"], dtype=np.float32)
    bias = (
        np.asarray(inputs["b_ih"], dtype=np.float32)
        + np.asarray(inputs["b_hh"], dtype=np.float32)
    )
    # gate order (i, f, g, o) -> (o, i, f, g); o/i/f rows x0.5 (tanh trick);
    # all W_hh entries x0.5 again because the device streams Z = 2h
    perm = np.concatenate(
        [np.arange(3 * H, 4 * H), np.arange(0, 2 * H), np.arange(2 * H, 3 * H)]
    )
    rs = np.concatenate([np.full(3 * H, 0.5, np.float32), np.ones(H, np.float32)])
    W_ih = W_ih[perm] * rs[:, None]
    W_hh = W_hh[perm] * rs[:, None] * 0.5
    bias = bias[perm] * rs
    W_lin = np.asarray(inputs["W_lin"], dtype=np.float32)
    b_lin = np.asarray(inputs["b_lin"], dtype=np.float32)

    wihT = np.ascontiguousarray(W_ih.T.astype(bf))                # (E, 4H)
    whhT = np.ascontiguousarray(
        W_hh.T.reshape(2, 128, G4H).astype(bf)
    )                                                             # (2,128,4H)
    biasT = np.ascontiguousarray(bias.reshape(NCH, 128).T)        # (128, NCH)
    # h-columns of W_lin x0.5 (Z = 2h); ctx columns unscaled
    wlinT_full = W_lin.T.copy()
    wlinT_full[:H] *= 0.5
    wlinT_full = wlinT_full.astype(bf)                            # (512, V)

    in_maps = []
    for i in range(NCORES):
        bg = i % NBG
        vh = i // NBG
        sl = slice(bg * BL, (bg + 1) * BL)
        vsl = slice(vh * VL, (vh + 1) * VL)
        # t-major flat index = t*BL + b, laid out [128, NMT]
        tok = np.ascontiguousarray(
            target[sl].T.reshape(BT).reshape(NMT, 128).T.astype(np.int32)
        )
        enc_i = enc[sl]                                           # (BL, S, H)
        enc_sbh = np.ascontiguousarray(
            enc_i.transpose(1, 0, 2).astype(bf)
        )                                                         # (S, BL, H)
        encT = np.ascontiguousarray(
            enc_i.transpose(2, 0, 1).reshape(2, 128, BL, S).astype(bf)
        )                                                         # (2,128,BL,S)
        # device state carries 2*h0 / 2*c0
        h0T = np.ascontiguousarray(
            (2.0 * h0[sl]).T.reshape(2, 128, BL).transpose(1, 0, 2)
        )
        c0T = np.ascontiguousarray(
            (2.0 * c0[sl]).T.reshape(2, 128, BL).transpose(1, 0, 2)
        )
        # additive mask (0 valid / -30000 masked), broadcast over TB
        m01 = (np.arange(S)[:, None] < lens[sl][None, :]).astype(np.float32)
        mask24 = np.ascontiguousarray(
            np.broadcast_to(
                ((m01 - 1.0) * 30000.0)[:, :, None], (S, BL, TB)
            ).astype(np.float32)
        )
        wlinT = np.ascontiguousarray(
            wlinT_full[:, vsl].reshape(4, 128, VL)
        )                                                         # (4,128,VL)
        in_maps.append(
            {
                "tok": tok,
                "emb": emb,
                "enc": enc_sbh,
                "encT": encT,
                "h0T": h0T,
                "c0T": c0T,
                "mask24": mask24,
                "biasT": biasT,
                "wihT": wihT,
                "whhT": whhT,
                "wlinT": wlinT,
            }
        )
    return in_maps, b_lin


LAST_RESULTS = None


def _install_ntff_shim():
    """Provide antenv.axon_hooks if the image's antenv lacks it, so
    trace=True/BASS_TRACE=1 can capture NTFF profiles under axon."""
    import sys
    import types

    try:
        from antenv.axon_hooks import get_axon_ntff_profile_hook  # noqa: F401

        return
    except ImportError:
        pass
    try:
        from trn_agent_boot.trn_boot import _ntff_profile_via_ctypes

        hook = _ntff_profile_via_ctypes("/opt/axon/libaxon_pjrt.so")
        m = types.ModuleType("antenv.axon_hooks")
        m.get_axon_ntff_profile_hook = lambda: hook
        m.set_axon_ntff_profile_hook = lambda h: None
        sys.modules["antenv.axon_hooks"] = m
    except Exception:
        pass


def kernel(**inputs):
    global LAST_RESULTS
    _install_ntff_shim()
    if "nc" not in _CACHE:
        _CACHE["nc"] = _build()
    nc = _CACHE["nc"]
    in_maps, b_lin = _prep_inputs(inputs)
    res = run_bass_kernel_spmd(nc, in_maps, core_ids=list(range(NCORES)))
    LAST_RESULTS = res
    out = np.empty((B, T, V), dtype=np.float32)
    for i in range(NCORES):
        bg = i % NBG
        vh = i // NBG
        vsl = slice(vh * VL, (vh + 1) * VL)
        # logits rows are (superblock, batch, t_in); reorder to (b, t)
        lg = (
            res.results[i]["logits"]
            .astype(np.float32)
            .reshape(NSB, BL, TB, VL)
            .transpose(1, 0, 2, 3)
            .reshape(BL, T, VL)
        )
        out[bg * BL : (bg + 1) * BL, :, vsl] = lg + b_lin[None, None, vsl]
    return out


# revision 38
# speedup vs baseline: 1.1592x; 1.0030x over previous
"""Trainium2 Bass kernel for DecoderAttnRNN (LSTM + attention decoder).

Sharding: hybrid over 8 cores = 4 batch-groups x 2 vocab-halves.
Each core handles 16 batches and 16000 vocab columns.

v4 design — fused pipeline:
  phase 0: embedding gather (deep-pipelined indirect DMA), transpose,
           x@W_ih.T+bias precompute split so steps 0-7 unblock early
  fused loop over 72 LSTM steps in 3 super-blocks of 24 steps:
    - LSTM recurrence using ONLY tanh (sigmoid folded via
      sig(x) = (tanh(x/2)+1)/2 with all x0.5/x2 rescales folded into the
      host-side weights), so attention's exp shares one ACT table set
    - after each super-block: attention for its 24 timesteps
    - logits quarter-groups (4 matmuls -> 500 bf16 cols -> DMA) for
      completed super-blocks are interleaved into the tensor-engine idle
      gaps of later LSTM steps; this also keeps the PE HAM-warm
  tail: remaining logits quarter-groups back-to-back
Output rows are in (superblock, batch, t_in) order; host reorders, upcasts
bf16 -> f32 and adds b_lin.

LSTM cell with stored state C = 2c, Z = 2h, gate order (o, i, f, g):
  t8 = tanh([psum + xw])        (o,i,f rows pre-scaled x0.5 on host)
  AB = (t8[i,f] + 1) * [t8[g] | C]   -> [A | B] = [2*sig_i*tanh_g | 4*sig_f*c]
  C' = 0.5*B + A                (= 2*c_new)
  th = tanh(0.5*C')             (= tanh(c_new))
  Z  = (t8[o] + 1) * th         (= 2*h_new; W_hh, scores-scale, W_lin
                                   h-columns absorb the factor 2)
"""

import numpy as np
import ml_dtypes

import concourse.bass as bass
import concourse.mybir as mybir
import concourse.tile as tile
from concourse import bacc
from concourse.bass_utils import run_bass_kernel_spmd
from concourse.masks import make_identity

B, T, S, E, H, V = 64, 72, 72, 128, 256, 32000
NCORES = 8
NBG = 4                   # batch groups
NVH = 2                   # vocab halves
BL = B // NBG             # 16 batches per core
BT = BL * T               # 1152
VL = V // NVH             # 16000 vocab cols per core
G4H = 4 * H               # 1024
NCH = G4H // 128          # 8 gate chunks of 128
NC_N = 500                # logits n-chunk (one PSUM bank)
NQG_N = VL // NC_N        # 32 n-chunks per m-tile
NMT = BT // 128           # 9 m-tiles
NSB = 3                   # super-blocks of the time axis
TB = T // NSB             # 24 steps per super-block
SBR = BL * TB             # 384 logits rows per super-block (= 3 m-tiles)
MPS = SBR // 128          # m-tiles per super-block

f32 = mybir.dt.float32
bf16 = mybir.dt.bfloat16
i32 = mybir.dt.int32

_CACHE = {}


def _build():
    nc = bacc.Bacc(None, target_bir_lowering=False)

    tok_d = nc.declare_dram_parameter("tok", [128, NMT], i32, isOutput=False)
    emb_d = nc.declare_dram_parameter("emb", [V, E], f32, isOutput=False)
    enc_d = nc.declare_dram_parameter("enc", [S, BL, H], bf16, isOutput=False)
    encT_d = nc.declare_dram_parameter("encT", [2, 128, BL, S], bf16, isOutput=False)
    h0T_d = nc.declare_dram_parameter("h0T", [128, 2, BL], f32, isOutput=False)
    c0T_d = nc.declare_dram_parameter("c0T", [128, 2, BL], f32, isOutput=False)
    mask_d = nc.declare_dram_parameter("mask24", [S, BL, TB], f32, isOutput=False)
    biasT_d = nc.declare_dram_parameter("biasT", [128, NCH], f32, isOutput=False)
    wihT_d = nc.declare_dram_parameter("wihT", [E, G4H], bf16, isOutput=False)
    whhT_d = nc.declare_dram_parameter("whhT", [2, 128, G4H], bf16, isOutput=False)
    wlinT_d = nc.declare_dram_parameter("wlinT", [4, 128, VL], bf16, isOutput=False)
    out_d = nc.declare_dram_parameter("logits", [BT, VL], bf16, isOutput=True)

    with tile.TileContext(nc) as tc:
        with tc.tile_pool(name="persist", bufs=1) as pp:
            # ---- setup DMAs: phase-0-critical loads first ----
            tok_sb = pp.tile([128, NMT], i32)
            nc.sync.dma_start(out=tok_sb[:], in_=tok_d[:])
            wih_sb = pp.tile([128, G4H], bf16)
            nc.scalar.dma_start(out=wih_sb[:], in_=wihT_d[:])
            biasT_sb = pp.tile([128, NCH], f32)
            nc.scalar.dma_start(out=biasT_sb[:], in_=biasT_d[:])
            whh_sb = pp.tile([128, 2, G4H], bf16)
            for k in range(2):
                nc.scalar.dma_start(out=whh_sb[:, k, :], in_=whhT_d[k])
            # tg8 holds the 8 tanh'd gate chunks (o,i,f,g) plus C=2c in
            # slots 8:10, so one fused op computes both cell products
            tg8 = pp.tile([128, 10, BL], f32)
            nc.scalar.dma_start(out=tg8[:, 8:10], in_=c0T_d[:])
            h0f = pp.tile([128, 2, BL], f32)
            nc.scalar.dma_start(out=h0f[:], in_=h0T_d[:])
            h_init = pp.tile([128, 2, BL], bf16)
            mask_sb = pp.tile([S, BL, TB], f32)
            nc.sync.dma_start(out=mask_sb[:], in_=mask_d[:])

            ident = pp.tile([128, 128], f32)
            make_identity(nc, ident[:])

            xwT = pp.tile([128, T, NCH, BL], bf16)       # x@W_ih.T + bias
            z01 = pp.tile([128, 2, NSB, BL, TB], bf16)   # Z=2h (k-tiles 0,1)
            z23 = pp.tile([128, 2, NSB, BL, TB], bf16)   # ctx (k-tiles 2,3)
            x_allT = pp.tile([128, BT], bf16)

            # bulk loads, needed later; queued behind the critical ones
            encT_sb = pp.tile([128, 2, BL, S], bf16)
            for k in range(2):
                nc.sync.dma_start(out=encT_sb[:, k], in_=encT_d[k])
            enc_sb = pp.tile([S, BL, H], bf16)
            nc.scalar.dma_start(out=enc_sb[:], in_=enc_d[:])
            wpre = pp.tile([128, 4, VL], bf16)
            for k in range(4):
                eng = nc.sync if k % 2 == 0 else nc.scalar
                eng.dma_start(out=wpre[:, k, :], in_=wlinT_d[k])

            ones_col = pp.tile([S, 16], bf16)
            ones_row_f = pp.tile([1, 128], f32)
            nc.vector.memset(ones_col[:], 1.0)
            nc.vector.memset(ones_row_f[:], 1.0)

            # ---- phase 0: embedding gather + transpose (deep pipeline) ----
            with (
                tc.tile_pool(name="p0", bufs=4) as wp,
                tc.tile_pool(name="p0ps", bufs=2, space="PSUM") as psp,
            ):
                casts = []
                for j in range(NMT):
                    x_t = wp.tile([128, E], f32, tag="x")
                    nc.gpsimd.indirect_dma_start(
                        out=x_t[:],
                        out_offset=None,
                        in_=emb_d[:],
                        in_offset=bass.IndirectOffsetOnAxis(
                            ap=tok_sb[:, j : j + 1], axis=0
                        ),
                    )
                    ps_t = psp.tile([128, 128], f32, tag="pst")
                    nc.tensor.transpose(out=ps_t[:], in_=x_t[:], identity=ident[:])
                    nc.vector.tensor_copy(
                        out=x_allT[:, j * 128 : (j + 1) * 128], in_=ps_t[:]
                    )

                # early xW for t<8 (x_allT cols 0:128) so the LSTM can start
                ps_xw8 = psp.tile([128, NCH, 128], f32, tag="psxw8")
                for c in range(NCH):
                    nc.tensor.matmul(
                        ps_xw8[:, c, :],
                        wih_sb[:, c * 128 : (c + 1) * 128],
                        x_allT[:, 0:128],
                        start=True,
                        stop=True,
                    )
                for c in range(NCH):
                    nc.vector.tensor_scalar(
                        out=xwT[:, 0:8, c, :],
                        in0=ps_xw8[:, c, :].rearrange("p (t b) -> p t b", b=BL),
                        scalar1=biasT_sb[:, c : c + 1],
                        scalar2=None,
                        op0=mybir.AluOpType.add,
                    )

            # h_init conversion deferred to here so the copy never blocks the
            # gather-cast pipeline at the head of the DVE queue
            nc.vector.tensor_copy(out=h_init[:], in_=h0f[:])

            def emit_xw_rest(c):
                # two 512-wide passes to keep PSUM small (32 t per pass)
                for half in range(2):
                    t0 = 8 + 32 * half
                    ps_xw = xwp.tile([128, 512], f32, tag="psxw")
                    nc.tensor.matmul(
                        ps_xw[:],
                        wih_sb[:, c * 128 : (c + 1) * 128],
                        x_allT[:, t0 * BL : (t0 + 32) * BL],
                        start=True,
                        stop=True,
                    )
                    nc.vector.tensor_scalar(
                        out=xwT[:, t0 : t0 + 32, c, :],
                        in0=ps_xw[:].rearrange("p (t b) -> p t b", b=BL),
                        scalar1=biasT_sb[:, c : c + 1],
                        scalar2=None,
                        op0=mybir.AluOpType.add,
                    )

            # ---- fused loop: LSTM steps + per-superblock attention +
            #      interleaved logits quarter-groups ----
            zt = [
                z01[:, 0].rearrange("p s b t -> p (s b t)"),
                z01[:, 1].rearrange("p s b t -> p (s b t)"),
                z23[:, 0].rearrange("p s b t -> p (s b t)"),
                z23[:, 1].rearrange("p s b t -> p (s b t)"),
            ]
            qready = []          # (mi, ng) logits quarter-groups ready to run
            qcount = [0]
            _attn_state = {}
            _apools = {}

            lstm_pool = tc.tile_pool(name="lstm", bufs=3)
            lp = lstm_pool.__enter__()
            lstmps_pool = tc.tile_pool(name="lstmps", bufs=2, space="PSUM")
            lps = lstmps_pool.__enter__()
            outq_pool = tc.tile_pool(name="outq", bufs=3)
            oqp = outq_pool.__enter__()
            outqps_pool = tc.tile_pool(name="outqps", bufs=2, space="PSUM")
            oqps = outqps_pool.__enter__()
            # innermost: remainder-xW PSUM, released once the attention pools
            # are needed (pools close in stack order)
            xw_pool = tc.tile_pool(name="pxw", bufs=2, space="PSUM")
            xwp = xw_pool.__enter__()
            dmaq = [nc.sync, nc.scalar, nc.gpsimd]

            pending_ev = []

            def emit_qgroup():
                # matmuls now; the eviction+DMA is deferred so the scheduler
                # gives the next LSTM step's chain ops priority over it
                if not qready:
                    return
                mi, nq = qready.pop(0)
                n0 = nq * NC_N
                ps_o = oqps.tile([128, 512], f32, tag="po")
                for k in range(4):
                    nc.tensor.matmul(
                        ps_o[:, :NC_N],
                        zt[k][:, mi * 128 : (mi + 1) * 128],
                        wpre[:, k, n0 : n0 + NC_N],
                        start=(k == 0),
                        stop=(k == 3),
                    )
                cnt = qcount[0]
                qcount[0] += 1

                def evict():
                    o_sb = oqp.tile([128, NC_N], bf16, tag="osb")
                    eng = nc.vector.tensor_copy if cnt % 2 == 0 else nc.scalar.copy
                    eng(out=o_sb[:], in_=ps_o[:, :NC_N])
                    dmaq[cnt % 3].dma_start(
                        out=out_d[mi * 128 : (mi + 1) * 128, n0 : n0 + NC_N],
                        in_=o_sb[:],
                    )

                pending_ev.append(evict)

            def flush_evicts():
                while pending_ev:
                    pending_ev.pop(0)()

            def emit_step(t):
                sb, ti = divmod(t, TB)
                ps_g = lps.tile([128, NCH, BL], f32, tag="psg")
                for c in range(NCH):
                    for k in range(2):
                        rhs = (
                            h_init[:, k, :] if t == 0
                            else z01[:, k, (t - 1) // TB, :, (t - 1) % TB]
                        )
                        nc.tensor.matmul(
                            ps_g[:, c, :],
                            whh_sb[:, k, c * 128 : (c + 1) * 128],
                            rhs,
                            start=(k == 0),
                            stop=(k == 1),
                        )
                gsum = lp.tile([128, NCH, BL], f32, tag="gsum")
                nc.vector.tensor_tensor(
                    out=gsum[:], in0=ps_g[:], in1=xwT[:, t],
                    op=mybir.AluOpType.add,
                )
                # single tanh over all gates (o,i,f pre-scaled x0.5 on host)
                nc.scalar.activation(
                    out=tg8[:, 0:8], in_=gsum[:],
                    func=mybir.ActivationFunctionType.Tanh,
                )
                # AB = (t8[i,f]+1) * [t8[g] | C]
                ab = lp.tile([128, 4, BL], f32, tag="ab")
                nc.vector.scalar_tensor_tensor(
                    out=ab[:], in0=tg8[:, 2:6], scalar=1.0, in1=tg8[:, 6:10],
                    op0=mybir.AluOpType.add, op1=mybir.AluOpType.mult,
                )
                # C' = 0.5*B + A
                nc.vector.scalar_tensor_tensor(
                    out=tg8[:, 8:10], in0=ab[:, 2:4], scalar=0.5, in1=ab[:, 0:2],
                    op0=mybir.AluOpType.mult, op1=mybir.AluOpType.add,
                )
                th = lp.tile([128, 2, BL], f32, tag="th")
                nc.scalar.activation(
                    out=th[:], in_=tg8[:, 8:10],
                    func=mybir.ActivationFunctionType.Tanh, scale=0.5,
                )
                # Z = (t8[o]+1) * th  (bf16, = 2*h)
                nc.vector.scalar_tensor_tensor(
                    out=z01[:, :, sb, :, ti], in0=tg8[:, 0:2], scalar=1.0,
                    in1=th[:], op0=mybir.AluOpType.add, op1=mybir.AluOpType.mult,
                )

            def emit_attention_chunk(sb, step):
                # attention for super-block sb, split into 6 chunks emitted
                # across consecutive later steps to bound PE-queue delay
                atp = _apools["atp"]
                atps = _apools["atps"]
                if step == 0:
                    st = atps.tile([S, BL, 32], f32, tag="ps_s")
                    exb = atp.tile([S, BL, TB], bf16, tag="exb")
                    rcp = atp.tile([1, BL, TB], f32, tag="rcp")
                    att = atp.tile([S, BL, TB], bf16, tag="att")
                    den = atps.tile([16, 4, 128], f32, tag="den")
                    bc = atps.tile([128, 512], f32, tag="bc")
                    ctx = atps.tile([128, 2, 256], f32, tag="ctx")
                    _attn_state[sb] = (st, exb, rcp, att, den, bc, ctx)
                st, exb, rcp, att, den, bc, ctx = _attn_state[sb]
                if step in (0, 1):
                    for b in range(8 * step, 8 * (step + 1)):
                        for k in range(2):
                            nc.tensor.matmul(
                                st[:, b, :TB],
                                encT_sb[:, k, b, :],
                                z01[:, k, sb, b, :],
                                start=(k == 0),
                                stop=(k == 1),
                            )
                elif step == 2:
                    # masked exp over all (b,t) of the block; Z=2h so the
                    # score scale halves
                    nc.vector.tensor_tensor(
                        out=exb[:], in0=st[:, :, :TB], in1=mask_sb[:],
                        op=mybir.AluOpType.add,
                    )
                    nc.scalar.activation(
                        out=exb[:], in_=exb[:],
                        func=mybir.ActivationFunctionType.Exp,
                        scale=float(0.5 / np.sqrt(H)),
                    )
                elif step == 3:
                    for g in range(4):
                        nc.tensor.matmul(
                            den[:, g, : 4 * TB],
                            ones_col[:],
                            exb[:, 4 * g : 4 * (g + 1), :].rearrange(
                                "p b t -> p (b t)"
                            ),
                            start=True,
                            stop=True,
                        )
                        nc.vector.reciprocal(
                            out=rcp[:, 4 * g : 4 * (g + 1), :].rearrange(
                                "p b t -> p (b t)"
                            ),
                            in_=den[0:1, g, : 4 * TB],
                        )
                    nc.tensor.matmul(
                        bc[:, : BL * TB], ones_row_f[:],
                        rcp[:].rearrange("p b t -> p (b t)"),
                        start=True, stop=True,
                    )
                    nc.vector.tensor_tensor(
                        out=att[:].rearrange("p b t -> p (b t)"),
                        in0=exb[:].rearrange("p b t -> p (b t)"),
                        in1=bc[0:S, : BL * TB],
                        op=mybir.AluOpType.mult,
                    )
                elif step in (4, 5):
                    # half the batches per chunk; evict frees the PSUM tile
                    # for the second half (attnps has bufs=1)
                    b0 = 8 * (step - 4)
                    for b in range(b0, b0 + 8):
                        for j in range(2):
                            nc.tensor.matmul(
                                ctx[:, j, (b - b0) * TB : (b - b0 + 1) * TB],
                                enc_sb[:, b, j * 128 : (j + 1) * 128],
                                att[:, b, :],
                                start=True,
                                stop=True,
                            )
                    nc.vector.tensor_copy(
                        out=z23[:, :, sb, b0 : b0 + 8, :].rearrange(
                            "p k b t -> p k (b t)"
                        ),
                        in_=ctx[:, :, : 8 * TB],
                    )
                    if step == 5:
                        for mi in range(sb * MPS, (sb + 1) * MPS):
                            for nq in range(NQG_N):
                                qready.append((mi, nq))

            # ---- emit the fused schedule ----
            for t in range(T):
                if 1 <= t <= NCH:
                    emit_xw_rest(t - 1)
                if t == NCH + 1:
                    # xW PSUM freed; attention pools take its place
                    xw_pool.__exit__(None, None, None)
                    attn_pool = tc.tile_pool(name="attn", bufs=2)
                    attnps_pool = tc.tile_pool(name="attnps", bufs=1, space="PSUM")
                    _apools["atp"] = attn_pool.__enter__()
                    _apools["atps"] = attnps_pool.__enter__()
                emit_step(t)
                flush_evicts()
                sb_prev = t // TB - 1
                ph = t % TB
                if sb_prev >= 0 and ph < 6:
                    emit_attention_chunk(sb_prev, ph)
                elif t >= TB + 6:
                    emit_qgroup()
                    emit_qgroup()
                    if t % 2 == 0:
                        emit_qgroup()
            # last super-block's attention
            for stp in range(6):
                emit_attention_chunk(NSB - 1, stp)
                emit_qgroup()
                flush_evicts()
            # drain: attention PSUM is free now, so switch to 2-bank pairs
            # (8 matmuls per PSUM tile, parallel DVE+ACT eviction, 1MB DMAs)
            attnps_pool.__exit__(None, None, None)
            attn_pool.__exit__(None, None, None)
            flush_evicts()
            tailps_pool = tc.tile_pool(name="tailps", bufs=2, space="PSUM")
            tps = tailps_pool.__enter__()

            def emit_tail_pair():
                mi, nq = qready.pop(0)
                mi2, nq2 = qready.pop(0)
                assert mi2 == mi and nq2 == nq + 1
                ps = tps.tile([128, 2, 512], f32, tag="tp")
                for half, nqh in enumerate((nq, nq2)):
                    for k in range(4):
                        nc.tensor.matmul(
                            ps[:, half, :NC_N],
                            zt[k][:, mi * 128 : (mi + 1) * 128],
                            wpre[:, k, nqh * NC_N : (nqh + 1) * NC_N],
                            start=(k == 0),
                            stop=(k == 3),
                        )
                cnt = qcount[0]
                qcount[0] += 1

                def evict():
                    o_sb = oqp.tile([128, 2, NC_N], bf16, tag="osb2")
                    nc.vector.tensor_copy(out=o_sb[:, 0, :], in_=ps[:, 0, :NC_N])
                    nc.scalar.copy(out=o_sb[:, 1, :], in_=ps[:, 1, :NC_N])
                    dmaq[cnt % 3].dma_start(
                        out=out_d[
                            mi * 128 : (mi + 1) * 128, nq * NC_N : (nq + 2) * NC_N
                        ],
                        in_=o_sb[:].rearrange("p g n -> p (g n)"),
                    )

                pending_ev.append(evict)

            if qready and qready[0][1] % 2 == 1:
                emit_qgroup()     # re-align to even n-chunk for pairing
            while len(qready) >= 2 and qready[0][0] == qready[1][0]:
                emit_tail_pair()
                if len(pending_ev) > 1:
                    pending_ev.pop(0)()
            while qready:
                emit_qgroup()
            flush_evicts()

            tailps_pool.__exit__(None, None, None)
            for pool in (
                outqps_pool, outq_pool, lstmps_pool, lstm_pool,
            ):
                pool.__exit__(None, None, None)
    nc.compile()
    return nc


def _prep_inputs(inputs):
    bf = ml_dtypes.bfloat16
    target = np.asarray(inputs["target_tensor"])
    enc = np.asarray(inputs["encoder_outputs"], dtype=np.float32)
    lens = np.asarray(inputs["encoder_seq_lens"])
    h0 = np.asarray(inputs["h0"], dtype=np.float32)
    c0 = np.asarray(inputs["c0"], dtype=np.float32)
    emb = np.ascontiguousarray(np.asarray(inputs["emb"], dtype=np.float32))
    W_ih = np.asarray(inputs["W_ih"], dtype=np.float32)
    W_hh = np.asarray(inputs["W_hh"], dtype=np.float32)
    bias = (
        np.asarray(inputs["b_ih"], dtype=np.float32)
        + np.asarray(inputs["b_hh"], dtype=np.float32)
    )
    # gate order (i, f, g, o) -> (o, i, f, g); o/i/f rows x0.5 (tanh trick);
    # all W_hh entries x0.5 again because the device streams Z = 2h
    perm = np.concatenate(
        [np.arange(3 * H, 4 * H), np.arange(0, 2 * H), np.arange(2 * H, 3 * H)]
    )
    rs = np.concatenate([np.full(3 * H, 0.5, np.float32), np.ones(H, np.float32)])
    W_ih = W_ih[perm] * rs[:, None]
    W_hh = W_hh[perm] * rs[:, None] * 0.5
    bias = bias[perm] * rs
    W_lin = np.asarray(inputs["W_lin"], dtype=np.float32)
    b_lin = np.asarray(inputs["b_lin"], dtype=np.float32)

    wihT = np.ascontiguousarray(W_ih.T.astype(bf))                # (E, 4H)
    whhT = np.ascontiguousarray(
        W_hh.T.reshape(2, 128, G4H).astype(bf)
    )                                                             # (2,128,4H)
    biasT = np.ascontiguousarray(bias.reshape(NCH, 128).T)        # (128, NCH)
    # h-columns of W_lin x0.5 (Z = 2h); ctx columns unscaled
    wlinT_full = W_lin.T.copy()
    wlinT_full[:H] *= 0.5
    wlinT_full = wlinT_full.astype(bf)                            # (512, V)

    in_maps = []
    for i in range(NCORES):
        bg = i % NBG
        vh = i // NBG
        sl = slice(bg * BL, (bg + 1) * BL)
        vsl = slice(vh * VL, (vh + 1) * VL)
        # t-major flat index = t*BL + b, laid out [128, NMT]
        tok = np.ascontiguousarray(
            target[sl].T.reshape(BT).reshape(NMT, 128).T.astype(np.int32)
        )
        enc_i = enc[sl]                                           # (BL, S, H)
        enc_sbh = np.ascontiguousarray(
            enc_i.transpose(1, 0, 2).astype(bf)
        )                                                         # (S, BL, H)
        encT = np.ascontiguousarray(
            enc_i.transpose(2, 0, 1).reshape(2, 128, BL, S).astype(bf)
        )                                                         # (2,128,BL,S)
        # device state carries 2*h0 / 2*c0
        h0T = np.ascontiguousarray(
            (2.0 * h0[sl]).T.reshape(2, 128, BL).transpose(1, 0, 2)
        )
        c0T = np.ascontiguousarray(
            (2.0 * c0[sl]).T.reshape(2, 128, BL).transpose(1, 0, 2)
        )
        # additive mask (0 valid / -30000 masked), broadcast over TB
        m01 = (np.arange(S)[:, None] < lens[sl][None, :]).astype(np.float32)
        mask24 = np.ascontiguousarray(
            np.broadcast_to(
                ((m01 - 1.0) * 30000.0)[:, :, None], (S, BL, TB)
            ).astype(np.float32)
        )
        wlinT = np.ascontiguousarray(
            wlinT_full[:, vsl].reshape(4, 128, VL)
        )                                                         # (4,128,VL)
        in_maps.append(
            {
                "tok": tok,
                "emb": emb,
                "enc": enc_sbh,
                "encT": encT,
                "h0T": h0T,
                "c0T": c0T,
                "mask24": mask24,
                "biasT": biasT,
                "wihT": wihT,
                "whhT": whhT,
                "wlinT": wlinT,
            }
        )
    return in_maps, b_lin


LAST_RESULTS = None


def _install_ntff_shim():
    """Provide antenv.axon_hooks if the image's antenv lacks it, so
    trace=True/BASS_TRACE=1 can capture NTFF profiles under axon."""
    import sys
    import types

    try:
        from antenv.axon_hooks import get_axon_ntff_profile_hook  # noqa: F401

        return
    except ImportError:
        pass
    try:
        from trn_agent_boot.trn_boot import _ntff_profile_via_ctypes

        hook = _ntff_profile_via_ctypes("/opt/axon/libaxon_pjrt.so")
        m = types.ModuleType("antenv.axon_hooks")
        m.get_axon_ntff_profile_hook = lambda: hook
        m.set_axon_ntff_profile_hook = lambda h: None
        sys.modules["antenv.axon_hooks"] = m
    except Exception:
        pass


def kernel(**inputs):
    global LAST_RESULTS
    _install_ntff_shim()
    if "nc" not in _CACHE:
        _CACHE["nc"] = _build()
    nc = _CACHE["nc"]
    in_maps, b_lin = _prep_inputs(inputs)
    res = run_bass_kernel_spmd(nc, in_maps, core_ids=list(range(NCORES)))
    LAST_RESULTS = res
    out = np.empty((B, T, V), dtype=np.float32)
    for i in range(NCORES):
        bg = i % NBG
        vh = i // NBG
        vsl = slice(vh * VL, (vh + 1) * VL)
        # logits rows are (superblock, batch, t_in); reorder to (b, t)
        lg = (
            res.results[i]["logits"]
            .astype(np.float32)
            .reshape(NSB, BL, TB, VL)
            .transpose(1, 0, 2, 3)
            .reshape(BL, T, VL)
        )
        out[bg * BL : (bg + 1) * BL, :, vsl] = lg + b_lin[None, None, vsl]
    return out


# revision 39
# speedup vs baseline: 1.2489x; 1.0774x over previous
"""Trainium2 Bass kernel for DecoderAttnRNN (LSTM + attention decoder).

Sharding: hybrid over 8 cores = 4 batch-groups x 2 vocab-halves.
Each core handles 16 batches and 16000 vocab columns.

v4 design — fused pipeline:
  phase 0: embedding gather (deep-pipelined indirect DMA), transpose,
           x@W_ih.T+bias precompute split so steps 0-7 unblock early
  fused loop over 72 LSTM steps in 3 super-blocks of 24 steps:
    - LSTM recurrence using ONLY tanh (sigmoid folded via
      sig(x) = (tanh(x/2)+1)/2 with all x0.5/x2 rescales folded into the
      host-side weights), so attention's exp shares one ACT table set
    - after each super-block: attention for its 24 timesteps
    - logits quarter-groups (4 matmuls -> 500 bf16 cols -> DMA) for
      completed super-blocks are interleaved into the tensor-engine idle
      gaps of later LSTM steps; this also keeps the PE HAM-warm
  tail: remaining logits quarter-groups back-to-back
Output rows are in (superblock, batch, t_in) order; host reorders, upcasts
bf16 -> f32 and adds b_lin.

LSTM cell with stored state C = 2c, Z = 2h, gate order (o, i, f, g):
  t8 = tanh([psum + xw])        (o,i,f rows pre-scaled x0.5 on host)
  AB = (t8[i,f] + 1) * [t8[g] | C]   -> [A | B] = [2*sig_i*tanh_g | 4*sig_f*c]
  C' = 0.5*B + A                (= 2*c_new)
  th = tanh(0.5*C')             (= tanh(c_new))
  Z  = (t8[o] + 1) * th         (= 2*h_new; W_hh, scores-scale, W_lin
                                   h-columns absorb the factor 2)
"""

import numpy as np
import ml_dtypes

import concourse.bass as bass
import concourse.mybir as mybir
import concourse.tile as tile
from concourse import bacc
from concourse.bass_utils import run_bass_kernel_spmd
from concourse.masks import make_identity

B, T, S, E, H, V = 64, 72, 72, 128, 256, 32000
NCORES = 8
NBG = 4                   # batch groups
NVH = 2                   # vocab halves
BL = B // NBG             # 16 batches per core
BT = BL * T               # 1152
VL = V // NVH             # 16000 vocab cols per core
G4H = 4 * H               # 1024
NCH = G4H // 128          # 8 gate chunks of 128
NC_N = 500                # logits n-chunk (one PSUM bank)
NQG_N = VL // NC_N        # 32 n-chunks per m-tile
NMT = BT // 128           # 9 m-tiles
NSB = 3                   # super-blocks of the time axis
TB = T // NSB             # 24 steps per super-block
SBR = BL * TB             # 384 logits rows per super-block (= 3 m-tiles)
MPS = SBR // 128          # m-tiles per super-block

f32 = mybir.dt.float32
bf16 = mybir.dt.bfloat16
i32 = mybir.dt.int32

_CACHE = {}


def _build():
    nc = bacc.Bacc(None, target_bir_lowering=False)

    tok_d = nc.declare_dram_parameter("tok", [128, NMT], i32, isOutput=False)
    emb_d = nc.declare_dram_parameter("emb", [V, E], f32, isOutput=False)
    enc_d = nc.declare_dram_parameter("enc", [S, BL, H], bf16, isOutput=False)
    encT_d = nc.declare_dram_parameter("encT", [2, 128, BL, S], bf16, isOutput=False)
    h0T_d = nc.declare_dram_parameter("h0T", [128, 2, BL], f32, isOutput=False)
    c0T_d = nc.declare_dram_parameter("c0T", [128, 2, BL], f32, isOutput=False)
    mask_d = nc.declare_dram_parameter("mask24", [S, BL, TB], f32, isOutput=False)
    biasT_d = nc.declare_dram_parameter("biasT", [128, NCH], f32, isOutput=False)
    wihT_d = nc.declare_dram_parameter("wihT", [E, G4H], bf16, isOutput=False)
    whhT_d = nc.declare_dram_parameter("whhT", [2, 128, G4H], bf16, isOutput=False)
    wlinT_d = nc.declare_dram_parameter("wlinT", [4, 128, VL], bf16, isOutput=False)
    out_d = nc.declare_dram_parameter("logits", [BT, VL], bf16, isOutput=True)

    with tile.TileContext(nc) as tc:
        with tc.tile_pool(name="persist", bufs=1) as pp:
            # ---- setup DMAs: phase-0-critical loads first ----
            tok_sb = pp.tile([128, NMT], i32)
            nc.sync.dma_start(out=tok_sb[:], in_=tok_d[:])
            wih_sb = pp.tile([128, G4H], bf16)
            nc.scalar.dma_start(out=wih_sb[:], in_=wihT_d[:])
            biasT_sb = pp.tile([128, NCH], f32)
            nc.scalar.dma_start(out=biasT_sb[:], in_=biasT_d[:])
            whh_sb = pp.tile([128, 2, G4H], bf16)
            for k in range(2):
                nc.scalar.dma_start(out=whh_sb[:, k, :], in_=whhT_d[k])
            # tg8 holds the 8 tanh'd gate chunks (o,i,f,g) plus C=2c in
            # slots 8:10, so one fused op computes both cell products.
            # These tiny state loads ride the gpsimd queue so they cannot
            # queue behind the multi-MB weight streams.
            tg8 = pp.tile([128, 10, BL], f32)
            nc.gpsimd.dma_start(out=tg8[:, 8:10], in_=c0T_d[:])
            h0f = pp.tile([128, 2, BL], f32)
            nc.gpsimd.dma_start(out=h0f[:], in_=h0T_d[:])
            h_init = pp.tile([128, 2, BL], bf16)
            mask_sb = pp.tile([S, BL, TB], f32)
            nc.gpsimd.dma_start(out=mask_sb[:], in_=mask_d[:])

            ident = pp.tile([128, 128], f32)
            make_identity(nc, ident[:])

            xwT = pp.tile([128, T, NCH, BL], bf16)       # x@W_ih.T + bias
            z01 = pp.tile([128, 2, NSB, BL, TB], bf16)   # Z=2h (k-tiles 0,1)
            z23 = pp.tile([128, 2, NSB, BL, TB], bf16)   # ctx (k-tiles 2,3)
            x_allT = pp.tile([128, BT], bf16)

            # bulk loads, needed later; queued behind the critical ones
            encT_sb = pp.tile([128, 2, BL, S], bf16)
            for k in range(2):
                nc.sync.dma_start(out=encT_sb[:, k], in_=encT_d[k])
            enc_sb = pp.tile([S, BL, H], bf16)
            nc.scalar.dma_start(out=enc_sb[:], in_=enc_d[:])
            wpre = pp.tile([128, 4, VL], bf16)
            for k in range(4):
                eng = nc.sync if k % 2 == 0 else nc.scalar
                eng.dma_start(out=wpre[:, k, :], in_=wlinT_d[k])

            ones_col = pp.tile([S, 16], bf16)
            ones_row_f = pp.tile([1, 128], f32)
            nc.vector.memset(ones_col[:], 1.0)
            nc.vector.memset(ones_row_f[:], 1.0)

            # ---- phase 0: embedding gather + transpose (deep pipeline) ----
            with (
                tc.tile_pool(name="p0", bufs=4) as wp,
                tc.tile_pool(name="p0ps", bufs=2, space="PSUM") as psp,
            ):
                casts = []
                for j in range(NMT):
                    x_t = wp.tile([128, E], f32, tag="x")
                    nc.gpsimd.indirect_dma_start(
                        out=x_t[:],
                        out_offset=None,
                        in_=emb_d[:],
                        in_offset=bass.IndirectOffsetOnAxis(
                            ap=tok_sb[:, j : j + 1], axis=0
                        ),
                    )
                    ps_t = psp.tile([128, 128], f32, tag="pst")
                    nc.tensor.transpose(out=ps_t[:], in_=x_t[:], identity=ident[:])
                    nc.vector.tensor_copy(
                        out=x_allT[:, j * 128 : (j + 1) * 128], in_=ps_t[:]
                    )

                # early xW for t<8 (x_allT cols 0:128) so the LSTM can start
                ps_xw8 = psp.tile([128, NCH, 128], f32, tag="psxw8")
                for c in range(NCH):
                    nc.tensor.matmul(
                        ps_xw8[:, c, :],
                        wih_sb[:, c * 128 : (c + 1) * 128],
                        x_allT[:, 0:128],
                        start=True,
                        stop=True,
                    )
                for c in range(NCH):
                    nc.vector.tensor_scalar(
                        out=xwT[:, 0:8, c, :],
                        in0=ps_xw8[:, c, :].rearrange("p (t b) -> p t b", b=BL),
                        scalar1=biasT_sb[:, c : c + 1],
                        scalar2=None,
                        op0=mybir.AluOpType.add,
                    )

            # h_init conversion deferred to here so the copy never blocks the
            # gather-cast pipeline at the head of the DVE queue
            nc.vector.tensor_copy(out=h_init[:], in_=h0f[:])

            def emit_xw_rest(c):
                # two 512-wide passes to keep PSUM small (32 t per pass)
                for half in range(2):
                    t0 = 8 + 32 * half
                    ps_xw = xwp.tile([128, 512], f32, tag="psxw")
                    nc.tensor.matmul(
                        ps_xw[:],
                        wih_sb[:, c * 128 : (c + 1) * 128],
                        x_allT[:, t0 * BL : (t0 + 32) * BL],
                        start=True,
                        stop=True,
                    )
                    nc.vector.tensor_scalar(
                        out=xwT[:, t0 : t0 + 32, c, :],
                        in0=ps_xw[:].rearrange("p (t b) -> p t b", b=BL),
                        scalar1=biasT_sb[:, c : c + 1],
                        scalar2=None,
                        op0=mybir.AluOpType.add,
                    )

            # ---- fused loop: LSTM steps + per-superblock attention +
            #      interleaved logits quarter-groups ----
            zt = [
                z01[:, 0].rearrange("p s b t -> p (s b t)"),
                z01[:, 1].rearrange("p s b t -> p (s b t)"),
                z23[:, 0].rearrange("p s b t -> p (s b t)"),
                z23[:, 1].rearrange("p s b t -> p (s b t)"),
            ]
            qready = []          # (mi, ng) logits quarter-groups ready to run
            qcount = [0]
            _attn_state = {}
            _apools = {}

            lstm_pool = tc.tile_pool(name="lstm", bufs=3)
            lp = lstm_pool.__enter__()
            lstmps_pool = tc.tile_pool(name="lstmps", bufs=2, space="PSUM")
            lps = lstmps_pool.__enter__()
            outq_pool = tc.tile_pool(name="outq", bufs=3)
            oqp = outq_pool.__enter__()
            outqps_pool = tc.tile_pool(name="outqps", bufs=2, space="PSUM")
            oqps = outqps_pool.__enter__()
            # innermost: remainder-xW PSUM, released once the attention pools
            # are needed (pools close in stack order)
            xw_pool = tc.tile_pool(name="pxw", bufs=2, space="PSUM")
            xwp = xw_pool.__enter__()
            dmaq = [nc.sync, nc.scalar, nc.gpsimd]

            pending_ev = []

            def emit_qgroup():
                # matmuls now; the eviction+DMA is deferred so the scheduler
                # gives the next LSTM step's chain ops priority over it
                if not qready:
                    return
                mi, nq = qready.pop(0)
                n0 = nq * NC_N
                ps_o = oqps.tile([128, 512], f32, tag="po")
                for k in range(4):
                    nc.tensor.matmul(
                        ps_o[:, :NC_N],
                        zt[k][:, mi * 128 : (mi + 1) * 128],
                        wpre[:, k, n0 : n0 + NC_N],
                        start=(k == 0),
                        stop=(k == 3),
                    )
                cnt = qcount[0]
                qcount[0] += 1

                def evict():
                    o_sb = oqp.tile([128, NC_N], bf16, tag="osb")
                    eng = nc.vector.tensor_copy if cnt % 2 == 0 else nc.scalar.copy
                    eng(out=o_sb[:], in_=ps_o[:, :NC_N])
                    dmaq[cnt % 3].dma_start(
                        out=out_d[mi * 128 : (mi + 1) * 128, n0 : n0 + NC_N],
                        in_=o_sb[:],
                    )

                pending_ev.append(evict)

            def flush_evicts():
                while pending_ev:
                    pending_ev.pop(0)()

            def emit_step(t):
                sb, ti = divmod(t, TB)
                ps_g = lps.tile([128, NCH, BL], f32, tag="psg")
                for c in range(NCH):
                    for k in range(2):
                        rhs = (
                            h_init[:, k, :] if t == 0
                            else z01[:, k, (t - 1) // TB, :, (t - 1) % TB]
                        )
                        nc.tensor.matmul(
                            ps_g[:, c, :],
                            whh_sb[:, k, c * 128 : (c + 1) * 128],
                            rhs,
                            start=(k == 0),
                            stop=(k == 1),
                        )
                gsum = lp.tile([128, NCH, BL], f32, tag="gsum")
                nc.vector.tensor_tensor(
                    out=gsum[:], in0=ps_g[:], in1=xwT[:, t],
                    op=mybir.AluOpType.add,
                )
                # single tanh over all gates (o,i,f pre-scaled x0.5 on host)
                nc.scalar.activation(
                    out=tg8[:, 0:8], in_=gsum[:],
                    func=mybir.ActivationFunctionType.Tanh,
                )
                # AB = (t8[i,f]+1) * [t8[g] | C]
                ab = lp.tile([128, 4, BL], f32, tag="ab")
                nc.vector.scalar_tensor_tensor(
                    out=ab[:], in0=tg8[:, 2:6], scalar=1.0, in1=tg8[:, 6:10],
                    op0=mybir.AluOpType.add, op1=mybir.AluOpType.mult,
                )
                # C' = 0.5*B + A
                nc.vector.scalar_tensor_tensor(
                    out=tg8[:, 8:10], in0=ab[:, 2:4], scalar=0.5, in1=ab[:, 0:2],
                    op0=mybir.AluOpType.mult, op1=mybir.AluOpType.add,
                )
                th = lp.tile([128, 2, BL], f32, tag="th")
                nc.scalar.activation(
                    out=th[:], in_=tg8[:, 8:10],
                    func=mybir.ActivationFunctionType.Tanh, scale=0.5,
                )
                # Z = (t8[o]+1) * th  (bf16, = 2*h)
                nc.vector.scalar_tensor_tensor(
                    out=z01[:, :, sb, :, ti], in0=tg8[:, 0:2], scalar=1.0,
                    in1=th[:], op0=mybir.AluOpType.add, op1=mybir.AluOpType.mult,
                )

            def emit_attention_chunk(sb, step):
                # attention for super-block sb, split into 6 chunks emitted
                # across consecutive later steps to bound PE-queue delay
                atp = _apools["atp"]
                atps = _apools["atps"]
                if step == 0:
                    st = atps.tile([S, BL, 32], f32, tag="ps_s")
                    exb = atp.tile([S, BL, TB], bf16, tag="exb")
                    rcp = atp.tile([1, BL, TB], f32, tag="rcp")
                    att = atp.tile([S, BL, TB], bf16, tag="att")
                    den = atps.tile([16, 4, 128], f32, tag="den")
                    bc = atps.tile([128, 512], f32, tag="bc")
                    ctx = atps.tile([128, 2, 256], f32, tag="ctx")
                    _attn_state[sb] = (st, exb, rcp, att, den, bc, ctx)
                st, exb, rcp, att, den, bc, ctx = _attn_state[sb]
                if step in (0, 1):
                    for b in range(8 * step, 8 * (step + 1)):
                        for k in range(2):
                            nc.tensor.matmul(
                                st[:, b, :TB],
                                encT_sb[:, k, b, :],
                                z01[:, k, sb, b, :],
                                start=(k == 0),
                                stop=(k == 1),
                            )
                elif step == 2:
                    # masked exp over all (b,t) of the block; Z=2h so the
                    # score scale halves
                    nc.vector.tensor_tensor(
                        out=exb[:], in0=st[:, :, :TB], in1=mask_sb[:],
                        op=mybir.AluOpType.add,
                    )
                    nc.scalar.activation(
                        out=exb[:], in_=exb[:],
                        func=mybir.ActivationFunctionType.Exp,
                        scale=float(0.5 / np.sqrt(H)),
                    )
                elif step == 3:
                    for g in range(4):
                        nc.tensor.matmul(
                            den[:, g, : 4 * TB],
                            ones_col[:],
                            exb[:, 4 * g : 4 * (g + 1), :].rearrange(
                                "p b t -> p (b t)"
                            ),
                            start=True,
                            stop=True,
                        )
                        nc.vector.reciprocal(
                            out=rcp[:, 4 * g : 4 * (g + 1), :].rearrange(
                                "p b t -> p (b t)"
                            ),
                            in_=den[0:1, g, : 4 * TB],
                        )
                    nc.tensor.matmul(
                        bc[:, : BL * TB], ones_row_f[:],
                        rcp[:].rearrange("p b t -> p (b t)"),
                        start=True, stop=True,
                    )
                    nc.vector.tensor_tensor(
                        out=att[:].rearrange("p b t -> p (b t)"),
                        in0=exb[:].rearrange("p b t -> p (b t)"),
                        in1=bc[0:S, : BL * TB],
                        op=mybir.AluOpType.mult,
                    )
                elif step in (4, 5):
                    # half the batches per chunk; evict frees the PSUM tile
                    # for the second half (attnps has bufs=1)
                    b0 = 8 * (step - 4)
                    for b in range(b0, b0 + 8):
                        for j in range(2):
                            nc.tensor.matmul(
                                ctx[:, j, (b - b0) * TB : (b - b0 + 1) * TB],
                                enc_sb[:, b, j * 128 : (j + 1) * 128],
                                att[:, b, :],
                                start=True,
                                stop=True,
                            )
                    nc.vector.tensor_copy(
                        out=z23[:, :, sb, b0 : b0 + 8, :].rearrange(
                            "p k b t -> p k (b t)"
                        ),
                        in_=ctx[:, :, : 8 * TB],
                    )
                    if step == 5:
                        for mi in range(sb * MPS, (sb + 1) * MPS):
                            for nq in range(NQG_N):
                                qready.append((mi, nq))

            # ---- emit the fused schedule ----
            for t in range(T):
                if 1 <= t <= NCH:
                    emit_xw_rest(t - 1)
                if t == NCH + 1:
                    # xW PSUM freed; attention pools take its place
                    xw_pool.__exit__(None, None, None)
                    attn_pool = tc.tile_pool(name="attn", bufs=2)
                    attnps_pool = tc.tile_pool(name="attnps", bufs=1, space="PSUM")
                    _apools["atp"] = attn_pool.__enter__()
                    _apools["atps"] = attnps_pool.__enter__()
                emit_step(t)
                flush_evicts()
                sb_prev = t // TB - 1
                ph = t % TB
                if sb_prev >= 0 and ph < 6:
                    emit_attention_chunk(sb_prev, ph)
                elif t >= TB + 6:
                    emit_qgroup()
                    emit_qgroup()
                    if t % 2 == 0:
                        emit_qgroup()
            # last super-block's attention
            for stp in range(6):
                emit_attention_chunk(NSB - 1, stp)
                emit_qgroup()
                flush_evicts()
            # drain: attention PSUM is free now, so switch to 2-bank pairs
            # (8 matmuls per PSUM tile, parallel DVE+ACT eviction, 1MB DMAs)
            attnps_pool.__exit__(None, None, None)
            attn_pool.__exit__(None, None, None)
            flush_evicts()
            tailps_pool = tc.tile_pool(name="tailps", bufs=2, space="PSUM")
            tps = tailps_pool.__enter__()

            def emit_tail_pair():
                mi, nq = qready.pop(0)
                mi2, nq2 = qready.pop(0)
                assert mi2 == mi and nq2 == nq + 1
                ps = tps.tile([128, 2, 512], f32, tag="tp")
                for half, nqh in enumerate((nq, nq2)):
                    for k in range(4):
                        nc.tensor.matmul(
                            ps[:, half, :NC_N],
                            zt[k][:, mi * 128 : (mi + 1) * 128],
                            wpre[:, k, nqh * NC_N : (nqh + 1) * NC_N],
                            start=(k == 0),
                            stop=(k == 3),
                        )
                cnt = qcount[0]
                qcount[0] += 1

                def evict():
                    o_sb = oqp.tile([128, 2, NC_N], bf16, tag="osb2")
                    nc.vector.tensor_copy(out=o_sb[:, 0, :], in_=ps[:, 0, :NC_N])
                    nc.scalar.copy(out=o_sb[:, 1, :], in_=ps[:, 1, :NC_N])
                    dmaq[cnt % 3].dma_start(
                        out=out_d[
                            mi * 128 : (mi + 1) * 128, nq * NC_N : (nq + 2) * NC_N
                        ],
                        in_=o_sb[:].rearrange("p g n -> p (g n)"),
                    )

                pending_ev.append(evict)

            if qready and qready[0][1] % 2 == 1:
                emit_qgroup()     # re-align to even n-chunk for pairing
            while len(qready) >= 2 and qready[0][0] == qready[1][0]:
                emit_tail_pair()
                if len(pending_ev) > 1:
                    pending_ev.pop(0)()
            while qready:
                emit_qgroup()
            flush_evicts()

            tailps_pool.__exit__(None, None, None)
            for pool in (
                outqps_pool, outq_pool, lstmps_pool, lstm_pool,
            ):
                pool.__exit__(None, None, None)
    nc.compile()
    return nc


def _prep_inputs(inputs):
    bf = ml_dtypes.bfloat16
    target = np.asarray(inputs["target_tensor"])
    enc = np.asarray(inputs["encoder_outputs"], dtype=np.float32)
    lens = np.asarray(inputs["encoder_seq_lens"])
    h0 = np.asarray(inputs["h0"], dtype=np.float32)
    c0 = np.asarray(inputs["c0"], dtype=np.float32)
    emb = np.ascontiguousarray(np.asarray(inputs["emb"], dtype=np.float32))
    W_ih = np.asarray(inputs["W_ih"], dtype=np.float32)
    W_hh = np.asarray(inputs["W_hh"], dtype=np.float32)
    bias = (
        np.asarray(inputs["b_ih"], dtype=np.float32)
        + np.asarray(inputs["b_hh"], dtype=np.float32)
    )
    # gate order (i, f, g, o) -> (o, i, f, g); o/i/f rows x0.5 (tanh trick);
    # all W_hh entries x0.5 again because the device streams Z = 2h
    perm = np.concatenate(
        [np.arange(3 * H, 4 * H), np.arange(0, 2 * H), np.arange(2 * H, 3 * H)]
    )
    rs = np.concatenate([np.full(3 * H, 0.5, np.float32), np.ones(H, np.float32)])
    W_ih = W_ih[perm] * rs[:, None]
    W_hh = W_hh[perm] * rs[:, None] * 0.5
    bias = bias[perm] * rs
    W_lin = np.asarray(inputs["W_lin"], dtype=np.float32)
    b_lin = np.asarray(inputs["b_lin"], dtype=np.float32)

    wihT = np.ascontiguousarray(W_ih.T.astype(bf))                # (E, 4H)
    whhT = np.ascontiguousarray(
        W_hh.T.reshape(2, 128, G4H).astype(bf)
    )                                                             # (2,128,4H)
    biasT = np.ascontiguousarray(bias.reshape(NCH, 128).T)        # (128, NCH)
    # h-columns of W_lin x0.5 (Z = 2h); ctx columns unscaled
    wlinT_full = W_lin.T.copy()
    wlinT_full[:H] *= 0.5
    wlinT_full = wlinT_full.astype(bf)                            # (512, V)

    in_maps = []
    for i in range(NCORES):
        bg = i % NBG
        vh = i // NBG
        sl = slice(bg * BL, (bg + 1) * BL)
        vsl = slice(vh * VL, (vh + 1) * VL)
        # t-major flat index = t*BL + b, laid out [128, NMT]
        tok = np.ascontiguousarray(
            target[sl].T.reshape(BT).reshape(NMT, 128).T.astype(np.int32)
        )
        enc_i = enc[sl]                                           # (BL, S, H)
        enc_sbh = np.ascontiguousarray(
            enc_i.transpose(1, 0, 2).astype(bf)
        )                                                         # (S, BL, H)
        encT = np.ascontiguousarray(
            enc_i.transpose(2, 0, 1).reshape(2, 128, BL, S).astype(bf)
        )                                                         # (2,128,BL,S)
        # device state carries 2*h0 / 2*c0
        h0T = np.ascontiguousarray(
            (2.0 * h0[sl]).T.reshape(2, 128, BL).transpose(1, 0, 2)
        )
        c0T = np.ascontiguousarray(
            (2.0 * c0[sl]).T.reshape(2, 128, BL).transpose(1, 0, 2)
        )
        # additive mask (0 valid / -30000 masked), broadcast over TB
        m01 = (np.arange(S)[:, None] < lens[sl][None, :]).astype(np.float32)
        mask24 = np.ascontiguousarray(
            np.broadcast_to(
                ((m01 - 1.0) * 30000.0)[:, :, None], (S, BL, TB)
            ).astype(np.float32)
        )
        wlinT = np.ascontiguousarray(
            wlinT_full[:, vsl].reshape(4, 128, VL)
        )                                                         # (4,128,VL)
        in_maps.append(
            {
                "tok": tok,
                "emb": emb,
                "enc": enc_sbh,
                "encT": encT,
                "h0T": h0T,
                "c0T": c0T,
                "mask24": mask24,
                "biasT": biasT,
                "wihT": wihT,
                "whhT": whhT,
                "wlinT": wlinT,
            }
        )
    return in_maps, b_lin


LAST_RESULTS = None


def _install_ntff_shim():
    """Provide antenv.axon_hooks if the image's antenv lacks it, so
    trace=True/BASS_TRACE=1 can capture NTFF profiles under axon."""
    import sys
    import types

    try:
        from antenv.axon_hooks import get_axon_ntff_profile_hook  # noqa: F401

        return
    except ImportError:
        pass
    try:
        from trn_agent_boot.trn_boot import _ntff_profile_via_ctypes

        hook = _ntff_profile_via_ctypes("/opt/axon/libaxon_pjrt.so")
        m = types.ModuleType("antenv.axon_hooks")
        m.get_axon_ntff_profile_hook = lambda: hook
        m.set_axon_ntff_profile_hook = lambda h: None
        sys.modules["antenv.axon_hooks"] = m
    except Exception:
        pass


def kernel(**inputs):
    global LAST_RESULTS
    _install_ntff_shim()
    if "nc" not in _CACHE:
        _CACHE["nc"] = _build()
    nc = _CACHE["nc"]
    in_maps, b_lin = _prep_inputs(inputs)
    res = run_bass_kernel_spmd(nc, in_maps, core_ids=list(range(NCORES)))
    LAST_RESULTS = res
    out = np.empty((B, T, V), dtype=np.float32)
    for i in range(NCORES):
        bg = i % NBG
        vh = i // NBG
        vsl = slice(vh * VL, (vh + 1) * VL)
        # logits rows are (superblock, batch, t_in); reorder to (b, t)
        lg = (
            res.results[i]["logits"]
            .astype(np.float32)
            .reshape(NSB, BL, TB, VL)
            .transpose(1, 0, 2, 3)
            .reshape(BL, T, VL)
        )
        out[bg * BL : (bg + 1) * BL, :, vsl] = lg + b_lin[None, None, vsl]
    return out


# revision 46
# speedup vs baseline: 1.3283x; 1.0636x over previous
"""Trainium2 Bass kernel for DecoderAttnRNN (LSTM + attention decoder).

Sharding: hybrid over 8 cores = 4 batch-groups x 2 vocab-halves.
Each core handles 16 batches and 16000 vocab columns.

v4 design — fused pipeline:
  phase 0: embedding gather (deep-pipelined indirect DMA), transpose,
           x@W_ih.T+bias precompute split so steps 0-7 unblock early
  fused loop over 72 LSTM steps in 3 super-blocks of 24 steps:
    - LSTM recurrence using ONLY tanh (sigmoid folded via
      sig(x) = (tanh(x/2)+1)/2 with all x0.5/x2 rescales folded into the
      host-side weights), so attention's exp shares one ACT table set
    - after each super-block: attention for its 24 timesteps
    - logits quarter-groups (4 matmuls -> 500 bf16 cols -> DMA) for
      completed super-blocks are interleaved into the tensor-engine idle
      gaps of later LSTM steps; this also keeps the PE HAM-warm
  tail: remaining logits quarter-groups back-to-back
Output rows are in (superblock, batch, t_in) order; host reorders, upcasts
bf16 -> f32 and adds b_lin.

LSTM cell with stored state C = 2c, Z = 2h, gate order (o, i, f, g):
  t8 = tanh([psum + xw])        (o,i,f rows pre-scaled x0.5 on host)
  AB = (t8[i,f] + 1) * [t8[g] | C]   -> [A | B] = [2*sig_i*tanh_g | 4*sig_f*c]
  C' = 0.5*B + A                (= 2*c_new)
  th = tanh(0.5*C')             (= tanh(c_new))
  Z  = (t8[o] + 1) * th         (= 2*h_new; W_hh, scores-scale, W_lin
                                   h-columns absorb the factor 2)
"""

import numpy as np
import ml_dtypes

import concourse.bass as bass
import concourse.mybir as mybir
import concourse.tile as tile
from concourse import bacc
from concourse.bass_utils import run_bass_kernel_spmd
from concourse.masks import make_identity

B, T, S, E, H, V = 64, 72, 72, 128, 256, 32000
NCORES = 8
NBG = 4                   # batch groups
NVH = 2                   # vocab halves
BL = B // NBG             # 16 batches per core
BT = BL * T               # 1152
VL = V // NVH             # 16000 vocab cols per core
G4H = 4 * H               # 1024
NCH = G4H // 128          # 8 gate chunks of 128
NC_N = 500                # logits n-chunk (one PSUM bank)
NQG_N = VL // NC_N        # 32 n-chunks per m-tile
NMT = BT // 128           # 9 m-tiles
NSB = 3                   # super-blocks of the time axis
TB = T // NSB             # 24 steps per super-block
SBR = BL * TB             # 384 logits rows per super-block (= 3 m-tiles)
MPS = SBR // 128          # m-tiles per super-block

f32 = mybir.dt.float32
bf16 = mybir.dt.bfloat16
i32 = mybir.dt.int32

_CACHE = {}


def _build():
    nc = bacc.Bacc(None, target_bir_lowering=False)

    tok_d = nc.declare_dram_parameter("tok", [128, NMT], i32, isOutput=False)
    emb_d = nc.declare_dram_parameter("emb", [V, E], f32, isOutput=False)
    enc_d = nc.declare_dram_parameter("enc", [S, BL, H], bf16, isOutput=False)
    encT_d = nc.declare_dram_parameter("encT", [2, 128, BL, S], bf16, isOutput=False)
    h0T_d = nc.declare_dram_parameter("h0T", [128, 2, BL], f32, isOutput=False)
    c0T_d = nc.declare_dram_parameter("c0T", [128, 2, BL], f32, isOutput=False)
    mask_d = nc.declare_dram_parameter("mask24", [S, BL, TB], f32, isOutput=False)
    biasT_d = nc.declare_dram_parameter("biasT", [128, NCH], f32, isOutput=False)
    wihT_d = nc.declare_dram_parameter("wihT", [E, G4H], bf16, isOutput=False)
    whhT_d = nc.declare_dram_parameter("whhT", [2, 128, G4H], bf16, isOutput=False)
    wlinT_d = nc.declare_dram_parameter("wlinT", [4, 128, VL], bf16, isOutput=False)
    out_d = nc.declare_dram_parameter("logits", [BT, VL], bf16, isOutput=True)

    with tile.TileContext(nc) as tc:
        with tc.tile_pool(name="persist", bufs=1) as pp:
            # ---- setup DMAs: phase-0-critical loads first ----
            tok_sb = pp.tile([128, NMT], i32)
            nc.sync.dma_start(out=tok_sb[:], in_=tok_d[:])
            wih_sb = pp.tile([128, G4H], bf16)
            nc.scalar.dma_start(out=wih_sb[:], in_=wihT_d[:])
            biasT_sb = pp.tile([128, NCH], f32)
            nc.scalar.dma_start(out=biasT_sb[:], in_=biasT_d[:])
            whh_sb = pp.tile([128, 2, G4H], bf16)
            for k in range(2):
                nc.scalar.dma_start(out=whh_sb[:, k, :], in_=whhT_d[k])
            # tg8 holds the 8 tanh'd gate chunks (o,i,f,g) plus C=2c in
            # slots 8:10, so one fused op computes both cell products.
            # These tiny state loads ride the gpsimd queue so they cannot
            # queue behind the multi-MB weight streams.
            tg8 = pp.tile([128, 10, BL], f32)
            nc.gpsimd.dma_start(out=tg8[:, 8:10], in_=c0T_d[:])
            h0f = pp.tile([128, 2, BL], f32)
            nc.gpsimd.dma_start(out=h0f[:], in_=h0T_d[:])
            h_init = pp.tile([128, 2, BL], bf16)
            mask_sb = pp.tile([S, BL, TB], f32)
            nc.gpsimd.dma_start(out=mask_sb[:], in_=mask_d[:])

            ident = pp.tile([128, 128], f32)
            make_identity(nc, ident[:])

            xwT = pp.tile([128, T, NCH, BL], bf16)       # x@W_ih.T + bias
            z01 = pp.tile([128, 2, NSB, BL, TB], bf16)   # Z=2h (k-tiles 0,1)
            z23 = pp.tile([128, 2, NSB, BL, TB], bf16)   # ctx (k-tiles 2,3)
            x_allT = pp.tile([128, BT], bf16)

            # bulk loads, needed later; queued behind the critical ones
            encT_sb = pp.tile([128, 2, BL, S], bf16)
            for k in range(2):
                nc.sync.dma_start(out=encT_sb[:, k], in_=encT_d[k])
            enc_sb = pp.tile([S, BL, H], bf16)
            nc.scalar.dma_start(out=enc_sb[:], in_=enc_d[:])
            wpre = pp.tile([128, 4, VL], bf16)

            ones_col = pp.tile([S, 16], bf16)
            ones_row_f = pp.tile([1, 128], f32)
            nc.vector.memset(ones_col[:], 1.0)
            nc.vector.memset(ones_row_f[:], 1.0)

            # ---- phase 0: embedding gather + transpose (deep pipeline) ----
            with (
                tc.tile_pool(name="p0", bufs=NMT, space="SBUF") as wp,
                tc.tile_pool(name="p0ps", bufs=2, space="PSUM") as psp,
            ):
                for j in range(NMT):
                    x_t = wp.tile([128, E], f32, tag="x")
                    nc.gpsimd.indirect_dma_start(
                        out=x_t[:],
                        out_offset=None,
                        in_=emb_d[:],
                        in_offset=bass.IndirectOffsetOnAxis(
                            ap=tok_sb[:, j : j + 1], axis=0
                        ),
                    )
                    ps_t = psp.tile([128, 128], f32, tag="pst")
                    nc.tensor.transpose(out=ps_t[:], in_=x_t[:], identity=ident[:])
                    nc.vector.tensor_copy(
                        out=x_allT[:, j * 128 : (j + 1) * 128], in_=ps_t[:]
                    )

                # early xW for t<8 (x_allT cols 0:128) so the LSTM can start
                ps_xw8 = psp.tile([128, NCH, 128], f32, tag="psxw8")
                for c in range(NCH):
                    nc.tensor.matmul(
                        ps_xw8[:, c, :],
                        wih_sb[:, c * 128 : (c + 1) * 128],
                        x_allT[:, 0:128],
                        start=True,
                        stop=True,
                    )
                for c in range(NCH):
                    nc.vector.tensor_scalar(
                        out=xwT[:, 0:8, c, :],
                        in0=ps_xw8[:, c, :].rearrange("p (t b) -> p t b", b=BL),
                        scalar1=biasT_sb[:, c : c + 1],
                        scalar2=None,
                        op0=mybir.AluOpType.add,
                    )

            # h_init conversion deferred to here so the copy never blocks the
            # gather-cast pipeline at the head of the DVE queue
            nc.vector.tensor_copy(out=h_init[:], in_=h0f[:])

            # W_lin preload, gated behind the 5th gather-cast (the dummy
            # write creates the dependency) so the 16 MiB stream stays out
            # of the latency-critical early-phase-0 window; it has until
            # ~step 30 to finish
            for k in range(4):
                nc.vector.tensor_copy(
                    out=wpre[0:1, k, 0:1], in_=x_allT[0:1, 5 * 128 - 1 : 5 * 128]
                )
                eng = nc.sync if k % 2 == 0 else nc.scalar
                eng.dma_start(out=wpre[:, k, :], in_=wlinT_d[k])

            def emit_xw_rest(c):
                # two 512-wide passes to keep PSUM small (32 t per pass)
                for half in range(2):
                    t0 = 8 + 32 * half
                    ps_xw = xwp.tile([128, 512], f32, tag="psxw")
                    nc.tensor.matmul(
                        ps_xw[:],
                        wih_sb[:, c * 128 : (c + 1) * 128],
                        x_allT[:, t0 * BL : (t0 + 32) * BL],
                        start=True,
                        stop=True,
                    )
                    nc.vector.tensor_scalar(
                        out=xwT[:, t0 : t0 + 32, c, :],
                        in0=ps_xw[:].rearrange("p (t b) -> p t b", b=BL),
                        scalar1=biasT_sb[:, c : c + 1],
                        scalar2=None,
                        op0=mybir.AluOpType.add,
                    )

            # ---- fused loop: LSTM steps + per-superblock attention +
            #      interleaved logits quarter-groups ----
            zt = [
                z01[:, 0].rearrange("p s b t -> p (s b t)"),
                z01[:, 1].rearrange("p s b t -> p (s b t)"),
                z23[:, 0].rearrange("p s b t -> p (s b t)"),
                z23[:, 1].rearrange("p s b t -> p (s b t)"),
            ]
            qready = []          # (mi, ng) logits quarter-groups ready to run
            qcount = [0]
            _attn_state = {}
            _apools = {}

            # SBUF pools first, PSUM pools inside, so every PSUM pool can be
            # released before the drain (which then gets all 8 banks)
            lstm_pool = tc.tile_pool(name="lstm", bufs=3)
            lp = lstm_pool.__enter__()
            outq_pool = tc.tile_pool(name="outq", bufs=3)
            oqp = outq_pool.__enter__()
            lstmps_pool = tc.tile_pool(name="lstmps", bufs=2, space="PSUM")
            lps = lstmps_pool.__enter__()
            outqps_pool = tc.tile_pool(name="outqps", bufs=2, space="PSUM")
            oqps = outqps_pool.__enter__()
            # innermost: remainder-xW PSUM, released once the attention pools
            # are needed (pools close in stack order)
            xw_pool = tc.tile_pool(name="pxw", bufs=2, space="PSUM")
            xwp = xw_pool.__enter__()
            dmaq = [nc.sync, nc.scalar, nc.gpsimd]

            pending_ev = []

            def emit_qgroup():
                # matmuls now; the eviction+DMA is deferred so the scheduler
                # gives the next LSTM step's chain ops priority over it
                if not qready:
                    return
                mi, nq = qready.pop(0)
                n0 = nq * NC_N
                ps_o = oqps.tile([128, 512], f32, tag="po")
                for k in range(4):
                    nc.tensor.matmul(
                        ps_o[:, :NC_N],
                        zt[k][:, mi * 128 : (mi + 1) * 128],
                        wpre[:, k, n0 : n0 + NC_N],
                        start=(k == 0),
                        stop=(k == 3),
                    )
                cnt = qcount[0]
                qcount[0] += 1

                def evict():
                    o_sb = oqp.tile([128, NC_N], bf16, tag="osb")
                    eng = nc.vector.tensor_copy if cnt % 2 == 0 else nc.scalar.copy
                    eng(out=o_sb[:], in_=ps_o[:, :NC_N])
                    dmaq[cnt % 3].dma_start(
                        out=out_d[mi * 128 : (mi + 1) * 128, n0 : n0 + NC_N],
                        in_=o_sb[:],
                    )

                pending_ev.append(evict)

            def flush_evicts():
                while pending_ev:
                    pending_ev.pop(0)()

            def emit_step(t):
                with tc.high_priority(offset=600):
                    _emit_step_body(t)

            def _emit_step_body(t):
                sb, ti = divmod(t, TB)
                ps_g = lps.tile([128, NCH, BL], f32, tag="psg")
                for c in range(NCH):
                    for k in range(2):
                        rhs = (
                            h_init[:, k, :] if t == 0
                            else z01[:, k, (t - 1) // TB, :, (t - 1) % TB]
                        )
                        nc.tensor.matmul(
                            ps_g[:, c, :],
                            whh_sb[:, k, c * 128 : (c + 1) * 128],
                            rhs,
                            start=(k == 0),
                            stop=(k == 1),
                        )
                gsum = lp.tile([128, NCH, BL], f32, tag="gsum")
                nc.vector.tensor_tensor(
                    out=gsum[:], in0=ps_g[:], in1=xwT[:, t],
                    op=mybir.AluOpType.add,
                )
                # single tanh over all gates (o,i,f pre-scaled x0.5 on host)
                nc.scalar.activation(
                    out=tg8[:, 0:8], in_=gsum[:],
                    func=mybir.ActivationFunctionType.Tanh,
                )
                # AB = (t8[i,f]+1) * [t8[g] | C]
                ab = lp.tile([128, 4, BL], f32, tag="ab")
                nc.vector.scalar_tensor_tensor(
                    out=ab[:], in0=tg8[:, 2:6], scalar=1.0, in1=tg8[:, 6:10],
                    op0=mybir.AluOpType.add, op1=mybir.AluOpType.mult,
                )
                # C' = 0.5*B + A
                nc.vector.scalar_tensor_tensor(
                    out=tg8[:, 8:10], in0=ab[:, 2:4], scalar=0.5, in1=ab[:, 0:2],
                    op0=mybir.AluOpType.mult, op1=mybir.AluOpType.add,
                )
                th = lp.tile([128, 2, BL], f32, tag="th")
                nc.scalar.activation(
                    out=th[:], in_=tg8[:, 8:10],
                    func=mybir.ActivationFunctionType.Tanh, scale=0.5,
                )
                # Z = (t8[o]+1) * th  (bf16, = 2*h)
                nc.vector.scalar_tensor_tensor(
                    out=z01[:, :, sb, :, ti], in0=tg8[:, 0:2], scalar=1.0,
                    in1=th[:], op0=mybir.AluOpType.add, op1=mybir.AluOpType.mult,
                )

            def emit_attention_chunk(sb, step):
                # attention for super-block sb, split into 6 chunks emitted
                # across consecutive later steps to bound PE-queue delay
                atp = _apools["atp"]
                atps = _apools["atps"]
                if step == 0:
                    st = atps.tile([S, BL, 32], f32, tag="ps_s")
                    exb = atp.tile([S, BL, TB], bf16, tag="exb")
                    rcp = atp.tile([1, BL, TB], f32, tag="rcp")
                    att = atp.tile([S, BL, TB], bf16, tag="att")
                    den = atps.tile([16, 4, 128], f32, tag="den")
                    bc = atps.tile([128, 512], f32, tag="bc")
                    ctx = atps.tile([128, 2, 256], f32, tag="ctx")
                    _attn_state[sb] = (st, exb, rcp, att, den, bc, ctx)
                st, exb, rcp, att, den, bc, ctx = _attn_state[sb]
                if step in (0, 1):
                    for b in range(8 * step, 8 * (step + 1)):
                        for k in range(2):
                            nc.tensor.matmul(
                                st[:, b, :TB],
                                encT_sb[:, k, b, :],
                                z01[:, k, sb, b, :],
                                start=(k == 0),
                                stop=(k == 1),
                            )
                elif step == 2:
                    # masked exp over all (b,t) of the block; Z=2h so the
                    # score scale halves
                    nc.vector.tensor_tensor(
                        out=exb[:], in0=st[:, :, :TB], in1=mask_sb[:],
                        op=mybir.AluOpType.add,
                    )
                    nc.scalar.activation(
                        out=exb[:], in_=exb[:],
                        func=mybir.ActivationFunctionType.Exp,
                        scale=float(0.5 / np.sqrt(H)),
                    )
                elif step == 3:
                    for g in range(4):
                        nc.tensor.matmul(
                            den[:, g, : 4 * TB],
                            ones_col[:],
                            exb[:, 4 * g : 4 * (g + 1), :].rearrange(
                                "p b t -> p (b t)"
                            ),
                            start=True,
                            stop=True,
                        )
                        nc.vector.reciprocal(
                            out=rcp[:, 4 * g : 4 * (g + 1), :].rearrange(
                                "p b t -> p (b t)"
                            ),
                            in_=den[0:1, g, : 4 * TB],
                        )
                    nc.tensor.matmul(
                        bc[:, : BL * TB], ones_row_f[:],
                        rcp[:].rearrange("p b t -> p (b t)"),
                        start=True, stop=True,
                    )
                    nc.vector.tensor_tensor(
                        out=att[:].rearrange("p b t -> p (b t)"),
                        in0=exb[:].rearrange("p b t -> p (b t)"),
                        in1=bc[0:S, : BL * TB],
                        op=mybir.AluOpType.mult,
                    )
                elif step in (4, 5):
                    # half the batches per chunk; evict frees the PSUM tile
                    # for the second half (attnps has bufs=1)
                    b0 = 8 * (step - 4)
                    for b in range(b0, b0 + 8):
                        for j in range(2):
                            nc.tensor.matmul(
                                ctx[:, j, (b - b0) * TB : (b - b0 + 1) * TB],
                                enc_sb[:, b, j * 128 : (j + 1) * 128],
                                att[:, b, :],
                                start=True,
                                stop=True,
                            )
                    nc.vector.tensor_copy(
                        out=z23[:, :, sb, b0 : b0 + 8, :].rearrange(
                            "p k b t -> p k (b t)"
                        ),
                        in_=ctx[:, :, : 8 * TB],
                    )
                    if step == 5:
                        for mi in range(sb * MPS, (sb + 1) * MPS):
                            for nq in range(NQG_N):
                                qready.append((mi, nq))

            # ---- emit the fused schedule ----
            for t in range(T):
                if 1 <= t <= NCH:
                    emit_xw_rest(t - 1)
                if t == NCH + 1:
                    # xW PSUM freed; attention pools take its place
                    xw_pool.__exit__(None, None, None)
                    attn_pool = tc.tile_pool(name="attn", bufs=2)
                    attnps_pool = tc.tile_pool(name="attnps", bufs=1, space="PSUM")
                    _apools["atp"] = attn_pool.__enter__()
                    _apools["atps"] = attnps_pool.__enter__()
                emit_step(t)
                flush_evicts()
                sb_prev = t // TB - 1
                ph = t % TB
                if sb_prev >= 0 and ph < 6:
                    emit_attention_chunk(sb_prev, ph)
                elif t >= TB + 6:
                    emit_qgroup()
                    emit_qgroup()
                    if t % 2 == 0:
                        emit_qgroup()
            # last super-block's attention; keep qgroups flowing so the PE
            # never idles past the HAM window during the transition
            for stp in range(6):
                emit_attention_chunk(NSB - 1, stp)
                emit_qgroup()
                emit_qgroup()
                flush_evicts()
            # drain: release ALL inner PSUM pools, then run 4-chunk groups
            # (16 matmuls per 8KB PSUM tile, DVE+ACT eviction, 1MB DMAs)
            attnps_pool.__exit__(None, None, None)
            attn_pool.__exit__(None, None, None)
            flush_evicts()
            outqps_pool.__exit__(None, None, None)
            lstmps_pool.__exit__(None, None, None)
            tailps_pool = tc.tile_pool(name="tailps", bufs=2, space="PSUM")
            tps = tailps_pool.__enter__()

            def emit_tail_group(n):
                mi, nq = qready[0]
                for x in range(n):
                    qready.pop(0)
                ps = tps.tile([128, 4, 512], f32, tag="tq")
                for idx in range(n):
                    for k in range(4):
                        nc.tensor.matmul(
                            ps[:, idx, :NC_N],
                            zt[k][:, mi * 128 : (mi + 1) * 128],
                            wpre[:, k, (nq + idx) * NC_N : (nq + idx + 1) * NC_N],
                            start=(k == 0),
                            stop=(k == 3),
                        )
                cnt = qcount[0]
                qcount[0] += 1
                nsplit = (n + 1) // 2

                def evict():
                    o_sb = oqp.tile([128, 4, NC_N], bf16, tag="osb4")
                    nc.vector.tensor_copy(
                        out=o_sb[:, 0:nsplit, :], in_=ps[:, 0:nsplit, :NC_N]
                    )
                    if n > nsplit:
                        nc.scalar.copy(
                            out=o_sb[:, nsplit:n, :], in_=ps[:, nsplit:n, :NC_N]
                        )
                    dmaq[cnt % 3].dma_start(
                        out=out_d[
                            mi * 128 : (mi + 1) * 128,
                            nq * NC_N : (nq + n) * NC_N,
                        ],
                        in_=o_sb[:, :n, :].rearrange("p g n -> p (g n)"),
                    )

                pending_ev.append(evict)

            while qready:
                mi0, nq0 = qready[0]
                n = 1
                while (
                    n < 4
                    and n < len(qready)
                    and qready[n] == (mi0, nq0 + n)
                    and (nq0 + n) % 4 != 0
                ):
                    n += 1
                emit_tail_group(n)
                if len(pending_ev) > 1:
                    pending_ev.pop(0)()
            flush_evicts()

            tailps_pool.__exit__(None, None, None)
            for pool in (
                outq_pool, lstm_pool,
            ):
                pool.__exit__(None, None, None)
    nc.compile()
    return nc


def _prep_inputs(inputs):
    bf = ml_dtypes.bfloat16
    target = np.asarray(inputs["target_tensor"])
    enc = np.asarray(inputs["encoder_outputs"], dtype=np.float32)
    lens = np.asarray(inputs["encoder_seq_lens"])
    h0 = np.asarray(inputs["h0"], dtype=np.float32)
    c0 = np.asarray(inputs["c0"], dtype=np.float32)
    emb = np.ascontiguousarray(np.asarray(inputs["emb"], dtype=np.float32))
    W_ih = np.asarray(inputs["W_ih"], dtype=np.float32)
    W_hh = np.asarray(inputs["W_hh"], dtype=np.float32)
    bias = (
        np.asarray(inputs["b_ih"], dtype=np.float32)
        + np.asarray(inputs["b_hh"], dtype=np.float32)
    )
    # gate order (i, f, g, o) -> (o, i, f, g); o/i/f rows x0.5 (tanh trick);
    # all W_hh entries x0.5 again because the device streams Z = 2h
    perm = np.concatenate(
        [np.arange(3 * H, 4 * H), np.arange(0, 2 * H), np.arange(2 * H, 3 * H)]
    )
    rs = np.concatenate([np.full(3 * H, 0.5, np.float32), np.ones(H, np.float32)])
    W_ih = W_ih[perm] * rs[:, None]
    W_hh = W_hh[perm] * rs[:, None] * 0.5
    bias = bias[perm] * rs
    W_lin = np.asarray(inputs["W_lin"], dtype=np.float32)
    b_lin = np.asarray(inputs["b_lin"], dtype=np.float32)

    wihT = np.ascontiguousarray(W_ih.T.astype(bf))                # (E, 4H)
    whhT = np.ascontiguousarray(
        W_hh.T.reshape(2, 128, G4H).astype(bf)
    )                                                             # (2,128,4H)
    biasT = np.ascontiguousarray(bias.reshape(NCH, 128).T)        # (128, NCH)
    # h-columns of W_lin x0.5 (Z = 2h); ctx columns unscaled
    wlinT_full = W_lin.T.copy()
    wlinT_full[:H] *= 0.5
    wlinT_full = wlinT_full.astype(bf)                            # (512, V)

    in_maps = []
    for i in range(NCORES):
        bg = i % NBG
        vh = i // NBG
        sl = slice(bg * BL, (bg + 1) * BL)
        vsl = slice(vh * VL, (vh + 1) * VL)
        # t-major flat index = t*BL + b, laid out [128, NMT]
        tok = np.ascontiguousarray(
            target[sl].T.reshape(BT).reshape(NMT, 128).T.astype(np.int32)
        )
        enc_i = enc[sl]                                           # (BL, S, H)
        enc_sbh = np.ascontiguousarray(
            enc_i.transpose(1, 0, 2).astype(bf)
        )                                                         # (S, BL, H)
        encT = np.ascontiguousarray(
            enc_i.transpose(2, 0, 1).reshape(2, 128, BL, S).astype(bf)
        )                                                         # (2,128,BL,S)
        # device state carries 2*h0 / 2*c0
        h0T = np.ascontiguousarray(
            (2.0 * h0[sl]).T.reshape(2, 128, BL).transpose(1, 0, 2)
        )
        c0T = np.ascontiguousarray(
            (2.0 * c0[sl]).T.reshape(2, 128, BL).transpose(1, 0, 2)
        )
        # additive mask (0 valid / -30000 masked), broadcast over TB
        m01 = (np.arange(S)[:, None] < lens[sl][None, :]).astype(np.float32)
        mask24 = np.ascontiguousarray(
            np.broadcast_to(
                ((m01 - 1.0) * 30000.0)[:, :, None], (S, BL, TB)
            ).astype(np.float32)
        )
        wlinT = np.ascontiguousarray(
            wlinT_full[:, vsl].reshape(4, 128, VL)
        )                                                         # (4,128,VL)
        in_maps.append(
            {
                "tok": tok,
                "emb": emb,
                "enc": enc_sbh,
                "encT": encT,
                "h0T": h0T,
                "c0T": c0T,
                "mask24": mask24,
                "biasT": biasT,
                "wihT": wihT,
                "whhT": whhT,
                "wlinT": wlinT,
            }
        )
    return in_maps, b_lin


LAST_RESULTS = None


def _install_ntff_shim():
    """Provide antenv.axon_hooks if the image's antenv lacks it, so
    trace=True/BASS_TRACE=1 can capture NTFF profiles under axon."""
    import sys
    import types

    try:
        from antenv.axon_hooks import get_axon_ntff_profile_hook  # noqa: F401

        return
    except ImportError:
        pass
    try:
        from trn_agent_boot.trn_boot import _ntff_profile_via_ctypes

        hook = _ntff_profile_via_ctypes("/opt/axon/libaxon_pjrt.so")
        m = types.ModuleType("antenv.axon_hooks")
        m.get_axon_ntff_profile_hook = lambda: hook
        m.set_axon_ntff_profile_hook = lambda h: None
        sys.modules["antenv.axon_hooks"] = m
    except Exception:
        pass


def kernel(**inputs):
    global LAST_RESULTS
    _install_ntff_shim()
    if "nc" not in _CACHE:
        _CACHE["nc"] = _build()
    nc = _CACHE["nc"]
    in_maps, b_lin = _prep_inputs(inputs)
    res = run_bass_kernel_spmd(nc, in_maps, core_ids=list(range(NCORES)))
    LAST_RESULTS = res
    out = np.empty((B, T, V), dtype=np.float32)
    for i in range(NCORES):
        bg = i % NBG
        vh = i // NBG
        vsl = slice(vh * VL, (vh + 1) * VL)
        # logits rows are (superblock, batch, t_in); reorder to (b, t)
        lg = (
            res.results[i]["logits"]
            .astype(np.float32)
            .reshape(NSB, BL, TB, VL)
            .transpose(1, 0, 2, 3)
            .reshape(BL, T, VL)
        )
        out[bg * BL : (bg + 1) * BL, :, vsl] = lg + b_lin[None, None, vsl]
    return out


# revision 50
# speedup vs baseline: 1.3774x; 1.0370x over previous
"""Trainium2 Bass kernel for DecoderAttnRNN (LSTM + attention decoder).

Sharding: hybrid over 8 cores = 4 batch-groups x 2 vocab-halves.
Each core handles 16 batches and 16000 vocab columns.

v4 design — fused pipeline:
  phase 0: embedding gather (deep-pipelined indirect DMA), transpose,
           x@W_ih.T+bias precompute split so steps 0-7 unblock early
  fused loop over 72 LSTM steps in 3 super-blocks of 24 steps:
    - LSTM recurrence using ONLY tanh (sigmoid folded via
      sig(x) = (tanh(x/2)+1)/2 with all x0.5/x2 rescales folded into the
      host-side weights), so attention's exp shares one ACT table set
    - after each super-block: attention for its 24 timesteps
    - logits quarter-groups (4 matmuls -> 500 bf16 cols -> DMA) for
      completed super-blocks are interleaved into the tensor-engine idle
      gaps of later LSTM steps; this also keeps the PE HAM-warm
  tail: remaining logits quarter-groups back-to-back
Output rows are in (superblock, batch, t_in) order; host reorders, upcasts
bf16 -> f32 and adds b_lin.

LSTM cell with stored state C = 2c, Z = 2h, gate order (o, i, f, g):
  t8 = tanh([psum + xw])        (o,i,f rows pre-scaled x0.5 on host)
  AB = (t8[i,f] + 1) * [t8[g] | C]   -> [A | B] = [2*sig_i*tanh_g | 4*sig_f*c]
  C' = 0.5*B + A                (= 2*c_new)
  th = tanh(0.5*C')             (= tanh(c_new))
  Z  = (t8[o] + 1) * th         (= 2*h_new; W_hh, scores-scale, W_lin
                                   h-columns absorb the factor 2)
"""

import numpy as np
import ml_dtypes

import concourse.bass as bass
import concourse.mybir as mybir
import concourse.tile as tile
from concourse import bacc
from concourse.bass_utils import run_bass_kernel_spmd
from concourse.masks import make_identity

B, T, S, E, H, V = 64, 72, 72, 128, 256, 32000
NCORES = 8
NBG = 4                   # batch groups
NVH = 2                   # vocab halves
BL = B // NBG             # 16 batches per core
BT = BL * T               # 1152
VL = V // NVH             # 16000 vocab cols per core
G4H = 4 * H               # 1024
NCH = G4H // 128          # 8 gate chunks of 128
NC_N = 500                # logits n-chunk (one PSUM bank)
NQG_N = VL // NC_N        # 32 n-chunks per m-tile
NMT = BT // 128           # 9 m-tiles
NSB = 3                   # super-blocks of the time axis
TB = T // NSB             # 24 steps per super-block
SBR = BL * TB             # 384 logits rows per super-block (= 3 m-tiles)
MPS = SBR // 128          # m-tiles per super-block

f32 = mybir.dt.float32
bf16 = mybir.dt.bfloat16
i32 = mybir.dt.int32

_CACHE = {}


def _build():
    nc = bacc.Bacc(None, target_bir_lowering=False)

    tok_d = nc.declare_dram_parameter("tok", [128, NMT], i32, isOutput=False)
    emb_d = nc.declare_dram_parameter("emb", [V, E], f32, isOutput=False)
    enc_d = nc.declare_dram_parameter("enc", [S, BL, H], bf16, isOutput=False)
    encT_d = nc.declare_dram_parameter("encT", [2, 128, BL, S], bf16, isOutput=False)
    h0T_d = nc.declare_dram_parameter("h0T", [128, 2, BL], f32, isOutput=False)
    c0T_d = nc.declare_dram_parameter("c0T", [128, 2, BL], f32, isOutput=False)
    mask_d = nc.declare_dram_parameter("mask24", [S, BL, TB], f32, isOutput=False)
    biasT_d = nc.declare_dram_parameter("biasT", [128, NCH], f32, isOutput=False)
    wihT_d = nc.declare_dram_parameter("wihT", [E, G4H], bf16, isOutput=False)
    whhT_d = nc.declare_dram_parameter("whhT", [2, 128, G4H], bf16, isOutput=False)
    wlinT_d = nc.declare_dram_parameter("wlinT", [4, 128, VL], bf16, isOutput=False)
    out_d = nc.declare_dram_parameter("logits", [BT, VL], bf16, isOutput=True)

    with tile.TileContext(nc) as tc:
        with tc.tile_pool(name="persist", bufs=1) as pp:
            # ---- setup DMAs: phase-0-critical loads first ----
            tok_sb = pp.tile([128, NMT], i32)
            nc.sync.dma_start(out=tok_sb[:], in_=tok_d[:])
            wih_sb = pp.tile([128, G4H], bf16)
            nc.scalar.dma_start(out=wih_sb[:], in_=wihT_d[:])
            biasT_sb = pp.tile([128, NCH], f32)
            nc.scalar.dma_start(out=biasT_sb[:], in_=biasT_d[:])
            whh_sb = pp.tile([128, 2, G4H], bf16)
            for k in range(2):
                nc.scalar.dma_start(out=whh_sb[:, k, :], in_=whhT_d[k])
            # tg8 holds the 8 tanh'd gate chunks (o,i,f,g) plus C=2c in
            # slots 8:10, so one fused op computes both cell products.
            # These tiny state loads ride the gpsimd queue so they cannot
            # queue behind the multi-MB weight streams.
            tg8 = pp.tile([128, 10, BL], f32)
            nc.gpsimd.dma_start(out=tg8[:, 8:10], in_=c0T_d[:])
            h0f = pp.tile([128, 2, BL], f32)
            nc.gpsimd.dma_start(out=h0f[:], in_=h0T_d[:])
            h_init = pp.tile([128, 2, BL], bf16)
            mask_sb = pp.tile([S, BL, TB], f32)
            nc.gpsimd.dma_start(out=mask_sb[:], in_=mask_d[:])

            ident = pp.tile([128, 128], f32)
            make_identity(nc, ident[:])

            xwT = pp.tile([128, T, NCH, BL], bf16)       # x@W_ih.T + bias
            z01 = pp.tile([128, 2, NSB, BL, TB], bf16)   # Z=2h (k-tiles 0,1)
            z23 = pp.tile([128, 2, NSB, BL, TB], bf16)   # ctx (k-tiles 2,3)
            x_allT = pp.tile([128, BT], bf16)

            # bulk loads, needed later; queued behind the critical ones
            encT_sb = pp.tile([128, 2, BL, S], bf16)
            for k in range(2):
                nc.sync.dma_start(out=encT_sb[:, k], in_=encT_d[k])
            enc_sb = pp.tile([S, BL, H], bf16)
            nc.scalar.dma_start(out=enc_sb[:], in_=enc_d[:])
            wpre = pp.tile([128, 4, VL], bf16)

            ones_col = pp.tile([S, 16], bf16)
            ones_row_f = pp.tile([1, 128], f32)
            nc.vector.memset(ones_col[:], 1.0)
            nc.vector.memset(ones_row_f[:], 1.0)

            # ---- phase 0: embedding gather + transpose (deep pipeline) ----
            with (
                tc.tile_pool(name="p0", bufs=NMT, space="SBUF") as wp,
                tc.tile_pool(name="p0ps", bufs=2, space="PSUM") as psp,
            ):
                for j in range(NMT):
                    x_t = wp.tile([128, E], f32, tag="x")
                    nc.gpsimd.indirect_dma_start(
                        out=x_t[:],
                        out_offset=None,
                        in_=emb_d[:],
                        in_offset=bass.IndirectOffsetOnAxis(
                            ap=tok_sb[:, j : j + 1], axis=0
                        ),
                    )
                    ps_t = psp.tile([128, 128], f32, tag="pst")
                    nc.tensor.transpose(out=ps_t[:], in_=x_t[:], identity=ident[:])
                    nc.vector.tensor_copy(
                        out=x_allT[:, j * 128 : (j + 1) * 128], in_=ps_t[:]
                    )

                # early xW for t<8 (x_allT cols 0:128) so the LSTM can start
                ps_xw8 = psp.tile([128, NCH, 128], f32, tag="psxw8")
                for c in range(NCH):
                    nc.tensor.matmul(
                        ps_xw8[:, c, :],
                        wih_sb[:, c * 128 : (c + 1) * 128],
                        x_allT[:, 0:128],
                        start=True,
                        stop=True,
                    )
                for c in range(NCH):
                    nc.vector.tensor_scalar(
                        out=xwT[:, 0:8, c, :],
                        in0=ps_xw8[:, c, :].rearrange("p (t b) -> p t b", b=BL),
                        scalar1=biasT_sb[:, c : c + 1],
                        scalar2=None,
                        op0=mybir.AluOpType.add,
                    )

            # h_init conversion deferred to here so the copy never blocks the
            # gather-cast pipeline at the head of the DVE queue
            nc.vector.tensor_copy(out=h_init[:], in_=h0f[:])

            # W_lin preload, gated behind the 5th gather-cast (the dummy
            # write creates the dependency) so the 16 MiB stream stays out
            # of the latency-critical early-phase-0 window; it has until
            # ~step 30 to finish
            for k in range(4):
                nc.vector.tensor_copy(
                    out=wpre[0:1, k, 0:1], in_=x_allT[0:1, 5 * 128 - 1 : 5 * 128]
                )
                eng = nc.sync if k % 2 == 0 else nc.scalar
                eng.dma_start(out=wpre[:, k, :], in_=wlinT_d[k])

            def emit_xw_rest(c):
                # two 512-wide passes to keep PSUM small (32 t per pass)
                for half in range(2):
                    t0 = 8 + 32 * half
                    ps_xw = xwp.tile([128, 512], f32, tag="psxw")
                    nc.tensor.matmul(
                        ps_xw[:],
                        wih_sb[:, c * 128 : (c + 1) * 128],
                        x_allT[:, t0 * BL : (t0 + 32) * BL],
                        start=True,
                        stop=True,
                    )
                    nc.vector.tensor_scalar(
                        out=xwT[:, t0 : t0 + 32, c, :],
                        in0=ps_xw[:].rearrange("p (t b) -> p t b", b=BL),
                        scalar1=biasT_sb[:, c : c + 1],
                        scalar2=None,
                        op0=mybir.AluOpType.add,
                    )

            # ---- fused loop: LSTM steps + per-superblock attention +
            #      interleaved logits quarter-groups ----
            zt = [
                z01[:, 0].rearrange("p s b t -> p (s b t)"),
                z01[:, 1].rearrange("p s b t -> p (s b t)"),
                z23[:, 0].rearrange("p s b t -> p (s b t)"),
                z23[:, 1].rearrange("p s b t -> p (s b t)"),
            ]
            qready = []          # (mi, ng) logits quarter-groups ready to run
            qcount = [0]
            _attn_state = {}
            _apools = {}

            # SBUF pools first, PSUM pools inside, so every PSUM pool can be
            # released before the drain (which then gets all 8 banks)
            lstm_pool = tc.tile_pool(name="lstm", bufs=3)
            lp = lstm_pool.__enter__()
            outq_pool = tc.tile_pool(name="outq", bufs=3)
            oqp = outq_pool.__enter__()
            lstmps_pool = tc.tile_pool(name="lstmps", bufs=2, space="PSUM")
            lps = lstmps_pool.__enter__()
            outqps_pool = tc.tile_pool(name="outqps", bufs=2, space="PSUM")
            oqps = outqps_pool.__enter__()
            # innermost: remainder-xW PSUM, released once the attention pools
            # are needed (pools close in stack order)
            xw_pool = tc.tile_pool(name="pxw", bufs=2, space="PSUM")
            xwp = xw_pool.__enter__()
            dmaq = [nc.sync, nc.scalar, nc.gpsimd]

            pending_ev = []

            def emit_qgroup():
                # matmuls now; the eviction+DMA is deferred so the scheduler
                # gives the next LSTM step's chain ops priority over it
                if not qready:
                    return
                mi, nq = qready.pop(0)
                n0 = nq * NC_N
                ps_o = oqps.tile([128, 512], f32, tag="po")
                for k in range(4):
                    nc.tensor.matmul(
                        ps_o[:, :NC_N],
                        zt[k][:, mi * 128 : (mi + 1) * 128],
                        wpre[:, k, n0 : n0 + NC_N],
                        start=(k == 0),
                        stop=(k == 3),
                    )
                cnt = qcount[0]
                qcount[0] += 1

                def evict():
                    o_sb = oqp.tile([128, NC_N], bf16, tag="osb")
                    eng = nc.vector.tensor_copy if cnt % 2 == 0 else nc.scalar.copy
                    eng(out=o_sb[:], in_=ps_o[:, :NC_N])
                    dmaq[cnt % 3].dma_start(
                        out=out_d[mi * 128 : (mi + 1) * 128, n0 : n0 + NC_N],
                        in_=o_sb[:],
                    )

                pending_ev.append(evict)

            def flush_evicts():
                while pending_ev:
                    pending_ev.pop(0)()

            def emit_step(t):
                with tc.high_priority(offset=600):
                    _emit_step_body(t)

            def _emit_step_body(t):
                sb, ti = divmod(t, TB)
                ps_g = lps.tile([128, NCH, BL], f32, tag="psg")
                for c in range(NCH):
                    for k in range(2):
                        rhs = (
                            h_init[:, k, :] if t == 0
                            else z01[:, k, (t - 1) // TB, :, (t - 1) % TB]
                        )
                        nc.tensor.matmul(
                            ps_g[:, c, :],
                            whh_sb[:, k, c * 128 : (c + 1) * 128],
                            rhs,
                            start=(k == 0),
                            stop=(k == 1),
                        )
                gsum = lp.tile([128, NCH, BL], f32, tag="gsum")
                nc.vector.tensor_tensor(
                    out=gsum[:], in0=ps_g[:], in1=xwT[:, t],
                    op=mybir.AluOpType.add,
                )
                # single tanh over all gates (o,i,f pre-scaled x0.5 on host)
                nc.scalar.activation(
                    out=tg8[:, 0:8], in_=gsum[:],
                    func=mybir.ActivationFunctionType.Tanh,
                )
                # AB = (t8[i,f]+1) * [t8[g] | C]
                ab = lp.tile([128, 4, BL], f32, tag="ab")
                nc.vector.scalar_tensor_tensor(
                    out=ab[:], in0=tg8[:, 2:6], scalar=1.0, in1=tg8[:, 6:10],
                    op0=mybir.AluOpType.add, op1=mybir.AluOpType.mult,
                )
                # C' = 0.5*B + A
                nc.vector.scalar_tensor_tensor(
                    out=tg8[:, 8:10], in0=ab[:, 2:4], scalar=0.5, in1=ab[:, 0:2],
                    op0=mybir.AluOpType.mult, op1=mybir.AluOpType.add,
                )
                th = lp.tile([128, 2, BL], f32, tag="th")
                nc.scalar.activation(
                    out=th[:], in_=tg8[:, 8:10],
                    func=mybir.ActivationFunctionType.Tanh, scale=0.5,
                )
                # Z = (t8[o]+1) * th  (bf16, = 2*h)
                nc.vector.scalar_tensor_tensor(
                    out=z01[:, :, sb, :, ti], in0=tg8[:, 0:2], scalar=1.0,
                    in1=th[:], op0=mybir.AluOpType.add, op1=mybir.AluOpType.mult,
                )

            def emit_attention_chunk(sb, step):
                # attention for super-block sb, split into 6 chunks emitted
                # across consecutive later steps to bound PE-queue delay
                atp = _apools["atp"]
                atps = _apools["atps"]
                if step == 0:
                    st = atps.tile([S, BL, 32], f32, tag="ps_s")
                    exb = atp.tile([S, BL, TB], bf16, tag="exb")
                    rcp = atp.tile([1, BL, TB], f32, tag="rcp")
                    att = atp.tile([S, BL, TB], bf16, tag="att")
                    den = atps.tile([16, 4, 128], f32, tag="den")
                    bc = atps.tile([128, 512], f32, tag="bc")
                    ctx = atps.tile([128, 2, 256], f32, tag="ctx")
                    _attn_state[sb] = (st, exb, rcp, att, den, bc, ctx)
                st, exb, rcp, att, den, bc, ctx = _attn_state[sb]
                if step in (0, 1):
                    for b in range(8 * step, 8 * (step + 1)):
                        for k in range(2):
                            nc.tensor.matmul(
                                st[:, b, :TB],
                                encT_sb[:, k, b, :],
                                z01[:, k, sb, b, :],
                                start=(k == 0),
                                stop=(k == 1),
                            )
                elif step == 2:
                    # masked exp over all (b,t) of the block; Z=2h so the
                    # score scale halves
                    nc.vector.tensor_tensor(
                        out=exb[:], in0=st[:, :, :TB], in1=mask_sb[:],
                        op=mybir.AluOpType.add,
                    )
                    nc.scalar.activation(
                        out=exb[:], in_=exb[:],
                        func=mybir.ActivationFunctionType.Exp,
                        scale=float(0.5 / np.sqrt(H)),
                    )
                elif step == 3:
                    for g in range(4):
                        nc.tensor.matmul(
                            den[:, g, : 4 * TB],
                            ones_col[:],
                            exb[:, 4 * g : 4 * (g + 1), :].rearrange(
                                "p b t -> p (b t)"
                            ),
                            start=True,
                            stop=True,
                        )
                        nc.vector.reciprocal(
                            out=rcp[:, 4 * g : 4 * (g + 1), :].rearrange(
                                "p b t -> p (b t)"
                            ),
                            in_=den[0:1, g, : 4 * TB],
                        )
                    nc.tensor.matmul(
                        bc[:, : BL * TB], ones_row_f[:],
                        rcp[:].rearrange("p b t -> p (b t)"),
                        start=True, stop=True,
                    )
                    nc.vector.tensor_tensor(
                        out=att[:].rearrange("p b t -> p (b t)"),
                        in0=exb[:].rearrange("p b t -> p (b t)"),
                        in1=bc[0:S, : BL * TB],
                        op=mybir.AluOpType.mult,
                    )
                elif step in (4, 5):
                    # half the batches per chunk; evict frees the PSUM tile
                    # for the second half (attnps has bufs=1)
                    b0 = 8 * (step - 4)
                    for b in range(b0, b0 + 8):
                        for j in range(2):
                            nc.tensor.matmul(
                                ctx[:, j, (b - b0) * TB : (b - b0 + 1) * TB],
                                enc_sb[:, b, j * 128 : (j + 1) * 128],
                                att[:, b, :],
                                start=True,
                                stop=True,
                            )
                    nc.vector.tensor_copy(
                        out=z23[:, :, sb, b0 : b0 + 8, :].rearrange(
                            "p k b t -> p k (b t)"
                        ),
                        in_=ctx[:, :, : 8 * TB],
                    )
                    if step == 5:
                        for mi in range(sb * MPS, (sb + 1) * MPS):
                            for nq in range(NQG_N):
                                qready.append((mi, nq))

            # ---- emit the fused schedule ----
            for t in range(T):
                if 1 <= t <= NCH:
                    emit_xw_rest(t - 1)
                if t == NCH + 1:
                    # xW PSUM freed; attention pools take its place
                    xw_pool.__exit__(None, None, None)
                    attn_pool = tc.tile_pool(name="attn", bufs=2)
                    attnps_pool = tc.tile_pool(name="attnps", bufs=1, space="PSUM")
                    _apools["atp"] = attn_pool.__enter__()
                    _apools["atps"] = attnps_pool.__enter__()
                emit_step(t)
                flush_evicts()
                sb_prev = t // TB - 1
                ph = t % TB
                if sb_prev >= 0 and ph < 6:
                    emit_attention_chunk(sb_prev, ph)
                elif t >= TB + 6:
                    emit_qgroup()
                    emit_qgroup()
                    if t % 2 == 0:
                        emit_qgroup()
            # last super-block's attention; keep qgroups flowing so the PE
            # never idles past the HAM window during the transition
            for stp in range(6):
                emit_attention_chunk(NSB - 1, stp)
                emit_qgroup()
                emit_qgroup()
                flush_evicts()
            # drain: release ALL inner PSUM pools, then run 4-chunk groups
            # (16 matmuls per 8KB PSUM tile, DVE+ACT eviction, 1MB DMAs)
            attnps_pool.__exit__(None, None, None)
            attn_pool.__exit__(None, None, None)
            flush_evicts()
            outqps_pool.__exit__(None, None, None)
            lstmps_pool.__exit__(None, None, None)
            tailps_pool = tc.tile_pool(name="tailps", bufs=2, space="PSUM")
            tps = tailps_pool.__enter__()

            def emit_tail_group(n):
                mi, nq = qready[0]
                for x in range(n):
                    qready.pop(0)
                ps = tps.tile([128, 4, 512], f32, tag="tq")
                for idx in range(n):
                    for k in range(4):
                        nc.tensor.matmul(
                            ps[:, idx, :NC_N],
                            zt[k][:, mi * 128 : (mi + 1) * 128],
                            wpre[:, k, (nq + idx) * NC_N : (nq + idx + 1) * NC_N],
                            start=(k == 0),
                            stop=(k == 3),
                        )
                cnt = qcount[0]
                qcount[0] += 1
                nsplit = (n + 1) // 2

                def evict():
                    o_sb = oqp.tile([128, 4, NC_N], bf16, tag="osb4")
                    nc.vector.tensor_copy(
                        out=o_sb[:, 0:nsplit, :], in_=ps[:, 0:nsplit, :NC_N]
                    )
                    if n > nsplit:
                        nc.scalar.copy(
                            out=o_sb[:, nsplit:n, :], in_=ps[:, nsplit:n, :NC_N]
                        )
                    dmaq[cnt % 3].dma_start(
                        out=out_d[
                            mi * 128 : (mi + 1) * 128,
                            nq * NC_N : (nq + n) * NC_N,
                        ],
                        in_=o_sb[:, :n, :].rearrange("p g n -> p (g n)"),
                    )

                pending_ev.append(evict)

            while qready:
                mi0, nq0 = qready[0]
                n = 1
                while (
                    n < 4
                    and n < len(qready)
                    and qready[n] == (mi0, nq0 + n)
                    and (nq0 + n) % 4 != 0
                ):
                    n += 1
                emit_tail_group(n)
                if len(pending_ev) > 1:
                    pending_ev.pop(0)()
            flush_evicts()

            tailps_pool.__exit__(None, None, None)
            for pool in (
                outq_pool, lstm_pool,
            ):
                pool.__exit__(None, None, None)
    nc.compile()
    return nc


def _prep_inputs(inputs):
    bf = ml_dtypes.bfloat16
    target = np.asarray(inputs["target_tensor"])
    enc = np.asarray(inputs["encoder_outputs"], dtype=np.float32)
    lens = np.asarray(inputs["encoder_seq_lens"])
    h0 = np.asarray(inputs["h0"], dtype=np.float32)
    c0 = np.asarray(inputs["c0"], dtype=np.float32)
    emb = np.ascontiguousarray(np.asarray(inputs["emb"], dtype=np.float32))
    W_ih = np.asarray(inputs["W_ih"], dtype=np.float32)
    W_hh = np.asarray(inputs["W_hh"], dtype=np.float32)
    bias = (
        np.asarray(inputs["b_ih"], dtype=np.float32)
        + np.asarray(inputs["b_hh"], dtype=np.float32)
    )
    # gate order (i, f, g, o) -> (o, i, f, g); o/i/f rows x0.5 (tanh trick);
    # all W_hh entries x0.5 again because the device streams Z = 2h
    perm = np.concatenate(
        [np.arange(3 * H, 4 * H), np.arange(0, 2 * H), np.arange(2 * H, 3 * H)]
    )
    rs = np.concatenate([np.full(3 * H, 0.5, np.float32), np.ones(H, np.float32)])
    W_ih = W_ih[perm] * rs[:, None]
    W_hh = W_hh[perm] * rs[:, None] * 0.5
    bias = bias[perm] * rs
    W_lin = np.asarray(inputs["W_lin"], dtype=np.float32)
    b_lin = np.asarray(inputs["b_lin"], dtype=np.float32)

    wihT = np.ascontiguousarray(W_ih.T.astype(bf))                # (E, 4H)
    whhT = np.ascontiguousarray(
        W_hh.T.reshape(2, 128, G4H).astype(bf)
    )                                                             # (2,128,4H)
    biasT = np.ascontiguousarray(bias.reshape(NCH, 128).T)        # (128, NCH)
    # h-columns of W_lin x0.5 (Z = 2h); ctx columns unscaled
    wlinT_full = W_lin.T.copy()
    wlinT_full[:H] *= 0.5
    wlinT_full = wlinT_full.astype(bf)                            # (512, V)

    in_maps = []
    for i in range(NCORES):
        bg = i % NBG
        vh = i // NBG
        sl = slice(bg * BL, (bg + 1) * BL)
        vsl = slice(vh * VL, (vh + 1) * VL)
        # t-major flat index = t*BL + b, laid out [128, NMT]
        tok = np.ascontiguousarray(
            target[sl].T.reshape(BT).reshape(NMT, 128).T.astype(np.int32)
        )
        enc_i = enc[sl]                                           # (BL, S, H)
        enc_sbh = np.ascontiguousarray(
            enc_i.transpose(1, 0, 2).astype(bf)
        )                                                         # (S, BL, H)
        encT = np.ascontiguousarray(
            enc_i.transpose(2, 0, 1).reshape(2, 128, BL, S).astype(bf)
        )                                                         # (2,128,BL,S)
        # device state carries 2*h0 / 2*c0
        h0T = np.ascontiguousarray(
            (2.0 * h0[sl]).T.reshape(2, 128, BL).transpose(1, 0, 2)
        )
        c0T = np.ascontiguousarray(
            (2.0 * c0[sl]).T.reshape(2, 128, BL).transpose(1, 0, 2)
        )
        # additive mask (0 valid / -30000 masked), broadcast over TB
        m01 = (np.arange(S)[:, None] < lens[sl][None, :]).astype(np.float32)
        mask24 = np.ascontiguousarray(
            np.broadcast_to(
                ((m01 - 1.0) * 30000.0)[:, :, None], (S, BL, TB)
            ).astype(np.float32)
        )
        wlinT = np.ascontiguousarray(
            wlinT_full[:, vsl].reshape(4, 128, VL)
        )                                                         # (4,128,VL)
        in_maps.append(
            {
                "tok": tok,
                "emb": emb,
                "enc": enc_sbh,
                "encT": encT,
                "h0T": h0T,
                "c0T": c0T,
                "mask24": mask24,
                "biasT": biasT,
                "wihT": wihT,
                "whhT": whhT,
                "wlinT": wlinT,
            }
        )
    return in_maps, b_lin


LAST_RESULTS = None


def _install_ntff_shim():
    """Provide antenv.axon_hooks if the image's antenv lacks it, so
    trace=True/BASS_TRACE=1 can capture NTFF profiles under axon."""
    import sys
    import types

    try:
        from antenv.axon_hooks import get_axon_ntff_profile_hook  # noqa: F401

        return
    except ImportError:
        pass
    try:
        from trn_agent_boot.trn_boot import _ntff_profile_via_ctypes

        hook = _ntff_profile_via_ctypes("/opt/axon/libaxon_pjrt.so")
        m = types.ModuleType("antenv.axon_hooks")
        m.get_axon_ntff_profile_hook = lambda: hook
        m.set_axon_ntff_profile_hook = lambda h: None
        sys.modules["antenv.axon_hooks"] = m
    except Exception:
        pass


def kernel(**inputs):
    global LAST_RESULTS
    _install_ntff_shim()
    if "nc" not in _CACHE:
        _CACHE["nc"] = _build()
    nc = _CACHE["nc"]
    in_maps, b_lin = _prep_inputs(inputs)
    res = run_bass_kernel_spmd(nc, in_maps, core_ids=list(range(NCORES)))
    LAST_RESULTS = res
    out = np.empty((B, T, V), dtype=np.float32)
    for i in range(NCORES):
        bg = i % NBG
        vh = i // NBG
        vsl = slice(vh * VL, (vh + 1) * VL)
        # logits rows are (superblock, batch, t_in); reorder to (b, t)
        lg = (
            res.results[i]["logits"]
            .astype(np.float32)
            .reshape(NSB, BL, TB, VL)
            .transpose(1, 0, 2, 3)
            .reshape(BL, T, VL)
        )
        out[bg * BL : (bg + 1) * BL, :, vsl] = lg + b_lin[None, None, vsl]
    return out


# revision 52
# speedup vs baseline: 1.3875x; 1.0073x over previous
"""Trainium2 Bass kernel for DecoderAttnRNN (LSTM + attention decoder).

Sharding: hybrid over 8 cores = 4 batch-groups x 2 vocab-halves.
Each core handles 16 batches and 16000 vocab columns.

v4 design — fused pipeline:
  phase 0: embedding gather (deep-pipelined indirect DMA), transpose,
           x@W_ih.T+bias precompute split so steps 0-7 unblock early
  fused loop over 72 LSTM steps in 3 super-blocks of 24 steps:
    - LSTM recurrence using ONLY tanh (sigmoid folded via
      sig(x) = (tanh(x/2)+1)/2 with all x0.5/x2 rescales folded into the
      host-side weights), so attention's exp shares one ACT table set
    - after each super-block: attention for its 24 timesteps
    - logits quarter-groups (4 matmuls -> 500 bf16 cols -> DMA) for
      completed super-blocks are interleaved into the tensor-engine idle
      gaps of later LSTM steps; this also keeps the PE HAM-warm
  tail: remaining logits quarter-groups back-to-back
Output rows are in (superblock, batch, t_in) order; host reorders, upcasts
bf16 -> f32 and adds b_lin.

LSTM cell with stored state C = 2c, Z = 2h, gate order (o, i, f, g):
  t8 = tanh([psum + xw])        (o,i,f rows pre-scaled x0.5 on host)
  AB = (t8[i,f] + 1) * [t8[g] | C]   -> [A | B] = [2*sig_i*tanh_g | 4*sig_f*c]
  C' = 0.5*B + A                (= 2*c_new)
  th = tanh(0.5*C')             (= tanh(c_new))
  Z  = (t8[o] + 1) * th         (= 2*h_new; W_hh, scores-scale, W_lin
                                   h-columns absorb the factor 2)
"""

import numpy as np
import ml_dtypes

import concourse.bass as bass
import concourse.mybir as mybir
import concourse.tile as tile
from concourse import bacc
from concourse.bass_utils import run_bass_kernel_spmd
from concourse.masks import make_identity

B, T, S, E, H, V = 64, 72, 72, 128, 256, 32000
NCORES = 8
NBG = 4                   # batch groups
NVH = 2                   # vocab halves
BL = B // NBG             # 16 batches per core
BT = BL * T               # 1152
VL = V // NVH             # 16000 vocab cols per core
G4H = 4 * H               # 1024
NCH = G4H // 128          # 8 gate chunks of 128
NC_N = 500                # logits n-chunk (one PSUM bank)
NQG_N = VL // NC_N        # 32 n-chunks per m-tile
NMT = BT // 128           # 9 m-tiles
NSB = 3                   # super-blocks of the time axis
TB = T // NSB             # 24 steps per super-block
SBR = BL * TB             # 384 logits rows per super-block (= 3 m-tiles)
MPS = SBR // 128          # m-tiles per super-block

f32 = mybir.dt.float32
bf16 = mybir.dt.bfloat16
i32 = mybir.dt.int32

_CACHE = {}


def _build():
    nc = bacc.Bacc(None, target_bir_lowering=False)

    tok_d = nc.declare_dram_parameter("tok", [128, NMT], i32, isOutput=False)
    emb_d = nc.declare_dram_parameter("emb", [V, E], f32, isOutput=False)
    enc_d = nc.declare_dram_parameter("enc", [S, BL, H], bf16, isOutput=False)
    encT_d = nc.declare_dram_parameter("encT", [2, 128, BL, S], bf16, isOutput=False)
    h0T_d = nc.declare_dram_parameter("h0T", [128, 2, BL], f32, isOutput=False)
    c0T_d = nc.declare_dram_parameter("c0T", [128, 2, BL], f32, isOutput=False)
    mask_d = nc.declare_dram_parameter("mask24", [S, BL, TB], f32, isOutput=False)
    biasT_d = nc.declare_dram_parameter("biasT", [128, NCH], f32, isOutput=False)
    wihT_d = nc.declare_dram_parameter("wihT", [E, G4H], bf16, isOutput=False)
    whhT_d = nc.declare_dram_parameter("whhT", [2, 128, G4H], bf16, isOutput=False)
    wlinT_d = nc.declare_dram_parameter("wlinT", [4, 128, VL], bf16, isOutput=False)
    out_d = nc.declare_dram_parameter("logits", [BT, VL], bf16, isOutput=True)

    with tile.TileContext(nc) as tc:
        with tc.tile_pool(name="persist", bufs=1) as pp:
            # ---- setup DMAs: phase-0-critical loads first ----
            tok_sb = pp.tile([128, NMT], i32)
            nc.sync.dma_start(out=tok_sb[:], in_=tok_d[:])
            wih_sb = pp.tile([128, G4H], bf16)
            nc.scalar.dma_start(out=wih_sb[:], in_=wihT_d[:])
            biasT_sb = pp.tile([128, NCH], f32)
            nc.scalar.dma_start(out=biasT_sb[:], in_=biasT_d[:])
            whh_sb = pp.tile([128, 2, G4H], bf16)
            for k in range(2):
                nc.scalar.dma_start(out=whh_sb[:, k, :], in_=whhT_d[k])
            # tg8 holds the 8 tanh'd gate chunks (o,i,f,g) plus C=2c in
            # slots 8:10, so one fused op computes both cell products.
            # These tiny state loads ride the gpsimd queue so they cannot
            # queue behind the multi-MB weight streams.
            tg8 = pp.tile([128, 10, BL], f32)
            nc.gpsimd.dma_start(out=tg8[:, 8:10], in_=c0T_d[:])
            h0f = pp.tile([128, 2, BL], f32)
            nc.gpsimd.dma_start(out=h0f[:], in_=h0T_d[:])
            h_init = pp.tile([128, 2, BL], bf16)
            mask_sb = pp.tile([S, BL, TB], f32)
            nc.gpsimd.dma_start(out=mask_sb[:], in_=mask_d[:])

            ident = pp.tile([128, 128], f32)
            make_identity(nc, ident[:])

            xwT = pp.tile([128, T, NCH, BL], bf16)       # x@W_ih.T + bias
            z01 = pp.tile([128, 2, NSB, BL, TB], bf16)   # Z=2h (k-tiles 0,1)
            z23 = pp.tile([128, 2, NSB, BL, TB], bf16)   # ctx (k-tiles 2,3)
            x_allT = pp.tile([128, BT], bf16)

            # bulk loads, needed later; queued behind the critical ones
            encT_sb = pp.tile([128, 2, BL, S], bf16)
            for k in range(2):
                nc.sync.dma_start(out=encT_sb[:, k], in_=encT_d[k])
            enc_sb = pp.tile([S, BL, H], bf16)
            nc.scalar.dma_start(out=enc_sb[:], in_=enc_d[:])
            wpre = pp.tile([128, 4, VL], bf16)

            ones_col = pp.tile([S, 16], bf16)
            ones_row_f = pp.tile([1, 128], f32)
            nc.vector.memset(ones_col[:], 1.0)
            nc.vector.memset(ones_row_f[:], 1.0)

            # ---- phase 0: embedding gather + transpose (deep pipeline) ----
            with (
                tc.tile_pool(name="p0", bufs=NMT, space="SBUF") as wp,
                tc.tile_pool(name="p0ps", bufs=2, space="PSUM") as psp,
            ):
                for j in range(NMT):
                    x_t = wp.tile([128, E], f32, tag="x")
                    nc.gpsimd.indirect_dma_start(
                        out=x_t[:],
                        out_offset=None,
                        in_=emb_d[:],
                        in_offset=bass.IndirectOffsetOnAxis(
                            ap=tok_sb[:, j : j + 1], axis=0
                        ),
                    )
                    ps_t = psp.tile([128, 128], f32, tag="pst")
                    nc.tensor.transpose(out=ps_t[:], in_=x_t[:], identity=ident[:])
                    nc.vector.tensor_copy(
                        out=x_allT[:, j * 128 : (j + 1) * 128], in_=ps_t[:]
                    )

                # early xW for t<8 (x_allT cols 0:128) so the LSTM can start
                ps_xw8 = psp.tile([128, NCH, 128], f32, tag="psxw8")
                for c in range(NCH):
                    nc.tensor.matmul(
                        ps_xw8[:, c, :],
                        wih_sb[:, c * 128 : (c + 1) * 128],
                        x_allT[:, 0:128],
                        start=True,
                        stop=True,
                    )
                for c in range(NCH):
                    nc.vector.tensor_scalar(
                        out=xwT[:, 0:8, c, :],
                        in0=ps_xw8[:, c, :].rearrange("p (t b) -> p t b", b=BL),
                        scalar1=biasT_sb[:, c : c + 1],
                        scalar2=None,
                        op0=mybir.AluOpType.add,
                    )

            # h_init conversion deferred to here so the copy never blocks the
            # gather-cast pipeline at the head of the DVE queue
            nc.vector.tensor_copy(out=h_init[:], in_=h0f[:])

            # W_lin preload, gated behind the 5th gather-cast (the dummy
            # write creates the dependency) so the 16 MiB stream stays out
            # of the latency-critical early-phase-0 window; it has until
            # ~step 30 to finish
            for k in range(4):
                nc.vector.tensor_copy(
                    out=wpre[0:1, k, 0:1], in_=x_allT[0:1, 5 * 128 - 1 : 5 * 128]
                )
                eng = nc.sync if k % 2 == 0 else nc.scalar
                eng.dma_start(out=wpre[:, k, :], in_=wlinT_d[k])

            def emit_xw_rest(c):
                # two 512-wide passes to keep PSUM small (32 t per pass)
                for half in range(2):
                    t0 = 8 + 32 * half
                    ps_xw = xwp.tile([128, 512], f32, tag="psxw")
                    nc.tensor.matmul(
                        ps_xw[:],
                        wih_sb[:, c * 128 : (c + 1) * 128],
                        x_allT[:, t0 * BL : (t0 + 32) * BL],
                        start=True,
                        stop=True,
                    )
                    nc.vector.tensor_scalar(
                        out=xwT[:, t0 : t0 + 32, c, :],
                        in0=ps_xw[:].rearrange("p (t b) -> p t b", b=BL),
                        scalar1=biasT_sb[:, c : c + 1],
                        scalar2=None,
                        op0=mybir.AluOpType.add,
                    )

            # ---- fused loop: LSTM steps + per-superblock attention +
            #      interleaved logits quarter-groups ----
            zt = [
                z01[:, 0].rearrange("p s b t -> p (s b t)"),
                z01[:, 1].rearrange("p s b t -> p (s b t)"),
                z23[:, 0].rearrange("p s b t -> p (s b t)"),
                z23[:, 1].rearrange("p s b t -> p (s b t)"),
            ]
            qready = []          # (mi, ng) logits quarter-groups ready to run
            qcount = [0]
            _attn_state = {}
            _apools = {}

            # SBUF pools first, PSUM pools inside, so every PSUM pool can be
            # released before the drain (which then gets all 8 banks)
            lstm_pool = tc.tile_pool(name="lstm", bufs=3)
            lp = lstm_pool.__enter__()
            outq_pool = tc.tile_pool(name="outq", bufs=3)
            oqp = outq_pool.__enter__()
            lstmps_pool = tc.tile_pool(name="lstmps", bufs=2, space="PSUM")
            lps = lstmps_pool.__enter__()
            outqps_pool = tc.tile_pool(name="outqps", bufs=2, space="PSUM")
            oqps = outqps_pool.__enter__()
            # innermost: remainder-xW PSUM, released once the attention pools
            # are needed (pools close in stack order)
            xw_pool = tc.tile_pool(name="pxw", bufs=2, space="PSUM")
            xwp = xw_pool.__enter__()
            dmaq = [nc.sync, nc.scalar, nc.gpsimd]

            pending_ev = []

            def emit_qgroup():
                # matmuls now; the eviction+DMA is deferred so the scheduler
                # gives the next LSTM step's chain ops priority over it
                if not qready:
                    return
                mi, nq = qready.pop(0)
                n0 = nq * NC_N
                ps_o = oqps.tile([128, 512], f32, tag="po")
                for k in range(4):
                    nc.tensor.matmul(
                        ps_o[:, :NC_N],
                        zt[k][:, mi * 128 : (mi + 1) * 128],
                        wpre[:, k, n0 : n0 + NC_N],
                        start=(k == 0),
                        stop=(k == 3),
                    )
                cnt = qcount[0]
                qcount[0] += 1

                def evict():
                    o_sb = oqp.tile([128, NC_N], bf16, tag="osb")
                    eng = nc.vector.tensor_copy if cnt % 2 == 0 else nc.scalar.copy
                    eng(out=o_sb[:], in_=ps_o[:, :NC_N])
                    dmaq[cnt % 3].dma_start(
                        out=out_d[mi * 128 : (mi + 1) * 128, n0 : n0 + NC_N],
                        in_=o_sb[:],
                    )

                pending_ev.append(evict)

            def flush_evicts():
                while pending_ev:
                    pending_ev.pop(0)()

            def emit_step(t):
                # offset covers the recent ~4 steps' worth of instructions:
                # enough to outrank concurrently-ready evicts/attention ops,
                # small enough that old evicts don't starve (they gate PSUM
                # reuse for the interleaved logits groups)
                with tc.high_priority(offset=150):
                    _emit_step_body(t)

            def _emit_step_body(t):
                sb, ti = divmod(t, TB)
                ps_g = lps.tile([128, NCH, BL], f32, tag="psg")
                for c in range(NCH):
                    for k in range(2):
                        rhs = (
                            h_init[:, k, :] if t == 0
                            else z01[:, k, (t - 1) // TB, :, (t - 1) % TB]
                        )
                        nc.tensor.matmul(
                            ps_g[:, c, :],
                            whh_sb[:, k, c * 128 : (c + 1) * 128],
                            rhs,
                            start=(k == 0),
                            stop=(k == 1),
                        )
                gsum = lp.tile([128, NCH, BL], f32, tag="gsum")
                nc.vector.tensor_tensor(
                    out=gsum[:], in0=ps_g[:], in1=xwT[:, t],
                    op=mybir.AluOpType.add,
                )
                # single tanh over all gates (o,i,f pre-scaled x0.5 on host)
                nc.scalar.activation(
                    out=tg8[:, 0:8], in_=gsum[:],
                    func=mybir.ActivationFunctionType.Tanh,
                )
                # AB = (t8[i,f]+1) * [t8[g] | C]
                ab = lp.tile([128, 4, BL], f32, tag="ab")
                nc.vector.scalar_tensor_tensor(
                    out=ab[:], in0=tg8[:, 2:6], scalar=1.0, in1=tg8[:, 6:10],
                    op0=mybir.AluOpType.add, op1=mybir.AluOpType.mult,
                )
                # C' = 0.5*B + A
                nc.vector.scalar_tensor_tensor(
                    out=tg8[:, 8:10], in0=ab[:, 2:4], scalar=0.5, in1=ab[:, 0:2],
                    op0=mybir.AluOpType.mult, op1=mybir.AluOpType.add,
                )
                th = lp.tile([128, 2, BL], f32, tag="th")
                nc.scalar.activation(
                    out=th[:], in_=tg8[:, 8:10],
                    func=mybir.ActivationFunctionType.Tanh, scale=0.5,
                )
                # Z = (t8[o]+1) * th  (bf16, = 2*h)
                nc.vector.scalar_tensor_tensor(
                    out=z01[:, :, sb, :, ti], in0=tg8[:, 0:2], scalar=1.0,
                    in1=th[:], op0=mybir.AluOpType.add, op1=mybir.AluOpType.mult,
                )

            def emit_attention_chunk(sb, step):
                # attention for super-block sb, split into 6 chunks emitted
                # across consecutive later steps to bound PE-queue delay
                atp = _apools["atp"]
                atps = _apools["atps"]
                if step == 0:
                    st = atps.tile([S, BL, 32], f32, tag="ps_s")
                    exb = atp.tile([S, BL, TB], bf16, tag="exb")
                    rcp = atp.tile([1, BL, TB], f32, tag="rcp")
                    att = atp.tile([S, BL, TB], bf16, tag="att")
                    den = atps.tile([16, 4, 128], f32, tag="den")
                    bc = atps.tile([128, 512], f32, tag="bc")
                    ctx = atps.tile([128, 2, 256], f32, tag="ctx")
                    _attn_state[sb] = (st, exb, rcp, att, den, bc, ctx)
                st, exb, rcp, att, den, bc, ctx = _attn_state[sb]
                if step in (0, 1):
                    for b in range(8 * step, 8 * (step + 1)):
                        for k in range(2):
                            nc.tensor.matmul(
                                st[:, b, :TB],
                                encT_sb[:, k, b, :],
                                z01[:, k, sb, b, :],
                                start=(k == 0),
                                stop=(k == 1),
                            )
                elif step == 2:
                    # masked exp over all (b,t) of the block; Z=2h so the
                    # score scale halves
                    nc.vector.tensor_tensor(
                        out=exb[:], in0=st[:, :, :TB], in1=mask_sb[:],
                        op=mybir.AluOpType.add,
                    )
                    nc.scalar.activation(
                        out=exb[:], in_=exb[:],
                        func=mybir.ActivationFunctionType.Exp,
                        scale=float(0.5 / np.sqrt(H)),
                    )
                elif step == 3:
                    for g in range(4):
                        nc.tensor.matmul(
                            den[:, g, : 4 * TB],
                            ones_col[:],
                            exb[:, 4 * g : 4 * (g + 1), :].rearrange(
                                "p b t -> p (b t)"
                            ),
                            start=True,
                            stop=True,
                        )
                        nc.vector.reciprocal(
                            out=rcp[:, 4 * g : 4 * (g + 1), :].rearrange(
                                "p b t -> p (b t)"
                            ),
                            in_=den[0:1, g, : 4 * TB],
                        )
                    nc.tensor.matmul(
                        bc[:, : BL * TB], ones_row_f[:],
                        rcp[:].rearrange("p b t -> p (b t)"),
                        start=True, stop=True,
                    )
                    nc.vector.tensor_tensor(
                        out=att[:].rearrange("p b t -> p (b t)"),
                        in0=exb[:].rearrange("p b t -> p (b t)"),
                        in1=bc[0:S, : BL * TB],
                        op=mybir.AluOpType.mult,
                    )
                elif step in (4, 5):
                    # half the batches per chunk; evict frees the PSUM tile
                    # for the second half (attnps has bufs=1)
                    b0 = 8 * (step - 4)
                    for b in range(b0, b0 + 8):
                        for j in range(2):
                            nc.tensor.matmul(
                                ctx[:, j, (b - b0) * TB : (b - b0 + 1) * TB],
                                enc_sb[:, b, j * 128 : (j + 1) * 128],
                                att[:, b, :],
                                start=True,
                                stop=True,
                            )
                    nc.vector.tensor_copy(
                        out=z23[:, :, sb, b0 : b0 + 8, :].rearrange(
                            "p k b t -> p k (b t)"
                        ),
                        in_=ctx[:, :, : 8 * TB],
                    )
                    if step == 5:
                        for mi in range(sb * MPS, (sb + 1) * MPS):
                            for nq in range(NQG_N):
                                qready.append((mi, nq))

            # ---- emit the fused schedule ----
            for t in range(T):
                if 1 <= t <= NCH:
                    emit_xw_rest(t - 1)
                if t == NCH + 1:
                    # xW PSUM freed; attention pools take its place
                    xw_pool.__exit__(None, None, None)
                    attn_pool = tc.tile_pool(name="attn", bufs=2)
                    attnps_pool = tc.tile_pool(name="attnps", bufs=1, space="PSUM")
                    _apools["atp"] = attn_pool.__enter__()
                    _apools["atps"] = attnps_pool.__enter__()
                emit_step(t)
                flush_evicts()
                sb_prev = t // TB - 1
                ph = t % TB
                if sb_prev >= 0 and ph < 6:
                    emit_attention_chunk(sb_prev, ph)
                    if t >= TB + 6:
                        # keep the PE fed through the attention chunks too
                        emit_qgroup()
                        emit_qgroup()
                elif t >= TB + 6:
                    emit_qgroup()
                    emit_qgroup()
                    if t % 2 == 0:
                        emit_qgroup()
            # last super-block's attention; keep qgroups flowing so the PE
            # never idles past the HAM window during the transition
            for stp in range(6):
                emit_attention_chunk(NSB - 1, stp)
                emit_qgroup()
                emit_qgroup()
                flush_evicts()
            # drain: release ALL inner PSUM pools, then run 4-chunk groups
            # (16 matmuls per 8KB PSUM tile, DVE+ACT eviction, 1MB DMAs)
            attnps_pool.__exit__(None, None, None)
            attn_pool.__exit__(None, None, None)
            flush_evicts()
            outqps_pool.__exit__(None, None, None)
            lstmps_pool.__exit__(None, None, None)
            tailps_pool = tc.tile_pool(name="tailps", bufs=2, space="PSUM")
            tps = tailps_pool.__enter__()

            def emit_tail_group(n):
                mi, nq = qready[0]
                for x in range(n):
                    qready.pop(0)
                ps = tps.tile([128, 4, 512], f32, tag="tq")
                for idx in range(n):
                    for k in range(4):
                        nc.tensor.matmul(
                            ps[:, idx, :NC_N],
                            zt[k][:, mi * 128 : (mi + 1) * 128],
                            wpre[:, k, (nq + idx) * NC_N : (nq + idx + 1) * NC_N],
                            start=(k == 0),
                            stop=(k == 3),
                        )
                cnt = qcount[0]
                qcount[0] += 1
                nsplit = (n + 1) // 2

                def evict():
                    o_sb = oqp.tile([128, 4, NC_N], bf16, tag="osb4")
                    nc.vector.tensor_copy(
                        out=o_sb[:, 0:nsplit, :], in_=ps[:, 0:nsplit, :NC_N]
                    )
                    if n > nsplit:
                        nc.scalar.copy(
                            out=o_sb[:, nsplit:n, :], in_=ps[:, nsplit:n, :NC_N]
                        )
                    dmaq[cnt % 3].dma_start(
                        out=out_d[
                            mi * 128 : (mi + 1) * 128,
                            nq * NC_N : (nq + n) * NC_N,
                        ],
                        in_=o_sb[:, :n, :].rearrange("p g n -> p (g n)"),
                    )

                pending_ev.append(evict)

            while qready:
                mi0, nq0 = qready[0]
                n = 1
                while (
                    n < 4
                    and n < len(qready)
                    and qready[n] == (mi0, nq0 + n)
                    and (nq0 + n) % 4 != 0
                ):
                    n += 1
                emit_tail_group(n)
                if len(pending_ev) > 1:
                    pending_ev.pop(0)()
            flush_evicts()

            tailps_pool.__exit__(None, None, None)
            for pool in (
                outq_pool, lstm_pool,
            ):
                pool.__exit__(None, None, None)
    nc.compile()
    return nc


def _prep_inputs(inputs):
    bf = ml_dtypes.bfloat16
    target = np.asarray(inputs["target_tensor"])
    enc = np.asarray(inputs["encoder_outputs"], dtype=np.float32)
    lens = np.asarray(inputs["encoder_seq_lens"])
    h0 = np.asarray(inputs["h0"], dtype=np.float32)
    c0 = np.asarray(inputs["c0"], dtype=np.float32)
    emb = np.ascontiguousarray(np.asarray(inputs["emb"], dtype=np.float32))
    W_ih = np.asarray(inputs["W_ih"], dtype=np.float32)
    W_hh = np.asarray(inputs["W_hh"], dtype=np.float32)
    bias = (
        np.asarray(inputs["b_ih"], dtype=np.float32)
        + np.asarray(inputs["b_hh"], dtype=np.float32)
    )
    # gate order (i, f, g, o) -> (o, i, f, g); o/i/f rows x0.5 (tanh trick);
    # all W_hh entries x0.5 again because the device streams Z = 2h
    perm = np.concatenate(
        [np.arange(3 * H, 4 * H), np.arange(0, 2 * H), np.arange(2 * H, 3 * H)]
    )
    rs = np.concatenate([np.full(3 * H, 0.5, np.float32), np.ones(H, np.float32)])
    W_ih = W_ih[perm] * rs[:, None]
    W_hh = W_hh[perm] * rs[:, None] * 0.5
    bias = bias[perm] * rs
    W_lin = np.asarray(inputs["W_lin"], dtype=np.float32)
    b_lin = np.asarray(inputs["b_lin"], dtype=np.float32)

    wihT = np.ascontiguousarray(W_ih.T.astype(bf))                # (E, 4H)
    whhT = np.ascontiguousarray(
        W_hh.T.reshape(2, 128, G4H).astype(bf)
    )                                                             # (2,128,4H)
    biasT = np.ascontiguousarray(bias.reshape(NCH, 128).T)        # (128, NCH)
    # h-columns of W_lin x0.5 (Z = 2h); ctx columns unscaled
    wlinT_full = W_lin.T.copy()
    wlinT_full[:H] *= 0.5
    wlinT_full = wlinT_full.astype(bf)                            # (512, V)

    in_maps = []
    for i in range(NCORES):
        bg = i % NBG
        vh = i // NBG
        sl = slice(bg * BL, (bg + 1) * BL)
        vsl = slice(vh * VL, (vh + 1) * VL)
        # t-major flat index = t*BL + b, laid out [128, NMT]
        tok = np.ascontiguousarray(
            target[sl].T.reshape(BT).reshape(NMT, 128).T.astype(np.int32)
        )
        enc_i = enc[sl]                                           # (BL, S, H)
        enc_sbh = np.ascontiguousarray(
            enc_i.transpose(1, 0, 2).astype(bf)
        )                                                         # (S, BL, H)
        encT = np.ascontiguousarray(
            enc_i.transpose(2, 0, 1).reshape(2, 128, BL, S).astype(bf)
        )                                                         # (2,128,BL,S)
        # device state carries 2*h0 / 2*c0
        h0T = np.ascontiguousarray(
            (2.0 * h0[sl]).T.reshape(2, 128, BL).transpose(1, 0, 2)
        )
        c0T = np.ascontiguousarray(
            (2.0 * c0[sl]).T.reshape(2, 128, BL).transpose(1, 0, 2)
        )
        # additive mask (0 valid / -30000 masked), broadcast over TB
        m01 = (np.arange(S)[:, None] < lens[sl][None, :]).astype(np.float32)
        mask24 = np.ascontiguousarray(
            np.broadcast_to(
                ((m01 - 1.0) * 30000.0)[:, :, None], (S, BL, TB)
            ).astype(np.float32)
        )
        wlinT = np.ascontiguousarray(
            wlinT_full[:, vsl].reshape(4, 128, VL)
        )                                                         # (4,128,VL)
        in_maps.append(
            {
                "tok": tok,
                "emb": emb,
                "enc": enc_sbh,
                "encT": encT,
                "h0T": h0T,
                "c0T": c0T,
                "mask24": mask24,
                "biasT": biasT,
                "wihT": wihT,
                "whhT": whhT,
                "wlinT": wlinT,
            }
        )
    return in_maps, b_lin


LAST_RESULTS = None


def _install_ntff_shim():
    """Provide antenv.axon_hooks if the image's antenv lacks it, so
    trace=True/BASS_TRACE=1 can capture NTFF profiles under axon."""
    import sys
    import types

    try:
        from antenv.axon_hooks import get_axon_ntff_profile_hook  # noqa: F401

        return
    except ImportError:
        pass
    try:
        from trn_agent_boot.trn_boot import _ntff_profile_via_ctypes

        hook = _ntff_profile_via_ctypes("/opt/axon/libaxon_pjrt.so")
        m = types.ModuleType("antenv.axon_hooks")
        m.get_axon_ntff_profile_hook = lambda: hook
        m.set_axon_ntff_profile_hook = lambda h: None
        sys.modules["antenv.axon_hooks"] = m
    except Exception:
        pass


def kernel(**inputs):
    global LAST_RESULTS
    _install_ntff_shim()
    if "nc" not in _CACHE:
        _CACHE["nc"] = _build()
    nc = _CACHE["nc"]
    in_maps, b_lin = _prep_inputs(inputs)
    res = run_bass_kernel_spmd(nc, in_maps, core_ids=list(range(NCORES)))
    LAST_RESULTS = res
    out = np.empty((B, T, V), dtype=np.float32)
    for i in range(NCORES):
        bg = i % NBG
        vh = i // NBG
        vsl = slice(vh * VL, (vh + 1) * VL)
        # logits rows are (superblock, batch, t_in); reorder to (b, t)
        lg = (
            res.results[i]["logits"]
            .astype(np.float32)
            .reshape(NSB, BL, TB, VL)
            .transpose(1, 0, 2, 3)
            .reshape(BL, T, VL)
        )
        out[bg * BL : (bg + 1) * BL, :, vsl] = lg + b_lin[None, None, vsl]
    return out


# revision 56
# speedup vs baseline: 1.4407x; 1.0384x over previous
"""Trainium2 Bass kernel for DecoderAttnRNN (LSTM + attention decoder).

Sharding: hybrid over 8 cores = 4 batch-groups x 2 vocab-halves.
Each core handles 16 batches and 16000 vocab columns.

v4 design — fused pipeline:
  phase 0: embedding gather (deep-pipelined indirect DMA), transpose,
           x@W_ih.T+bias precompute split so steps 0-7 unblock early
  fused loop over 72 LSTM steps in 3 super-blocks of 24 steps:
    - LSTM recurrence using ONLY tanh (sigmoid folded via
      sig(x) = (tanh(x/2)+1)/2 with all x0.5/x2 rescales folded into the
      host-side weights), so attention's exp shares one ACT table set
    - after each super-block: attention for its 24 timesteps
    - logits quarter-groups (4 matmuls -> 500 bf16 cols -> DMA) for
      completed super-blocks are interleaved into the tensor-engine idle
      gaps of later LSTM steps; this also keeps the PE HAM-warm
  tail: remaining logits quarter-groups back-to-back
Output rows are in (superblock, batch, t_in) order; host reorders, upcasts
bf16 -> f32 and adds b_lin.

LSTM cell with stored state C = 2c, Z = 2h, gate order (o, i, f, g):
  t8 = tanh([psum + xw])        (o,i,f rows pre-scaled x0.5 on host)
  AB = (t8[i,f] + 1) * [t8[g] | C]   -> [A | B] = [2*sig_i*tanh_g | 4*sig_f*c]
  C' = 0.5*B + A                (= 2*c_new)
  th = tanh(0.5*C')             (= tanh(c_new))
  Z  = (t8[o] + 1) * th         (= 2*h_new; W_hh, scores-scale, W_lin
                                   h-columns absorb the factor 2)
"""

import numpy as np
import ml_dtypes

import concourse.bass as bass
import concourse.mybir as mybir
import concourse.tile as tile
from concourse import bacc
from concourse.bass_utils import run_bass_kernel_spmd
from concourse.masks import make_identity

B, T, S, E, H, V = 64, 72, 72, 128, 256, 32000
NCORES = 8
NBG = 4                   # batch groups
NVH = 2                   # vocab halves
BL = B // NBG             # 16 batches per core
BT = BL * T               # 1152
VL = V // NVH             # 16000 vocab cols per core
G4H = 4 * H               # 1024
NCH = G4H // 128          # 8 gate chunks of 128
NC_N = 500                # logits n-chunk (one PSUM bank)
NQG_N = VL // NC_N        # 32 n-chunks per m-tile
NMT = BT // 128           # 9 m-tiles
NSB = 3                   # super-blocks of the time axis
TB = T // NSB             # 24 steps per super-block
SBR = BL * TB             # 384 logits rows per super-block (= 3 m-tiles)
MPS = SBR // 128          # m-tiles per super-block

f32 = mybir.dt.float32
bf16 = mybir.dt.bfloat16
i32 = mybir.dt.int32

_CACHE = {}


def _build():
    nc = bacc.Bacc(None, target_bir_lowering=False)

    tok_d = nc.declare_dram_parameter("tok", [128, NMT], i32, isOutput=False)
    emb_d = nc.declare_dram_parameter("emb", [V, E], f32, isOutput=False)
    enc_d = nc.declare_dram_parameter("enc", [S, BL, H], bf16, isOutput=False)
    encT_d = nc.declare_dram_parameter("encT", [2, 128, BL, S], bf16, isOutput=False)
    h0T_d = nc.declare_dram_parameter("h0T", [128, 2, BL], f32, isOutput=False)
    c0T_d = nc.declare_dram_parameter("c0T", [128, 2, BL], f32, isOutput=False)
    mask_d = nc.declare_dram_parameter("mask24", [S, BL, TB], f32, isOutput=False)
    biasT_d = nc.declare_dram_parameter("biasT", [128, NCH], f32, isOutput=False)
    wihT_d = nc.declare_dram_parameter("wihT", [E, G4H], bf16, isOutput=False)
    whhT_d = nc.declare_dram_parameter("whhT", [2, 128, G4H], bf16, isOutput=False)
    wlinT_d = nc.declare_dram_parameter("wlinT", [4, 128, VL], bf16, isOutput=False)
    out_d = nc.declare_dram_parameter("logits", [BT, VL], bf16, isOutput=True)

    with tile.TileContext(nc) as tc:
        with tc.tile_pool(name="persist", bufs=1) as pp:
            # ---- setup DMAs: phase-0-critical loads first ----
            tok_sb = pp.tile([128, NMT], i32)
            nc.sync.dma_start(out=tok_sb[:], in_=tok_d[:])
            wih_sb = pp.tile([128, G4H], bf16)
            nc.scalar.dma_start(out=wih_sb[:], in_=wihT_d[:])
            biasT_sb = pp.tile([128, NCH], f32)
            nc.scalar.dma_start(out=biasT_sb[:], in_=biasT_d[:])
            whh_sb = pp.tile([128, 2, G4H], bf16)
            for k in range(2):
                nc.scalar.dma_start(out=whh_sb[:, k, :], in_=whhT_d[k])
            # tg8 holds the 8 tanh'd gate chunks (o,i,f,g) plus C=2c in
            # slots 8:10, so one fused op computes both cell products.
            # These tiny state loads ride the gpsimd queue so they cannot
            # queue behind the multi-MB weight streams.
            tg8 = pp.tile([128, 10, BL], f32)
            nc.gpsimd.dma_start(out=tg8[:, 8:10], in_=c0T_d[:])
            h0f = pp.tile([128, 2, BL], f32)
            nc.gpsimd.dma_start(out=h0f[:], in_=h0T_d[:])
            h_init = pp.tile([128, 2, BL], bf16)
            mask_sb = pp.tile([S, BL, TB], f32)
            nc.gpsimd.dma_start(out=mask_sb[:], in_=mask_d[:])

            ident = pp.tile([128, 128], f32)
            make_identity(nc, ident[:])

            xwT = pp.tile([128, T, NCH, BL], bf16)       # x@W_ih.T + bias
            z01 = pp.tile([128, 2, NSB, BL, TB], bf16)   # Z=2h (k-tiles 0,1)
            z23 = pp.tile([128, 2, NSB, BL, TB], bf16)   # ctx (k-tiles 2,3)
            x_allT = pp.tile([128, BT], bf16)

            # bulk loads, needed later; queued behind the critical ones
            encT_sb = pp.tile([128, 2, BL, S], bf16)
            for k in range(2):
                nc.sync.dma_start(out=encT_sb[:, k], in_=encT_d[k])
            enc_sb = pp.tile([S, BL, H], bf16)
            nc.scalar.dma_start(out=enc_sb[:], in_=enc_d[:])
            wpre = pp.tile([128, 4, VL], bf16)

            ones_col = pp.tile([S, 16], bf16)
            ones_row_f = pp.tile([1, 128], f32)
            nc.vector.memset(ones_col[:], 1.0)
            nc.vector.memset(ones_row_f[:], 1.0)

            # ---- phase 0: embedding gather + transpose (deep pipeline) ----
            with (
                tc.tile_pool(name="p0", bufs=NMT, space="SBUF") as wp,
                tc.tile_pool(name="p0ps", bufs=2, space="PSUM") as psp,
            ):
                for j in range(NMT):
                    x_t = wp.tile([128, E], f32, tag="x")
                    nc.gpsimd.indirect_dma_start(
                        out=x_t[:],
                        out_offset=None,
                        in_=emb_d[:],
                        in_offset=bass.IndirectOffsetOnAxis(
                            ap=tok_sb[:, j : j + 1], axis=0
                        ),
                    )
                    ps_t = psp.tile([128, 128], f32, tag="pst")
                    nc.tensor.transpose(out=ps_t[:], in_=x_t[:], identity=ident[:])
                    nc.vector.tensor_copy(
                        out=x_allT[:, j * 128 : (j + 1) * 128], in_=ps_t[:]
                    )

                # early xW for t<8 (x_allT cols 0:128) so the LSTM can start
                ps_xw8 = psp.tile([128, NCH, 128], f32, tag="psxw8")
                for c in range(NCH):
                    nc.tensor.matmul(
                        ps_xw8[:, c, :],
                        wih_sb[:, c * 128 : (c + 1) * 128],
                        x_allT[:, 0:128],
                        start=True,
                        stop=True,
                    )
                for c in range(NCH):
                    nc.vector.tensor_scalar(
                        out=xwT[:, 0:8, c, :],
                        in0=ps_xw8[:, c, :].rearrange("p (t b) -> p t b", b=BL),
                        scalar1=biasT_sb[:, c : c + 1],
                        scalar2=None,
                        op0=mybir.AluOpType.add,
                    )

            # h_init conversion deferred to here so the copy never blocks the
            # gather-cast pipeline at the head of the DVE queue
            nc.vector.tensor_copy(out=h_init[:], in_=h0f[:])

            # W_lin preload, gated behind the 5th gather-cast (the dummy
            # write creates the dependency) so the 16 MiB stream stays out
            # of the latency-critical early-phase-0 window; it has until
            # ~step 30 to finish
            for k in range(4):
                nc.vector.tensor_copy(
                    out=wpre[0:1, k, 0:1], in_=x_allT[0:1, BT - 1 : BT]
                )
                eng = nc.sync if k % 2 == 0 else nc.scalar
                eng.dma_start(out=wpre[:, k, :], in_=wlinT_d[k])

            def emit_xw_rest(c):
                # two 512-wide passes to keep PSUM small (32 t per pass)
                for half in range(2):
                    t0 = 8 + 32 * half
                    ps_xw = xwp.tile([128, 512], f32, tag="psxw")
                    nc.tensor.matmul(
                        ps_xw[:],
                        wih_sb[:, c * 128 : (c + 1) * 128],
                        x_allT[:, t0 * BL : (t0 + 32) * BL],
                        start=True,
                        stop=True,
                    )
                    nc.vector.tensor_scalar(
                        out=xwT[:, t0 : t0 + 32, c, :],
                        in0=ps_xw[:].rearrange("p (t b) -> p t b", b=BL),
                        scalar1=biasT_sb[:, c : c + 1],
                        scalar2=None,
                        op0=mybir.AluOpType.add,
                    )

            # ---- fused loop: LSTM steps + per-superblock attention +
            #      interleaved logits quarter-groups ----
            zt = [
                z01[:, 0].rearrange("p s b t -> p (s b t)"),
                z01[:, 1].rearrange("p s b t -> p (s b t)"),
                z23[:, 0].rearrange("p s b t -> p (s b t)"),
                z23[:, 1].rearrange("p s b t -> p (s b t)"),
            ]
            qready = []          # (mi, ng) logits quarter-groups ready to run
            qcount = [0]
            _attn_state = {}
            _apools = {}

            # SBUF pools first, PSUM pools inside, so every PSUM pool can be
            # released before the drain (which then gets all 8 banks)
            lstm_pool = tc.tile_pool(name="lstm", bufs=3)
            lp = lstm_pool.__enter__()
            outq_pool = tc.tile_pool(name="outq", bufs=4)
            oqp = outq_pool.__enter__()
            lstmps_pool = tc.tile_pool(name="lstmps", bufs=2, space="PSUM")
            lps = lstmps_pool.__enter__()
            outqps_pool = tc.tile_pool(name="outqps", bufs=2, space="PSUM")
            oqps = outqps_pool.__enter__()
            # innermost: remainder-xW PSUM, released once the attention pools
            # are needed (pools close in stack order)
            xw_pool = tc.tile_pool(name="pxw", bufs=2, space="PSUM")
            xwp = xw_pool.__enter__()
            dmaq = [nc.sync, nc.scalar, nc.gpsimd]

            pending_ev = []

            def emit_qgroup():
                # matmuls now; the eviction+DMA is deferred so the scheduler
                # gives the next LSTM step's chain ops priority over it
                if not qready:
                    return
                mi, nq = qready.pop(0)
                n0 = nq * NC_N
                ps_o = oqps.tile([128, 512], f32, tag="po")
                for k in range(4):
                    nc.tensor.matmul(
                        ps_o[:, :NC_N],
                        zt[k][:, mi * 128 : (mi + 1) * 128],
                        wpre[:, k, n0 : n0 + NC_N],
                        start=(k == 0),
                        stop=(k == 3),
                    )
                cnt = qcount[0]
                qcount[0] += 1

                def evict():
                    o_sb = oqp.tile([128, NC_N], bf16, tag="osb")
                    eng = nc.vector.tensor_copy if cnt % 2 == 0 else nc.scalar.copy
                    eng(out=o_sb[:], in_=ps_o[:, :NC_N])
                    dmaq[cnt % 3].dma_start(
                        out=out_d[mi * 128 : (mi + 1) * 128, n0 : n0 + NC_N],
                        in_=o_sb[:],
                    )

                pending_ev.append(evict)

            def flush_evicts():
                while pending_ev:
                    pending_ev.pop(0)()

            def emit_step(t):
                # offset covers the recent ~4 steps' worth of instructions:
                # enough to outrank concurrently-ready evicts/attention ops,
                # small enough that old evicts don't starve (they gate PSUM
                # reuse for the interleaved logits groups)
                with tc.high_priority(offset=150):
                    _emit_step_body(t)

            def _emit_step_body(t):
                sb, ti = divmod(t, TB)
                ps_g = lps.tile([128, NCH, BL], f32, tag="psg")
                for c in range(NCH):
                    for k in range(2):
                        rhs = (
                            h_init[:, k, :] if t == 0
                            else z01[:, k, (t - 1) // TB, :, (t - 1) % TB]
                        )
                        nc.tensor.matmul(
                            ps_g[:, c, :],
                            whh_sb[:, k, c * 128 : (c + 1) * 128],
                            rhs,
                            start=(k == 0),
                            stop=(k == 1),
                        )
                gsum = lp.tile([128, NCH, BL], f32, tag="gsum")
                nc.vector.tensor_tensor(
                    out=gsum[:], in0=ps_g[:], in1=xwT[:, t],
                    op=mybir.AluOpType.add,
                )
                # single tanh over all gates (o,i,f pre-scaled x0.5 on host)
                nc.scalar.activation(
                    out=tg8[:, 0:8], in_=gsum[:],
                    func=mybir.ActivationFunctionType.Tanh,
                )
                # AB = (t8[i,f]+1) * [t8[g] | C]
                ab = lp.tile([128, 4, BL], f32, tag="ab")
                nc.vector.scalar_tensor_tensor(
                    out=ab[:], in0=tg8[:, 2:6], scalar=1.0, in1=tg8[:, 6:10],
                    op0=mybir.AluOpType.add, op1=mybir.AluOpType.mult,
                )
                # C' = 0.5*B + A
                nc.vector.scalar_tensor_tensor(
                    out=tg8[:, 8:10], in0=ab[:, 2:4], scalar=0.5, in1=ab[:, 0:2],
                    op0=mybir.AluOpType.mult, op1=mybir.AluOpType.add,
                )
                th = lp.tile([128, 2, BL], f32, tag="th")
                nc.scalar.activation(
                    out=th[:], in_=tg8[:, 8:10],
                    func=mybir.ActivationFunctionType.Tanh, scale=0.5,
                )
                # Z = (t8[o]+1) * th  (bf16, = 2*h)
                nc.vector.scalar_tensor_tensor(
                    out=z01[:, :, sb, :, ti], in0=tg8[:, 0:2], scalar=1.0,
                    in1=th[:], op0=mybir.AluOpType.add, op1=mybir.AluOpType.mult,
                )

            def emit_attention_chunk(sb, step):
                # attention for super-block sb, split into 6 chunks emitted
                # across consecutive later steps to bound PE-queue delay
                atp = _apools["atp"]
                atps = _apools["atps"]
                if step == 0:
                    st = atps.tile([S, BL, 32], f32, tag="ps_s")
                    exb = atp.tile([S, BL, TB], bf16, tag="exb")
                    rcp = atp.tile([1, BL, TB], f32, tag="rcp")
                    att = atp.tile([S, BL, TB], bf16, tag="att")
                    den = atps.tile([16, 4, 128], f32, tag="den")
                    bc = atps.tile([128, 512], f32, tag="bc")
                    ctx = atps.tile([128, 2, 256], f32, tag="ctx")
                    _attn_state[sb] = (st, exb, rcp, att, den, bc, ctx)
                st, exb, rcp, att, den, bc, ctx = _attn_state[sb]
                if step in (0, 1):
                    for b in range(8 * step, 8 * (step + 1)):
                        for k in range(2):
                            nc.tensor.matmul(
                                st[:, b, :TB],
                                encT_sb[:, k, b, :],
                                z01[:, k, sb, b, :],
                                start=(k == 0),
                                stop=(k == 1),
                            )
                elif step == 2:
                    # masked exp over all (b,t) of the block; Z=2h so the
                    # score scale halves
                    nc.vector.tensor_tensor(
                        out=exb[:], in0=st[:, :, :TB], in1=mask_sb[:],
                        op=mybir.AluOpType.add,
                    )
                    nc.scalar.activation(
                        out=exb[:], in_=exb[:],
                        func=mybir.ActivationFunctionType.Exp,
                        scale=float(0.5 / np.sqrt(H)),
                    )
                elif step == 3:
                    for g in range(4):
                        nc.tensor.matmul(
                            den[:, g, : 4 * TB],
                            ones_col[:],
                            exb[:, 4 * g : 4 * (g + 1), :].rearrange(
                                "p b t -> p (b t)"
                            ),
                            start=True,
                            stop=True,
                        )
                        nc.vector.reciprocal(
                            out=rcp[:, 4 * g : 4 * (g + 1), :].rearrange(
                                "p b t -> p (b t)"
                            ),
                            in_=den[0:1, g, : 4 * TB],
                        )
                    nc.tensor.matmul(
                        bc[:, : BL * TB], ones_row_f[:],
                        rcp[:].rearrange("p b t -> p (b t)"),
                        start=True, stop=True,
                    )
                    nc.vector.tensor_tensor(
                        out=att[:].rearrange("p b t -> p (b t)"),
                        in0=exb[:].rearrange("p b t -> p (b t)"),
                        in1=bc[0:S, : BL * TB],
                        op=mybir.AluOpType.mult,
                    )
                elif step in (4, 5):
                    # half the batches per chunk; evict frees the PSUM tile
                    # for the second half (attnps has bufs=1)
                    b0 = 8 * (step - 4)
                    for b in range(b0, b0 + 8):
                        for j in range(2):
                            nc.tensor.matmul(
                                ctx[:, j, (b - b0) * TB : (b - b0 + 1) * TB],
                                enc_sb[:, b, j * 128 : (j + 1) * 128],
                                att[:, b, :],
                                start=True,
                                stop=True,
                            )
                    nc.vector.tensor_copy(
                        out=z23[:, :, sb, b0 : b0 + 8, :].rearrange(
                            "p k b t -> p k (b t)"
                        ),
                        in_=ctx[:, :, : 8 * TB],
                    )
                    if step == 5:
                        for mi in range(sb * MPS, (sb + 1) * MPS):
                            for nq in range(NQG_N):
                                qready.append((mi, nq))

            # ---- emit the fused schedule ----
            for t in range(T):
                if 1 <= t <= NCH:
                    emit_xw_rest(t - 1)
                if t == NCH + 1:
                    # xW PSUM freed; attention pools take its place
                    xw_pool.__exit__(None, None, None)
                    attn_pool = tc.tile_pool(name="attn", bufs=2)
                    attnps_pool = tc.tile_pool(name="attnps", bufs=1, space="PSUM")
                    _apools["atp"] = attn_pool.__enter__()
                    _apools["atps"] = attnps_pool.__enter__()
                emit_step(t)
                flush_evicts()
                sb_prev = t // TB - 1
                ph = t % TB
                if sb_prev >= 0 and ph < 6:
                    emit_attention_chunk(sb_prev, ph)
                    if t >= TB + 6:
                        # keep the PE fed through the attention chunks too
                        emit_qgroup()
                        emit_qgroup()
                elif t >= TB + 6:
                    emit_qgroup()
                    emit_qgroup()
                    emit_qgroup()
            # last super-block's attention; keep qgroups flowing so the PE
            # never idles past the HAM window during the transition
            for stp in range(6):
                emit_attention_chunk(NSB - 1, stp)
                emit_qgroup()
                emit_qgroup()
                emit_qgroup()
                flush_evicts()
            # drain: release ALL inner PSUM pools, then run 4-chunk groups
            # (16 matmuls per 8KB PSUM tile, DVE+ACT eviction, 1MB DMAs)
            attnps_pool.__exit__(None, None, None)
            attn_pool.__exit__(None, None, None)
            flush_evicts()
            outqps_pool.__exit__(None, None, None)
            lstmps_pool.__exit__(None, None, None)
            tailps_pool = tc.tile_pool(name="tailps", bufs=2, space="PSUM")
            tps = tailps_pool.__enter__()

            def emit_tail_group(n):
                mi, nq = qready[0]
                for x in range(n):
                    qready.pop(0)
                ps = tps.tile([128, 4, 512], f32, tag="tq")
                for idx in range(n):
                    for k in range(4):
                        nc.tensor.matmul(
                            ps[:, idx, :NC_N],
                            zt[k][:, mi * 128 : (mi + 1) * 128],
                            wpre[:, k, (nq + idx) * NC_N : (nq + idx + 1) * NC_N],
                            start=(k == 0),
                            stop=(k == 3),
                        )
                cnt = qcount[0]
                qcount[0] += 1
                nsplit = (n + 1) // 2

                def evict():
                    o_sb = oqp.tile([128, 4, NC_N], bf16, tag="osb4")
                    nc.vector.tensor_copy(
                        out=o_sb[:, 0:nsplit, :], in_=ps[:, 0:nsplit, :NC_N]
                    )
                    if n > nsplit:
                        nc.scalar.copy(
                            out=o_sb[:, nsplit:n, :], in_=ps[:, nsplit:n, :NC_N]
                        )
                    dmaq[cnt % 3].dma_start(
                        out=out_d[
                            mi * 128 : (mi + 1) * 128,
                            nq * NC_N : (nq + n) * NC_N,
                        ],
                        in_=o_sb[:, :n, :].rearrange("p g n -> p (g n)"),
                    )

                pending_ev.append(evict)

            while qready:
                mi0, nq0 = qready[0]
                n = 1
                while (
                    n < 4
                    and n < len(qready)
                    and qready[n] == (mi0, nq0 + n)
                    and (nq0 + n) % 4 != 0
                ):
                    n += 1
                emit_tail_group(n)
                if len(pending_ev) > 1:
                    pending_ev.pop(0)()
            flush_evicts()

            tailps_pool.__exit__(None, None, None)
            for pool in (
                outq_pool, lstm_pool,
            ):
                pool.__exit__(None, None, None)
    nc.compile()
    return nc


def _prep_inputs(inputs):
    bf = ml_dtypes.bfloat16
    target = np.asarray(inputs["target_tensor"])
    enc = np.asarray(inputs["encoder_outputs"], dtype=np.float32)
    lens = np.asarray(inputs["encoder_seq_lens"])
    h0 = np.asarray(inputs["h0"], dtype=np.float32)
    c0 = np.asarray(inputs["c0"], dtype=np.float32)
    emb = np.ascontiguousarray(np.asarray(inputs["emb"], dtype=np.float32))
    W_ih = np.asarray(inputs["W_ih"], dtype=np.float32)
    W_hh = np.asarray(inputs["W_hh"], dtype=np.float32)
    bias = (
        np.asarray(inputs["b_ih"], dtype=np.float32)
        + np.asarray(inputs["b_hh"], dtype=np.float32)
    )
    # gate order (i, f, g, o) -> (o, i, f, g); o/i/f rows x0.5 (tanh trick);
    # all W_hh entries x0.5 again because the device streams Z = 2h
    perm = np.concatenate(
        [np.arange(3 * H, 4 * H), np.arange(0, 2 * H), np.arange(2 * H, 3 * H)]
    )
    rs = np.concatenate([np.full(3 * H, 0.5, np.float32), np.ones(H, np.float32)])
    W_ih = W_ih[perm] * rs[:, None]
    W_hh = W_hh[perm] * rs[:, None] * 0.5
    bias = bias[perm] * rs
    W_lin = np.asarray(inputs["W_lin"], dtype=np.float32)
    b_lin = np.asarray(inputs["b_lin"], dtype=np.float32)

    wihT = np.ascontiguousarray(W_ih.T.astype(bf))                # (E, 4H)
    whhT = np.ascontiguousarray(
        W_hh.T.reshape(2, 128, G4H).astype(bf)
    )                                                             # (2,128,4H)
    biasT = np.ascontiguousarray(bias.reshape(NCH, 128).T)        # (128, NCH)
    # h-columns of W_lin x0.5 (Z = 2h); ctx columns unscaled
    wlinT_full = W_lin.T.copy()
    wlinT_full[:H] *= 0.5
    wlinT_full = wlinT_full.astype(bf)                            # (512, V)

    in_maps = []
    for i in range(NCORES):
        bg = i % NBG
        vh = i // NBG
        sl = slice(bg * BL, (bg + 1) * BL)
        vsl = slice(vh * VL, (vh + 1) * VL)
        # t-major flat index = t*BL + b, laid out [128, NMT]
        tok = np.ascontiguousarray(
            target[sl].T.reshape(BT).reshape(NMT, 128).T.astype(np.int32)
        )
        enc_i = enc[sl]                                           # (BL, S, H)
        enc_sbh = np.ascontiguousarray(
            enc_i.transpose(1, 0, 2).astype(bf)
        )                                                         # (S, BL, H)
        encT = np.ascontiguousarray(
            enc_i.transpose(2, 0, 1).reshape(2, 128, BL, S).astype(bf)
        )                                                         # (2,128,BL,S)
        # device state carries 2*h0 / 2*c0
        h0T = np.ascontiguousarray(
            (2.0 * h0[sl]).T.reshape(2, 128, BL).transpose(1, 0, 2)
        )
        c0T = np.ascontiguousarray(
            (2.0 * c0[sl]).T.reshape(2, 128, BL).transpose(1, 0, 2)
        )
        # additive mask (0 valid / -30000 masked), broadcast over TB
        m01 = (np.arange(S)[:, None] < lens[sl][None, :]).astype(np.float32)
        mask24 = np.ascontiguousarray(
            np.broadcast_to(
                ((m01 - 1.0) * 30000.0)[:, :, None], (S, BL, TB)
            ).astype(np.float32)
        )
        wlinT = np.ascontiguousarray(
            wlinT_full[:, vsl].reshape(4, 128, VL)
        )                                                         # (4,128,VL)
        in_maps.append(
            {
                "tok": tok,
                "emb": emb,
                "enc": enc_sbh,
                "encT": encT,
                "h0T": h0T,
                "c0T": c0T,
                "mask24": mask24,
                "biasT": biasT,
                "wihT": wihT,
                "whhT": whhT,
                "wlinT": wlinT,
            }
        )
    return in_maps, b_lin


LAST_RESULTS = None


def _install_ntff_shim():
    """Provide antenv.axon_hooks if the image's antenv lacks it, so
    trace=True/BASS_TRACE=1 can capture NTFF profiles under axon."""
    import sys
    import types

    try:
        from antenv.axon_hooks import get_axon_ntff_profile_hook  # noqa: F401

        return
    except ImportError:
        pass
    try:
        from trn_agent_boot.trn_boot import _ntff_profile_via_ctypes

        hook = _ntff_profile_via_ctypes("/opt/axon/libaxon_pjrt.so")
        m = types.ModuleType("antenv.axon_hooks")
        m.get_axon_ntff_profile_hook = lambda: hook
        m.set_axon_ntff_profile_hook = lambda h: None
        sys.modules["antenv.axon_hooks"] = m
    except Exception:
        pass


def kernel(**inputs):
    global LAST_RESULTS
    _install_ntff_shim()
    if "nc" not in _CACHE:
        _CACHE["nc"] = _build()
    nc = _CACHE["nc"]
    in_maps, b_lin = _prep_inputs(inputs)
    res = run_bass_kernel_spmd(nc, in_maps, core_ids=list(range(NCORES)))
    LAST_RESULTS = res
    out = np.empty((B, T, V), dtype=np.float32)
    for i in range(NCORES):
        bg = i % NBG
        vh = i // NBG
        vsl = slice(vh * VL, (vh + 1) * VL)
        # logits rows are (superblock, batch, t_in); reorder to (b, t)
        lg = (
            res.results[i]["logits"]
            .astype(np.float32)
            .reshape(NSB, BL, TB, VL)
            .transpose(1, 0, 2, 3)
            .reshape(BL, T, VL)
        )
        out[bg * BL : (bg + 1) * BL, :, vsl] = lg + b_lin[None, None, vsl]
    return out
